# revision 1
# baseline (speedup 1.0000x reference)
"""DCT Frequency Splitter — Trainium2 Bass kernel.

Math: FFT2 -> mask -> IFFT2 -> real is a linear operator on the 196 patch
tokens (per channel).  low_sp = A @ patches with A = Re(Finv diag(m) F)
(196x196, real, built on host from the 4 mask params).  Since
high_mask = 1 - mask(high_params):  high_sp = patches - C @ patches with
C = A when low/high params coincide (the common case; then one matmul
feeds both outputs).  The token-mean for the gate MLP is obtained for free
by stacking a ones/196 row onto A, so the whole FFT pipeline plus gate is
a single [197,196] x [196,768] matmul per image plus a tiny batched MLP.

Sharding: pure data parallel, batch 128 -> 16 per core across 8 cores.
"""

import os
import numpy as np

import concourse.bass as bass
import concourse.bacc as bacc_mod
import concourse.mybir as mybir
import concourse.tile as tile
from concourse.bass_utils import run_bass_kernel_spmd
from concourse.tile_rust import add_dep_helper

H, W = 14, 14
B, N, D = 128, 197, 768
P = 196  # patch tokens
NCORES = 8
BS = B // NCORES  # batches per core

# tunables (env overridable for experiments)
GRP = int(os.environ.get("KRN_GROUP", "3"))       # gate MLP group size
MM_DT = os.environ.get("KRN_MM_DT", "f32")         # f32 | f32r
BUFX = int(os.environ.get("KRN_BUFX", "12"))
BUFO = int(os.environ.get("KRN_BUFO", "10"))
DBG_NOGATE = bool(int(os.environ.get("KRN_NOGATE", "0")))
DBG_DMAONLY = bool(int(os.environ.get("KRN_DMAONLY", "0")))
F32 = mybir.dt.float32


def _freq_mask_np(params, low):
    ch, cw, radius, sharp = [np.float64(v) for v in np.asarray(params)]
    y = np.arange(H, dtype=np.float64)
    x = np.arange(W, dtype=np.float64)
    d2 = (y[:, None] - ch) ** 2 + (x[None, :] - cw) ** 2
    dist = np.sqrt(d2 + 1e-12)
    s = np.clip(sharp, 0.5, 10.0)
    r = np.clip(radius, 1.0, min(H, W) / 2.0)
    m = np.exp(-((dist / r) ** s))
    return m if low else 1.0 - m


def _conv_operator(mask):
    """Real 196x196 operator equivalent to ifft2(fft2(img)*mask).real."""
    F_H = np.exp(-2j * np.pi * np.outer(np.arange(H), np.arange(H)) / H)
    F_W = np.exp(-2j * np.pi * np.outer(np.arange(W), np.arange(W)) / W)
    Fi_H = np.conj(F_H) / H
    Fi_W = np.conj(F_W) / W
    op = np.kron(Fi_H, Fi_W) @ np.diag(mask.ravel()) @ np.kron(F_H, F_W)
    return np.real(op)


def _mm_ap(ap):
    if MM_DT == "f32r":
        return ap.bitcast(mybir.dt.float32r)
    return ap


def _build_program(consts, share_Y, b2lo, b2hi, alo, ahi):
    nc = bacc_mod.Bacc(None)

    xs_h = nc.dram_tensor("xs", [BS, N, D], F32, kind="ExternalInput")
    lo_h = nc.dram_tensor("lo", [BS, N, D], F32, kind="ExternalOutput")
    hi_h = nc.dram_tensor("hi", [BS, N, D], F32, kind="ExternalOutput")

    ch = {k: nc.inline_tensor(v, name=f"c_{k}") for k, v in consts.items()}

    Copy = mybir.ActivationFunctionType.Copy
    Relu = mybir.ActivationFunctionType.Relu
    Sig = mybir.ActivationFunctionType.Sigmoid

    with tile.TileContext(nc) as tc:
        with (
            tc.tile_pool(name="consts", bufs=1) as cp,
            tc.tile_pool(name="xp", bufs=BUFX) as xp,
            tc.tile_pool(name="outp", bufs=BUFO) as outp,
            tc.tile_pool(name="gp", bufs=2) as gp,
            tc.tile_pool(name="pm", bufs=(3 if share_Y else 2), space="PSUM") as pm,
            tc.tile_pool(name="pmz", bufs=1, space="PSUM") as pmz,
            tc.tile_pool(name="par", bufs=(2 if share_Y else 1), space="PSUM") as par,
        ):
            # ---- load constants to SBUF
            def cload(key):
                arr = consts[key]
                t = cp.tile(list(arr.shape), F32, tag=key)
                nc.sync.dma_start(out=t[:], in_=ch[key][...])
                return t

            # matmul-critical weights in ONE blob DMA so the first batch's
            # matmuls queue behind a single descriptor slot; everything the
            # gate MLP needs is a second blob deferred until after the first
            # group's data loads (first use is one group later)
            wb = cload("wtblob")        # [128, 410]: wt_lo | wt_hi | ident
            wt_lo = wb[:, 0:197]        # M'^T rows 0:128 (tokens 0..127)
            wt_hi = wb[0:69, 197:394]   # M'^T rows 128:197
            ident = wb[0:16, 394:410]
            if not share_Y:
                ct_lo = cload("ct_lo")  # [128, 197]
                ct_hi = cload("ct_hi")  # [69, 197]
            gate_consts = {}

            def load_deferred():
                gb = cload("gblob")     # [128, 1620] packed gate constants
                gate_consts["w1c"] = gb[:, 0:1152].rearrange(
                    "p (a b) -> p a b", a=6)             # [128, 6, 192]
                gate_consts["b1c"] = gb[0:1, 1152:1344]  # [1, 192]
                gate_consts["w2c0"] = gb[:, 1344:1346]   # [128, 2]
                gate_consts["w2c1"] = gb[0:64, 1346:1348]  # [64, 2]
                gate_consts["ones1"] = gb[0:1, 1348:1364]  # [1, 16]
                gate_consts["alr"] = gb[0:1, 1364:1492]  # [1,128] sig(alpha_low)
                gate_consts["ahr"] = gb[0:1, 1492:1620]  # [1,128] sig(alpha_high)
                # CLS passthrough for all batches in two strided DMAs
                nc.sync.dma_start(out=lo_h[:, 0:1, :], in_=xs_h[:, 0:1, :])
                nc.sync.dma_start(out=hi_h[:, 0:1, :], in_=xs_h[:, 0:1, :])

            n_groups = (BS + GRP - 1) // GRP
            NSPLIT = [(0, 512), (512, 768)]

            def gate_and_store(bs, Gn, arena, per_b):
                """Gate MLP + scales + stores for a finished group, traced
                one group late so the chain hides behind the next group's
                matmul stream."""
                # token means (lo_a row 0) -> transposed gT columns
                for j, b in enumerate(bs):
                    lo_a = per_b[b][2]
                    for c in range(6):
                        nc.tensor.transpose(
                            arena[:, c * 16 + j:c * 16 + j + 1],
                            lo_a[0:1, c * 128:(c + 1) * 128],
                            ident[0:1, 0:1])
                gT = gp.tile([128, 6, 16], F32, tag="gT")
                nc.vector.tensor_copy(
                    gT[:].rearrange("p a b -> p (a b)"), arena[:, 0:96])

                h_ps = arena[0:16, 96:288]
                for c in range(6):
                    nc.tensor.matmul(h_ps[0:Gn, :], _mm_ap(gT[:, c, 0:Gn]),
                                     _mm_ap(gate_consts["w1c"][:, c, :]), start=(c == 0),
                                     stop=False)
                nc.tensor.matmul(h_ps[0:Gn, :], _mm_ap(gate_consts["ones1"][0:1, 0:Gn]),
                                 _mm_ap(gate_consts["b1c"][0:1, :]), start=False, stop=True)
                hs = gp.tile([16, 192], F32, tag="hs")
                nc.vector.tensor_relu(hs[0:Gn, :], h_ps[0:Gn, :])

                hT = gp.tile([128, 2, 16], F32, tag="hT")
                nc.tensor.transpose(arena[:, 288:288 + Gn], hs[0:Gn, 0:128],
                                    ident[0:Gn, 0:Gn])
                nc.tensor.transpose(arena[0:64, 304:304 + Gn],
                                    hs[0:Gn, 128:192], ident[0:Gn, 0:Gn])
                nc.vector.tensor_copy(hT[:].rearrange("p a b -> p (a b)"),
                                      arena[:, 288:320])

                # final layer: two M=1 matmuls (gate rows at partition 0);
                # b2 folds into the sigmoid bias, alpha into a post-scale;
                # rows then replicated across partitions via K=1 matmuls
                crows = []
                for col, b2f, af in ((0, b2lo, alo), (1, b2hi, ahi)):
                    g_ps = arena[0:1, 320 + 16 * col:336 + 16 * col]
                    nc.tensor.matmul(g_ps[:, 0:Gn], _mm_ap(gate_consts["w2c0"][:, col:col + 1]),
                                     _mm_ap(hT[:, 0, 0:Gn]), start=True,
                                     stop=False)
                    nc.tensor.matmul(g_ps[:, 0:Gn], _mm_ap(gate_consts["w2c1"][:, col:col + 1]),
                                     _mm_ap(hT[0:64, 1, 0:Gn]), start=False,
                                     stop=True)
                    cr = gp.tile([1, 16], F32, tag=f"crow{col}")
                    nc.scalar.activation(cr[:, 0:Gn], g_ps[:, 0:Gn], Sig,
                                         bias=b2f)
                    crows.append(cr)
                # replication matmuls against alpha-scaled ones rows fold the
                # alpha multiply in; one copy lands both gate vectors
                for col, wrow in ((0, "alr"), (1, "ahr")):
                    nc.tensor.matmul(
                        arena[:, 352 + 16 * col:352 + 16 * col + Gn],
                        _mm_ap(gate_consts[wrow][0:1, :]),
                        _mm_ap(crows[col][0:1, 0:Gn]),
                        start=True, stop=True)
                crlh = gp.tile([128, 32], F32, tag="crlh")
                nc.vector.tensor_copy(crlh[:], arena[:, 352:384])
                crl = crlh[:, 0:16]
                crh = crlh[:, 16:32]

                # scale in place and store (hi lives in the x tiles)
                for j, b in enumerate(bs):
                    xa, xb, lo_a, lo_b = per_b[b]
                    nc.scalar.activation(lo_a[:], lo_a[:], Copy,
                                         scale=crl[:, j:j + 1])
                    nc.scalar.activation(lo_b[:], lo_b[:], Copy,
                                         scale=crl[0:69, j:j + 1])
                    nc.vector.tensor_scalar_mul(xa[:], xa[:],
                                                crh[:, j:j + 1])
                    nc.vector.tensor_scalar_mul(xb[:], xb[:], crh[0:69, j:j + 1])
                    nc.sync.dma_start(out=lo_h[b, 1:128, :], in_=lo_a[1:128])
                    nc.sync.dma_start(out=lo_h[b, 128:197, :], in_=lo_b[:])
                    nc.sync.dma_start(out=hi_h[b, 1:128, :], in_=xa[1:128])
                    nc.sync.dma_start(out=hi_h[b, 128:197, :], in_=xb[:])

            pending = None   # (bs, Gn, arena, per_b) of previous group
            for g in range(n_groups):
                bs = list(range(g * GRP, min((g + 1) * GRP, BS)))
                Gn = len(bs)
                # per-group psum arena for the gate pipeline (fresh column
                # ranges): 0:96 gT | 96:288 h | 288:320 hT | 320:352 gate |
                # 352:384 replication
                arena = par.tile([128, 512], F32, tag="arena")
                per_b = {}

                # issue the whole group's loads first so they sit AHEAD of
                # the previous group's store burst in the DMA queue FIFOs
                xt = {}
                for b in bs:
                    xa = xp.tile([128, D], F32, tag="xa")
                    xb = xp.tile([69, D], F32, tag="xb")
                    nc.sync.dma_start(out=xa[:], in_=xs_h[b, 0:128, :])
                    nc.sync.dma_start(out=xb[:], in_=xs_h[b, 128:197, :])
                    xt[b] = (xa, xb)
                if g == 0:
                    load_deferred()

                for j, b in enumerate(bs):
                    xa, xb = xt[b]

                    # Y = M' @ x[b]; M' row 0 = token-mean row, rows 1..196
                    # = low-pass operator (CLS column is zero)
                    ylo = pm.tile([128, D], F32, tag="ym")
                    yhi = pm.tile([128, D], F32, tag="ym")
                    for (n0, n1) in NSPLIT:
                        nc.tensor.matmul(ylo[:, n0:n1], _mm_ap(wt_lo[:, 0:128]),
                                         _mm_ap(xa[:, n0:n1]), start=True, stop=False)
                        nc.tensor.matmul(ylo[:, n0:n1], _mm_ap(wt_hi[:, 0:128]),
                                         _mm_ap(xb[:, n0:n1]), start=False, stop=True)
                    for (n0, n1) in NSPLIT:
                        nc.tensor.matmul(yhi[0:69, n0:n1], _mm_ap(wt_lo[:, 128:197]),
                                         _mm_ap(xa[:, n0:n1]), start=True, stop=False)
                        nc.tensor.matmul(yhi[0:69, n0:n1], _mm_ap(wt_hi[:, 128:197]),
                                         _mm_ap(xb[:, n0:n1]), start=False, stop=True)

                    if share_Y:
                        zlo, zhi = ylo, yhi
                    else:
                        zlo = pmz.tile([128, D], F32, tag="zm")
                        zhi = pmz.tile([128, D], F32, tag="zm")
                        for (n0, n1) in NSPLIT:
                            nc.tensor.matmul(zlo[:, n0:n1], _mm_ap(ct_lo[:, 0:128]),
                                             _mm_ap(xa[:, n0:n1]), start=True, stop=False)
                            nc.tensor.matmul(zlo[:, n0:n1], _mm_ap(ct_hi[:, 0:128]),
                                             _mm_ap(xb[:, n0:n1]), start=False, stop=True)
                        for (n0, n1) in NSPLIT:
                            nc.tensor.matmul(zhi[0:69, n0:n1], _mm_ap(ct_lo[:, 128:197]),
                                             _mm_ap(xa[:, n0:n1]), start=True, stop=False)
                            nc.tensor.matmul(zhi[0:69, n0:n1], _mm_ap(ct_hi[:, 128:197]),
                                             _mm_ap(xb[:, n0:n1]), start=False, stop=True)

                    # PSUM -> SBUF (unscaled); hi = x - Y in place in x
                    # tiles; consumed per N-chunk so psum frees sooner
                    lo_a = outp.tile([128, D], F32, tag="lo_a")
                    lo_b = outp.tile([69, D], F32, tag="lo_b")
                    for (n0, n1) in NSPLIT:
                        nc.scalar.activation(lo_a[:, n0:n1], ylo[:, n0:n1], Copy)
                        nc.vector.tensor_sub(xa[:, n0:n1], xa[:, n0:n1],
                                             zlo[:, n0:n1])
                    for (n0, n1) in NSPLIT:
                        nc.scalar.activation(lo_b[:, n0:n1], yhi[0:69, n0:n1],
                                             Copy)
                        nc.vector.tensor_sub(xb[:, n0:n1], xb[:, n0:n1],
                                             zhi[0:69, n0:n1])

                    per_b[b] = (xa, xb, lo_a, lo_b)

                if share_Y:
                    # lag the gate chain one group to hide its latency
                    if pending is not None:
                        gate_and_store(*pending)
                    pending = (bs, Gn, arena, per_b)
                else:
                    # generic path: flush immediately (simpler dependency
                    # structure; correctness over overlap)
                    gate_and_store(bs, Gn, arena, per_b)

            if pending is not None:
                gate_and_store(*pending)
    if not nc.is_finalized():
        nc.finalize()
    return nc


def kernel(x, low_params, high_params, alpha_low, alpha_high,
           w1, b1, w2, b2, cls_token_idx):
    assert int(cls_token_idx) == 0
    x = np.ascontiguousarray(np.asarray(x, dtype=np.float32))
    assert x.shape == (B, N, D)

    lm = _freq_mask_np(low_params, True)
    A = _conv_operator(lm)                       # low operator [196, 196]
    share_Y = np.allclose(np.asarray(low_params, np.float32),
                          np.asarray(high_params, np.float32))
    Cm = A if share_Y else _conv_operator(_freq_mask_np(high_params, True))

    w1 = np.asarray(w1, np.float32)
    sig = lambda v: 1.0 / (1.0 + np.exp(-np.float64(v)))

    def make_consts(OP):
        # M' [197,197]: row 0 = token-mean row, rows 1..196 = OP; CLS col 0
        Mfull = np.zeros((N, N), np.float64)
        Mfull[0, 1:] = 1.0 / P
        Mfull[1:, 1:] = OP
        WT = np.ascontiguousarray(Mfull.T).astype(np.float32)
        wtblob = np.zeros((128, 410), np.float32)
        wtblob[:, 0:197] = WT[0:128]
        wtblob[0:69, 197:394] = WT[128:197]
        wtblob[0:16, 394:410] = np.eye(16, dtype=np.float32)
        gblob = np.zeros((128, 1620), np.float32)
        gblob[:, 0:1152] = w1.reshape(6, 128, 192).transpose(1, 0, 2).reshape(128, 1152)
        gblob[0, 1152:1344] = np.asarray(b1, np.float32)
        gblob[:, 1344:1346] = np.asarray(w2, np.float32)[0:128]
        gblob[0:64, 1346:1348] = np.asarray(w2, np.float32)[128:192]
        gblob[0, 1348:1364] = 1.0
        gblob[0, 1364:1492] = sig(alpha_low)
        gblob[0, 1492:1620] = sig(alpha_high)
        return {"wtblob": wtblob, "gblob": gblob}

    b2v = np.asarray(b2, np.float64).reshape(2)

    def run_once(consts):
        nc = _build_program(consts, True,
                            b2lo=float(b2v[0]), b2hi=float(b2v[1]),
                            alo=float(sig(alpha_low)), ahi=float(sig(alpha_high)))
        xs = x.reshape(NCORES, BS, N, D)
        in_maps = [{"xs": np.ascontiguousarray(xs[c])} for c in range(NCORES)]
        want_trace = bool(int(os.environ.get("KRN_TRACE", "0")))
        try:
            res = run_bass_kernel_spmd(nc, in_maps, core_ids=list(range(NCORES)),
                                       trace=want_trace)
        except ModuleNotFoundError:
            res = run_bass_kernel_spmd(nc, in_maps, core_ids=list(range(NCORES)))
        lo = np.concatenate([r["lo"] for r in res.results], axis=0)
        hi = np.concatenate([r["hi"] for r in res.results], axis=0)
        if getattr(res, "exec_time_ns", None) is not None:
            print(f"HW exec time: {res.exec_time_ns} ns")
        return lo, hi

    if share_Y:
        return run_once(make_consts(A))
    # generic case (never hit by the reference inputs): two passes of the
    # validated single-operator program — lo from the A pass, hi from the C
    # pass (the gate depends only on x, so it is identical in both)
    lo, _ = run_once(make_consts(A))
    _, hi = run_once(make_consts(Cm))
    return lo, hi



# revision 13
# speedup vs baseline: 1.1988x; 1.1988x over previous
"""DCT Frequency Splitter — Trainium2 Bass kernel.

Math: FFT2 -> mask -> IFFT2 -> real is a linear operator on the 196 patch
tokens (per channel).  low_sp = A @ patches with A = Re(Finv diag(m) F)
(196x196, real, built on host from the 4 mask params).  Since
high_mask = 1 - mask(high_params):  high_sp = patches - C @ patches with
C = A when low/high params coincide (the common case; then one matmul
feeds both outputs).  The token-mean for the gate MLP is obtained for free
by stacking a ones/196 row onto A, so the whole FFT pipeline plus gate is
a single [197,196] x [196,768] matmul per image plus a tiny batched MLP.

Sharding: pure data parallel, batch 128 -> 16 per core across 8 cores.
"""

import os
import numpy as np

import concourse.bass as bass
import concourse.bacc as bacc_mod
import concourse.mybir as mybir
import concourse.tile as tile
from concourse.bass_utils import run_bass_kernel_spmd
from concourse.tile_rust import add_dep_helper

H, W = 14, 14
B, N, D = 128, 197, 768
P = 196  # patch tokens
NCORES = 8
BS = B // NCORES  # batches per core

# tunables (env overridable for experiments)
GRP = int(os.environ.get("KRN_GROUP", "3"))       # gate MLP group size
MM_DT = os.environ.get("KRN_MM_DT", "f32")         # f32 | f32r (gate matmuls)
YR = bool(int(os.environ.get("KRN_YR", "1")))      # f32r for the big Y matmuls
BUFX = int(os.environ.get("KRN_BUFX", "12"))
BUFO = int(os.environ.get("KRN_BUFO", "10"))
DBG_NOGATE = bool(int(os.environ.get("KRN_NOGATE", "0")))
DBG_DMAONLY = bool(int(os.environ.get("KRN_DMAONLY", "0")))
F32 = mybir.dt.float32
F32R = mybir.dt.float32r
YDT = F32R if YR else F32


def _freq_mask_np(params, low):
    ch, cw, radius, sharp = [np.float64(v) for v in np.asarray(params)]
    y = np.arange(H, dtype=np.float64)
    x = np.arange(W, dtype=np.float64)
    d2 = (y[:, None] - ch) ** 2 + (x[None, :] - cw) ** 2
    dist = np.sqrt(d2 + 1e-12)
    s = np.clip(sharp, 0.5, 10.0)
    r = np.clip(radius, 1.0, min(H, W) / 2.0)
    m = np.exp(-((dist / r) ** s))
    return m if low else 1.0 - m


def _conv_operator(mask):
    """Real 196x196 operator equivalent to ifft2(fft2(img)*mask).real."""
    F_H = np.exp(-2j * np.pi * np.outer(np.arange(H), np.arange(H)) / H)
    F_W = np.exp(-2j * np.pi * np.outer(np.arange(W), np.arange(W)) / W)
    Fi_H = np.conj(F_H) / H
    Fi_W = np.conj(F_W) / W
    op = np.kron(Fi_H, Fi_W) @ np.diag(mask.ravel()) @ np.kron(F_H, F_W)
    return np.real(op)


def _mm_ap(ap):
    if MM_DT == "f32r":
        return ap.bitcast(mybir.dt.float32r)
    return ap


def _build_program(consts, share_Y, b2lo, b2hi, alo, ahi):
    nc = bacc_mod.Bacc(None)

    xs_h = nc.dram_tensor("xs", [BS, N, D], F32, kind="ExternalInput")
    lo_h = nc.dram_tensor("lo", [BS, N, D], F32, kind="ExternalOutput")
    hi_h = nc.dram_tensor("hi", [BS, N, D], F32, kind="ExternalOutput")

    ch = {k: nc.inline_tensor(v, name=f"c_{k}") for k, v in consts.items()}

    Copy = mybir.ActivationFunctionType.Copy
    Relu = mybir.ActivationFunctionType.Relu
    Sig = mybir.ActivationFunctionType.Sigmoid

    with tile.TileContext(nc) as tc:
        with (
            tc.tile_pool(name="consts", bufs=1) as cp,
            tc.tile_pool(name="xp", bufs=BUFX) as xp,
            tc.tile_pool(name="outp", bufs=BUFO) as outp,
            tc.tile_pool(name="gp", bufs=2) as gp,
            tc.tile_pool(name="pm", bufs=(3 if share_Y else 2), space="PSUM") as pm,
            tc.tile_pool(name="pmz", bufs=1, space="PSUM") as pmz,
            tc.tile_pool(name="par", bufs=(2 if share_Y else 1), space="PSUM") as par,
        ):
            # ---- load constants to SBUF
            def cload(key, dtype=F32):
                arr = consts[key]
                t = cp.tile(list(arr.shape), dtype, tag=key)
                src = ch[key][...]
                if dtype != F32:
                    src = src.bitcast(dtype)
                nc.sync.dma_start(out=t[:], in_=src)
                return t

            # matmul-critical weights in ONE blob DMA so the first batch's
            # matmuls queue behind a single descriptor slot; everything the
            # gate MLP needs is a second blob deferred until after the first
            # group's data loads (first use is one group later)
            wb = cload("wtblob", YDT)   # [128, 410]: wt_lo | wt_hi | ident
            wt_lo = wb[:, 0:197]        # M'^T rows 0:128 (tokens 0..127)
            wt_hi = wb[0:69, 197:394]   # M'^T rows 128:197
            ident = wb[0:16, 394:410].bitcast(F32)
            if not share_Y:
                ct_lo = cload("ct_lo", YDT)  # [128, 197]
                ct_hi = cload("ct_hi", YDT)  # [69, 197]
            gate_consts = {}

            def load_deferred():
                gb = cload("gblob")     # [128, 1620] packed gate constants
                gate_consts["w1c"] = gb[:, 0:1152].rearrange(
                    "p (a b) -> p a b", a=6)             # [128, 6, 192]
                gate_consts["b1c"] = gb[0:1, 1152:1344]  # [1, 192]
                gate_consts["w2c0"] = gb[:, 1344:1346]   # [128, 2]
                gate_consts["w2c1"] = gb[0:64, 1346:1348]  # [64, 2]
                gate_consts["ones1"] = gb[0:1, 1348:1364]  # [1, 16]
                gate_consts["alr"] = gb[0:1, 1364:1492]  # [1,128] sig(alpha_low)
                gate_consts["ahr"] = gb[0:1, 1492:1620]  # [1,128] sig(alpha_high)
                # CLS passthrough for all batches in two strided DMAs
                nc.sync.dma_start(out=lo_h[:, 0:1, :], in_=xs_h[:, 0:1, :])
                nc.sync.dma_start(out=hi_h[:, 0:1, :], in_=xs_h[:, 0:1, :])

            n_groups = (BS + GRP - 1) // GRP
            NSPLIT = [(0, 512), (512, 768)]

            def gate_and_store(bs, Gn, arena, per_b):
                """Gate MLP + scales + stores for a finished group, traced
                one group late so the chain hides behind the next group's
                matmul stream."""
                # token means (lo_a row 0) -> transposed gT columns
                for j, b in enumerate(bs):
                    lo_a = per_b[b][2]
                    for c in range(6):
                        nc.tensor.transpose(
                            arena[:, c * 16 + j:c * 16 + j + 1],
                            lo_a[0:1, c * 128:(c + 1) * 128],
                            ident[0:1, 0:1])
                gT = gp.tile([128, 6, 16], F32, tag="gT")
                nc.vector.tensor_copy(
                    gT[:].rearrange("p a b -> p (a b)"), arena[:, 0:96])

                h_ps = arena[0:16, 96:288]
                for c in range(6):
                    nc.tensor.matmul(h_ps[0:Gn, :], _mm_ap(gT[:, c, 0:Gn]),
                                     _mm_ap(gate_consts["w1c"][:, c, :]), start=(c == 0),
                                     stop=False)
                nc.tensor.matmul(h_ps[0:Gn, :], _mm_ap(gate_consts["ones1"][0:1, 0:Gn]),
                                 _mm_ap(gate_consts["b1c"][0:1, :]), start=False, stop=True)
                hs = gp.tile([16, 192], F32, tag="hs")
                nc.vector.tensor_relu(hs[0:Gn, :], h_ps[0:Gn, :])

                hT = gp.tile([128, 2, 16], F32, tag="hT")
                nc.tensor.transpose(arena[:, 288:288 + Gn], hs[0:Gn, 0:128],
                                    ident[0:Gn, 0:Gn])
                nc.tensor.transpose(arena[0:64, 304:304 + Gn],
                                    hs[0:Gn, 128:192], ident[0:Gn, 0:Gn])
                nc.vector.tensor_copy(hT[:].rearrange("p a b -> p (a b)"),
                                      arena[:, 288:320])

                # final layer: two M=1 matmuls (gate rows at partition 0);
                # b2 folds into the sigmoid bias, alpha into a post-scale;
                # rows then replicated across partitions via K=1 matmuls
                crows = []
                for col, b2f, af in ((0, b2lo, alo), (1, b2hi, ahi)):
                    g_ps = arena[0:1, 320 + 16 * col:336 + 16 * col]
                    nc.tensor.matmul(g_ps[:, 0:Gn], _mm_ap(gate_consts["w2c0"][:, col:col + 1]),
                                     _mm_ap(hT[:, 0, 0:Gn]), start=True,
                                     stop=False)
                    nc.tensor.matmul(g_ps[:, 0:Gn], _mm_ap(gate_consts["w2c1"][:, col:col + 1]),
                                     _mm_ap(hT[0:64, 1, 0:Gn]), start=False,
                                     stop=True)
                    cr = gp.tile([1, 16], F32, tag=f"crow{col}")
                    nc.scalar.activation(cr[:, 0:Gn], g_ps[:, 0:Gn], Sig,
                                         bias=b2f)
                    crows.append(cr)
                # replication matmuls against alpha-scaled ones rows fold the
                # alpha multiply in; one copy lands both gate vectors
                for col, wrow in ((0, "alr"), (1, "ahr")):
                    nc.tensor.matmul(
                        arena[:, 352 + 16 * col:352 + 16 * col + Gn],
                        _mm_ap(gate_consts[wrow][0:1, :]),
                        _mm_ap(crows[col][0:1, 0:Gn]),
                        start=True, stop=True)
                crlh = gp.tile([128, 32], F32, tag="crlh")
                nc.vector.tensor_copy(crlh[:], arena[:, 352:384])
                crl = crlh[:, 0:16]
                crh = crlh[:, 16:32]

                # scale in place and store (hi lives in the x tiles)
                for j, b in enumerate(bs):
                    xa, xb, lo_a, lo_b = per_b[b]
                    nc.scalar.activation(lo_a[:], lo_a[:], Copy,
                                         scale=crl[:, j:j + 1])
                    nc.scalar.activation(lo_b[:], lo_b[:], Copy,
                                         scale=crl[0:69, j:j + 1])
                    nc.vector.tensor_scalar_mul(xa[:], xa[:],
                                                crh[:, j:j + 1])
                    nc.vector.tensor_scalar_mul(xb[:], xb[:], crh[0:69, j:j + 1])
                    nc.sync.dma_start(out=lo_h[b, 1:128, :], in_=lo_a[1:128])
                    nc.sync.dma_start(out=lo_h[b, 128:197, :], in_=lo_b[:])
                    nc.sync.dma_start(out=hi_h[b, 1:128, :],
                                      in_=xa[1:128].bitcast(F32))
                    nc.sync.dma_start(out=hi_h[b, 128:197, :],
                                      in_=xb[:].bitcast(F32))

            pending = None   # (bs, Gn, arena, per_b) of previous group
            for g in range(n_groups):
                bs = list(range(g * GRP, min((g + 1) * GRP, BS)))
                Gn = len(bs)
                # per-group psum arena for the gate pipeline (fresh column
                # ranges): 0:96 gT | 96:288 h | 288:320 hT | 320:352 gate |
                # 352:384 replication
                arena = par.tile([128, 512], F32, tag="arena")
                per_b = {}

                # issue the whole group's loads first so they sit AHEAD of
                # the previous group's store burst in the DMA queue FIFOs
                xt = {}
                for b in bs:
                    xa = xp.tile([128, D], YDT, tag="xa")
                    xb = xp.tile([69, D], YDT, tag="xb")
                    src_a = xs_h[b, 0:128, :]
                    src_b = xs_h[b, 128:197, :]
                    if YDT != F32:
                        src_a = src_a.bitcast(YDT)
                        src_b = src_b.bitcast(YDT)
                    nc.sync.dma_start(out=xa[:], in_=src_a)
                    nc.sync.dma_start(out=xb[:], in_=src_b)
                    xt[b] = (xa, xb)
                if g == 0:
                    load_deferred()

                for j, b in enumerate(bs):
                    xa, xb = xt[b]

                    # Y = M' @ x[b]; M' row 0 = token-mean row, rows 1..196
                    # = low-pass operator (CLS column is zero)
                    ylo = pm.tile([128, D], F32, tag="ym")
                    yhi = pm.tile([128, D], F32, tag="ym")
                    for (n0, n1) in NSPLIT:
                        nc.tensor.matmul(ylo[:, n0:n1], wt_lo[:, 0:128],
                                         xa[:, n0:n1], start=True, stop=False)
                        nc.tensor.matmul(ylo[:, n0:n1], wt_hi[:, 0:128],
                                         xb[:, n0:n1], start=False, stop=True)
                    for (n0, n1) in NSPLIT:
                        nc.tensor.matmul(yhi[0:69, n0:n1], wt_lo[:, 128:197],
                                         xa[:, n0:n1], start=True, stop=False)
                        nc.tensor.matmul(yhi[0:69, n0:n1], wt_hi[:, 128:197],
                                         xb[:, n0:n1], start=False, stop=True)

                    if share_Y:
                        zlo, zhi = ylo, yhi
                    else:
                        zlo = pmz.tile([128, D], F32, tag="zm")
                        zhi = pmz.tile([128, D], F32, tag="zm")
                        for (n0, n1) in NSPLIT:
                            nc.tensor.matmul(zlo[:, n0:n1], ct_lo[:, 0:128],
                                             xa[:, n0:n1], start=True, stop=False)
                            nc.tensor.matmul(zlo[:, n0:n1], ct_hi[:, 0:128],
                                             xb[:, n0:n1], start=False, stop=True)
                        for (n0, n1) in NSPLIT:
                            nc.tensor.matmul(zhi[0:69, n0:n1], ct_lo[:, 128:197],
                                             xa[:, n0:n1], start=True, stop=False)
                            nc.tensor.matmul(zhi[0:69, n0:n1], ct_hi[:, 128:197],
                                             xb[:, n0:n1], start=False, stop=True)

                    # PSUM -> SBUF (unscaled); hi = x - Y in place in x
                    # tiles; consumed per N-chunk so psum frees sooner
                    lo_a = outp.tile([128, D], F32, tag="lo_a")
                    lo_b = outp.tile([69, D], F32, tag="lo_b")
                    for (n0, n1) in NSPLIT:
                        nc.scalar.activation(lo_a[:, n0:n1], ylo[:, n0:n1], Copy)
                        nc.vector.tensor_sub(xa[:, n0:n1], xa[:, n0:n1],
                                             zlo[:, n0:n1])
                    for (n0, n1) in NSPLIT:
                        nc.scalar.activation(lo_b[:, n0:n1], yhi[0:69, n0:n1],
                                             Copy)
                        nc.vector.tensor_sub(xb[:, n0:n1], xb[:, n0:n1],
                                             zhi[0:69, n0:n1])

                    per_b[b] = (xa, xb, lo_a, lo_b)

                if share_Y:
                    # lag the gate chain one group to hide its latency
                    if pending is not None:
                        gate_and_store(*pending)
                    pending = (bs, Gn, arena, per_b)
                else:
                    # generic path: flush immediately (simpler dependency
                    # structure; correctness over overlap)
                    gate_and_store(bs, Gn, arena, per_b)

            if pending is not None:
                gate_and_store(*pending)
    if not nc.is_finalized():
        nc.finalize()
    return nc


def kernel(x, low_params, high_params, alpha_low, alpha_high,
           w1, b1, w2, b2, cls_token_idx):
    assert int(cls_token_idx) == 0
    x = np.ascontiguousarray(np.asarray(x, dtype=np.float32))
    assert x.shape == (B, N, D)

    lm = _freq_mask_np(low_params, True)
    A = _conv_operator(lm)                       # low operator [196, 196]
    share_Y = np.allclose(np.asarray(low_params, np.float32),
                          np.asarray(high_params, np.float32))
    Cm = A if share_Y else _conv_operator(_freq_mask_np(high_params, True))

    w1 = np.asarray(w1, np.float32)
    sig = lambda v: 1.0 / (1.0 + np.exp(-np.float64(v)))

    def make_consts(OP):
        # M' [197,197]: row 0 = token-mean row, rows 1..196 = OP; CLS col 0
        Mfull = np.zeros((N, N), np.float64)
        Mfull[0, 1:] = 1.0 / P
        Mfull[1:, 1:] = OP
        WT = np.ascontiguousarray(Mfull.T).astype(np.float32)
        wtblob = np.zeros((128, 410), np.float32)
        wtblob[:, 0:197] = WT[0:128]
        wtblob[0:69, 197:394] = WT[128:197]
        wtblob[0:16, 394:410] = np.eye(16, dtype=np.float32)
        gblob = np.zeros((128, 1620), np.float32)
        gblob[:, 0:1152] = w1.reshape(6, 128, 192).transpose(1, 0, 2).reshape(128, 1152)
        gblob[0, 1152:1344] = np.asarray(b1, np.float32)
        gblob[:, 1344:1346] = np.asarray(w2, np.float32)[0:128]
        gblob[0:64, 1346:1348] = np.asarray(w2, np.float32)[128:192]
        gblob[0, 1348:1364] = 1.0
        gblob[0, 1364:1492] = sig(alpha_low)
        gblob[0, 1492:1620] = sig(alpha_high)
        return {"wtblob": wtblob, "gblob": gblob}

    b2v = np.asarray(b2, np.float64).reshape(2)

    def run_once(consts):
        nc = _build_program(consts, True,
                            b2lo=float(b2v[0]), b2hi=float(b2v[1]),
                            alo=float(sig(alpha_low)), ahi=float(sig(alpha_high)))
        xs = x.reshape(NCORES, BS, N, D)
        in_maps = [{"xs": np.ascontiguousarray(xs[c])} for c in range(NCORES)]
        want_trace = bool(int(os.environ.get("KRN_TRACE", "0")))
        try:
            res = run_bass_kernel_spmd(nc, in_maps, core_ids=list(range(NCORES)),
                                       trace=want_trace)
        except ModuleNotFoundError:
            res = run_bass_kernel_spmd(nc, in_maps, core_ids=list(range(NCORES)))
        lo = np.concatenate([r["lo"] for r in res.results], axis=0)
        hi = np.concatenate([r["hi"] for r in res.results], axis=0)
        if getattr(res, "exec_time_ns", None) is not None:
            print(f"HW exec time: {res.exec_time_ns} ns")
        return lo, hi

    if share_Y:
        return run_once(make_consts(A))
    # generic case (never hit by the reference inputs): two passes of the
    # validated single-operator program — lo from the A pass, hi from the C
    # pass (the gate depends only on x, so it is identical in both)
    lo, _ = run_once(make_consts(A))
    _, hi = run_once(make_consts(Cm))
    return lo, hi



# revision 30
# speedup vs baseline: 1.9537x; 1.6298x over previous
"""DCT Frequency Splitter — Trainium2 Bass kernel (v3, bf16 end-to-end).

Math: FFT2 -> mask -> IFFT2 -> real is a linear operator on the 196 patch
tokens (per channel): z = A @ patches with A = Re(Finv diag(m) F) (196x196,
real, built on host from the 4 mask params).  With shared mask params the
high path is high = patches - z, so one matmul feeds both outputs:
lo = s_l * z, hi = s_h * (patches - z).

v3 layout decisions (all driven by the TimelineSim cost model):
- bf16 everywhere off-chip: x is converted to bf16 on the host, outputs are
  stored bf16 and upcast on the host.  Halves DMA traffic (the kernel is
  DMA-bound at ~360 GB/s/core); rel-err budget 2e-2 >> bf16's ~4e-3.
- gate-first: per-image token means are computed straight from the x tiles
  with tiny K-contraction matmuls into a PSUM arena (gT in [d, img] layout,
  no PSUM row drain + transpose shuffle), so the gate scales are ready when
  the main matmuls drain and the lo output leaves PSUM already scaled - one
  Activation pass instead of two.
- engine split per image: Act = scaled lo drains, DVE = hi subs + hi scale
  (a-part), Pool/GPSIMD = hi scale (b-part), PE = matmuls.
- group-batched DMAs (4 images per DMA) to keep the SP sequencer's ~1.1us
  per-DMA issue cost off the critical path.

Sharding: pure data parallel, batch 128 -> 16 per core across 8 cores.
"""

import os
import numpy as np

import concourse.bass as bass
import concourse.bacc as bacc_mod
import concourse.mybir as mybir
import concourse.tile as tile
from concourse.bass_utils import run_bass_kernel_spmd

H, W = 14, 14
B, N, D = 128, 197, 768
P = 196  # patch tokens
NCORES = 8
BS = B // NCORES  # batches per core

GRP = int(os.environ.get("KRN_GROUP", "4"))        # images per group
BUFX = int(os.environ.get("KRN_BUFX", "3"))        # x-tile group buffers
BUFO = int(os.environ.get("KRN_BUFO", "2"))        # lo-tile group buffers
WHOLE = bool(int(os.environ.get("KRN_WHOLE", "1")))  # 768-col drains/subs
F32 = mybir.dt.float32
BF16 = mybir.dt.bfloat16

# x tiles hold tokens 1..196 only (CLS skipped at load): xga = patches
# 0..127 on partitions 0..127, xgb = patches 128..195 on partitions 0..67,
# so matmul operands/outputs and the elementwise hi ops all share base
# partition 0 (the PE requires operand base partition in {0, 32, 64}).
# wtblob column layout (bf16, [128, 400])
WTA0 = 0      # A^T rows 0:128 (K = patches 0..127), cols 0:196
WTB0 = 196    # A^T rows 128:196 (K = patches 128..195), cols 196:392
ONES0 = 392   # [128, 1] column of 1/196
# gblob column layout (bf16, [128, 1648])
GW1, GB1 = 0, 1152
GW20, GW21 = 1344, 1346
GONES = 1348
GALR, GAHR = 1364, 1492


def _freq_mask_np(params, low):
    ch, cw, radius, sharp = [np.float64(v) for v in np.asarray(params)]
    y = np.arange(H, dtype=np.float64)
    x = np.arange(W, dtype=np.float64)
    d2 = (y[:, None] - ch) ** 2 + (x[None, :] - cw) ** 2
    dist = np.sqrt(d2 + 1e-12)
    s = np.clip(sharp, 0.5, 10.0)
    r = np.clip(radius, 1.0, min(H, W) / 2.0)
    m = np.exp(-((dist / r) ** s))
    return m if low else 1.0 - m


def _conv_operator(mask):
    """Real 196x196 operator equivalent to ifft2(fft2(img)*mask).real."""
    F_H = np.exp(-2j * np.pi * np.outer(np.arange(H), np.arange(H)) / H)
    F_W = np.exp(-2j * np.pi * np.outer(np.arange(W), np.arange(W)) / W)
    Fi_H = np.conj(F_H) / H
    Fi_W = np.conj(F_W) / W
    op = np.kron(Fi_H, Fi_W) @ np.diag(mask.ravel()) @ np.kron(F_H, F_W)
    return np.real(op)


def _build_program(consts, b2lo, b2hi):
    nc = bacc_mod.Bacc(None)

    xs_h = nc.dram_tensor("xs", [BS, N, D], BF16, kind="ExternalInput")
    lo_h = nc.dram_tensor("lo", [BS, N, D], BF16, kind="ExternalOutput")
    hi_h = nc.dram_tensor("hi", [BS, N, D], BF16, kind="ExternalOutput")

    ch = {k: nc.inline_tensor(v, name=f"c_{k}") for k, v in consts.items()}

    Copy = mybir.ActivationFunctionType.Copy
    Sig = mybir.ActivationFunctionType.Sigmoid

    n_groups = (BS + GRP - 1) // GRP
    groups = [list(range(g * GRP, min((g + 1) * GRP, BS)))
              for g in range(n_groups)]

    with tile.TileContext(nc) as tc:
        with (
            tc.tile_pool(name="consts", bufs=1) as cp,
            tc.tile_pool(name="xp", bufs=BUFX) as xp,
            tc.tile_pool(name="outp", bufs=BUFO) as outp,
            tc.tile_pool(name="gp", bufs=2) as gp,
            tc.tile_pool(name="pm", bufs=2, space="PSUM") as pm,
            tc.tile_pool(name="par", bufs=2, space="PSUM") as par,
        ):
            def cload(key, dtype):
                arr = consts[key]
                t = cp.tile(list(arr.shape), dtype, tag=key)
                nc.sync.dma_start(out=t[:], in_=ch[key][...])
                return t

            wb = cload("wtblob", BF16)      # [128, 400]
            gb = cload("gblob", BF16)       # [128, 1620]
            eyef = cload("eyef", F32)       # [16, 16] identity
            w1c = gb[:, GW1:GW1 + 1152].rearrange("p (a b) -> p a b", a=6)
            b1c = gb[0:1, GB1:GB1 + 192]
            w2c0 = gb[:, GW20:GW20 + 2]
            w2c1 = gb[0:64, GW21 + 0:GW21 + 2]
            ones1 = gb[0:1, GONES:GONES + 16]
            alr = gb[0:1, GALR:GALR + 128]
            ahr = gb[0:1, GAHR:GAHR + 128]
            onescol = wb[:, ONES0:ONES0 + 1]

            # CLS passthrough for all batches (DRAM -> DRAM)
            nc.sync.dma_start(out=lo_h[:, 0:1, :], in_=xs_h[:, 0:1, :])
            nc.sync.dma_start(out=hi_h[:, 0:1, :], in_=xs_h[:, 0:1, :])

            def load_group(g):
                bs = groups[g]
                b0, gn = bs[0], len(bs)
                xga = xp.tile([128, GRP, D], BF16, tag="xga")
                xgb = xp.tile([68, GRP, D], BF16, tag="xgb")
                nc.sync.dma_start(
                    out=xga[:, 0:gn, :],
                    in_=xs_h[b0:b0 + gn, 1:129, :].rearrange("b t d -> t b d"))
                nc.sync.dma_start(
                    out=xgb[:, 0:gn, :],
                    in_=xs_h[b0:b0 + gn, 129:197, :].rearrange("b t d -> t b d"))
                return xga, xgb

            xt = {0: load_group(0)}
            if n_groups > 1:
                xt[1] = load_group(1)

            for g, bs in enumerate(groups):
                gn = len(bs)
                b0 = bs[0]
                xga, xgb = xt.pop(g)

                # ---- gate for this group, ahead of the main matmuls.
                # gT[d, j] = mean over patch tokens of x (tiny K-contraction
                # matmuls straight into the psum arena, [d, img] layout).
                arena = par.tile([128, 384], F32, tag="arena")
                for j in range(gn):
                    for c in range(6):
                        col = c * 16 + j
                        nc.tensor.matmul(
                            arena[:, col:col + 1],
                            xga[:, j, c * 128:(c + 1) * 128],
                            onescol[:],
                            start=True, stop=False)
                        nc.tensor.matmul(
                            arena[:, col:col + 1],
                            xgb[0:68, j, c * 128:(c + 1) * 128],
                            onescol[0:68],
                            start=False, stop=True)
                gTt = gp.tile([128, 6, 16], BF16, tag="gTt")
                nc.vector.tensor_copy(
                    gTt[:].rearrange("p a b -> p (a b)"), arena[:, 0:96])

                h_ps = arena[0:16, 96:288]
                for c in range(6):
                    nc.tensor.matmul(h_ps[0:gn, :], gTt[:, c, 0:gn],
                                     w1c[:, c, :], start=(c == 0), stop=False)
                nc.tensor.matmul(h_ps[0:gn, :], ones1[0:1, 0:gn],
                                 b1c[0:1, :], start=False, stop=True)
                hs = gp.tile([16, 192], F32, tag="hs")
                nc.vector.tensor_relu(hs[0:gn, :], h_ps[0:gn, :])

                nc.tensor.transpose(arena[:, 288:288 + gn], hs[0:gn, 0:128],
                                    eyef[0:gn, 0:gn])
                nc.tensor.transpose(arena[0:64, 304:304 + gn],
                                    hs[0:gn, 128:192], eyef[0:gn, 0:gn])
                hTt = gp.tile([128, 2, 16], BF16, tag="hTt")
                nc.vector.tensor_copy(hTt[:].rearrange("p a b -> p (a b)"),
                                      arena[:, 288:320])

                crows = []
                for col, b2f in ((0, b2lo), (1, b2hi)):
                    g_ps = arena[0:1, 320 + 16 * col:336 + 16 * col]
                    nc.tensor.matmul(g_ps[:, 0:gn], w2c0[:, col:col + 1],
                                     hTt[:, 0, 0:gn], start=True, stop=False)
                    nc.tensor.matmul(g_ps[:, 0:gn], w2c1[:, col:col + 1],
                                     hTt[0:64, 1, 0:gn], start=False, stop=True)
                    cr = gp.tile([1, 16], BF16, tag=f"crow{col}")
                    nc.scalar.activation(cr[:, 0:gn], g_ps[:, 0:gn], Sig,
                                         bias=b2f)
                    crows.append(cr)
                # replicate the gate rows across partitions; the alpha
                # sigmoid is folded into the alr/ahr weight rows
                for col, wrow in ((0, alr), (1, ahr)):
                    nc.tensor.matmul(
                        arena[:, 352 + 16 * col:352 + 16 * col + gn],
                        wrow[0:1, :], crows[col][0:1, 0:gn],
                        start=True, stop=True)
                crlh = gp.tile([128, 32], F32, tag="crlh")
                nc.vector.tensor_copy(crlh[:], arena[:, 352:384])
                crl = crlh[:, 0:16]
                crh = crlh[:, 16:32]

                # ---- main matmuls + scaled drains per image
                lo_ga = outp.tile([128, GRP, D], BF16, tag="lo_ga")
                lo_gb = outp.tile([68, GRP, D], BF16, tag="lo_gb")
                for j in range(gn):
                    # PSUM packing: 3 banks/image (za1 | zb1 | za2+zb2)
                    za1 = pm.tile([128, 512], F32, tag="za1")
                    zb1 = pm.tile([68, 512], F32, tag="zb1")
                    zab2 = pm.tile([128, 512], F32, tag="zab2")
                    za_ch = [(0, 512, za1[:, :]), (512, 768, zab2[:, 0:256])]
                    zb_ch = [(0, 512, zb1[0:68, :]),
                             (512, 768, zab2[0:68, 256:512])]
                    for (n0, n1, zc) in za_ch:
                        nc.tensor.matmul(zc, wb[:, 0:128],
                                         xga[:, j, n0:n1],
                                         start=True, stop=False)
                        nc.tensor.matmul(zc, wb[0:68, WTB0:WTB0 + 128],
                                         xgb[0:68, j, n0:n1],
                                         start=False, stop=True)
                    for (n0, n1, zc) in zb_ch:
                        nc.tensor.matmul(zc, wb[:, 128:196],
                                         xga[:, j, n0:n1],
                                         start=True, stop=False)
                        nc.tensor.matmul(zc, wb[0:68, WTB0 + 128:WTB0 + 196],
                                         xgb[0:68, j, n0:n1],
                                         start=False, stop=True)

                    # lo leaves PSUM already scaled (single Act pass)
                    for (n0, n1, zc) in za_ch:
                        nc.scalar.activation(lo_ga[:, j, n0:n1], zc, Copy,
                                             scale=crl[:, j:j + 1])
                    for (n0, n1, zc) in zb_ch:
                        nc.scalar.activation(lo_gb[0:68, j, n0:n1], zc, Copy,
                                             scale=crl[0:68, j:j + 1])
                    # hi = crh * (x - z): subs on DVE; the scale is split
                    # between DVE (bf16 2x mode) and the idle GPSIMD engine
                    for (n0, n1, zc) in za_ch:
                        nc.vector.tensor_sub(xga[:, j, n0:n1],
                                             xga[:, j, n0:n1], zc)
                    for (n0, n1, zc) in zb_ch:
                        nc.vector.tensor_sub(xgb[0:68, j, n0:n1],
                                             xgb[0:68, j, n0:n1], zc)
                    nc.vector.tensor_scalar_mul(xga[:, j, 0:384],
                                                xga[:, j, 0:384],
                                                crh[:, j:j + 1])
                    nc.gpsimd.tensor_scalar_mul(xga[:, j, 384:768],
                                                xga[:, j, 384:768],
                                                crh[:, j:j + 1])
                    nc.gpsimd.tensor_scalar_mul(xgb[0:68, j, :],
                                                xgb[0:68, j, :],
                                                crh[0:68, j:j + 1])

                # next-next group's loads go ahead of this group's stores
                if g + 2 < n_groups:
                    xt[g + 2] = load_group(g + 2)

                nc.sync.dma_start(
                    out=lo_h[b0:b0 + gn, 1:129, :].rearrange("b t d -> t b d"),
                    in_=lo_ga[:, 0:gn, :])
                nc.sync.dma_start(
                    out=lo_h[b0:b0 + gn, 129:197, :].rearrange("b t d -> t b d"),
                    in_=lo_gb[:, 0:gn, :])
                nc.sync.dma_start(
                    out=hi_h[b0:b0 + gn, 1:129, :].rearrange("b t d -> t b d"),
                    in_=xga[:, 0:gn, :])
                nc.sync.dma_start(
                    out=hi_h[b0:b0 + gn, 129:197, :].rearrange("b t d -> t b d"),
                    in_=xgb[:, 0:gn, :])

    if not nc.is_finalized():
        nc.finalize()
    return nc


def _make_consts(OP, w1, b1, w2, alpha_low, alpha_high):
    import ml_dtypes
    sig = lambda v: 1.0 / (1.0 + np.exp(-np.float64(v)))
    WT = np.ascontiguousarray(np.asarray(OP, np.float64).T)
    wtblob = np.zeros((128, 400), np.float32)
    wtblob[0:128, 0:196] = WT[0:128]
    wtblob[0:68, 196:392] = WT[128:196]
    wtblob[:, ONES0] = 1.0 / P
    gblob = np.zeros((128, 1620), np.float32)
    gblob[:, GW1:GW1 + 1152] = np.asarray(w1, np.float32).reshape(
        6, 128, 192).transpose(1, 0, 2).reshape(128, 1152)
    gblob[0, GB1:GB1 + 192] = np.asarray(b1, np.float32)
    gblob[:, GW20:GW20 + 2] = np.asarray(w2, np.float32)[0:128]
    gblob[0:64, GW21:GW21 + 2] = np.asarray(w2, np.float32)[128:192]
    gblob[0, GONES:GONES + 16] = 1.0
    gblob[0, GALR:GALR + 128] = sig(alpha_low)
    gblob[0, GAHR:GAHR + 128] = sig(alpha_high)
    return {"wtblob": wtblob.astype(ml_dtypes.bfloat16),
            "gblob": gblob.astype(ml_dtypes.bfloat16),
            "eyef": np.eye(16, dtype=np.float32)}


def build_for_sim():
    """Program instance for cost-model simulation (dummy weights)."""
    import ml_dtypes
    consts = {
        "wtblob": np.zeros((128, 400), ml_dtypes.bfloat16),
        "gblob": np.zeros((128, 1620), ml_dtypes.bfloat16),
        "eyef": np.eye(16, dtype=np.float32),
    }
    return _build_program(consts, 0.0, 0.0)


def kernel(x, low_params, high_params, alpha_low, alpha_high,
           w1, b1, w2, b2, cls_token_idx):
    import ml_dtypes
    assert int(cls_token_idx) == 0
    x = np.asarray(x, dtype=np.float32)
    assert x.shape == (B, N, D)

    lm = _freq_mask_np(low_params, True)
    A = _conv_operator(lm)
    share_Y = np.allclose(np.asarray(low_params, np.float32),
                          np.asarray(high_params, np.float32))
    b2v = np.asarray(b2, np.float64).reshape(2)

    xbf = np.ascontiguousarray(x.astype(ml_dtypes.bfloat16))
    xs = xbf.reshape(NCORES, BS, N, D)
    in_maps = [{"xs": np.ascontiguousarray(xs[c])} for c in range(NCORES)]

    def run_once(OP):
        consts = _make_consts(OP, w1, b1, w2, alpha_low, alpha_high)
        nc = _build_program(consts, float(b2v[0]), float(b2v[1]))
        res = run_bass_kernel_spmd(nc, in_maps, core_ids=list(range(NCORES)))
        lo = np.concatenate([np.asarray(r["lo"]) for r in res.results],
                            axis=0).astype(np.float32)
        hi = np.concatenate([np.asarray(r["hi"]) for r in res.results],
                            axis=0).astype(np.float32)
        if getattr(res, "exec_time_ns", None) is not None:
            print(f"HW exec time: {res.exec_time_ns} ns")
        return lo, hi

    if share_Y:
        return run_once(A)
    # generic case (not hit by the reference inputs): hi needs its own
    # operator; run the validated single-operator program twice
    lo, _ = run_once(A)
    Cm = _conv_operator(_freq_mask_np(high_params, True))
    _, hi = run_once(Cm)
    return lo, hi


# revision 68
# speedup vs baseline: 2.3443x; 1.1999x over previous
"""DCT Frequency Splitter — Trainium2 Bass kernel (v3, bf16 end-to-end).

Math: FFT2 -> mask -> IFFT2 -> real is a linear operator on the 196 patch
tokens (per channel): z = A @ patches with A = Re(Finv diag(m) F) (196x196,
real, built on host from the 4 mask params).  With shared mask params the
high path is high = patches - z, so one matmul feeds both outputs:
lo = s_l * z, hi = s_h * (patches - z).

v3 layout decisions (all driven by the TimelineSim cost model):
- bf16 everywhere off-chip: x is converted to bf16 on the host, outputs are
  stored bf16 and upcast on the host.  Halves DMA traffic (the kernel is
  DMA-bound at ~360 GB/s/core); rel-err budget 2e-2 >> bf16's ~4e-3.
- gate-first: per-image token means are computed straight from the x tiles
  with tiny K-contraction matmuls into a PSUM arena (gT in [d, img] layout,
  no PSUM row drain + transpose shuffle), so the gate scales are ready when
  the main matmuls drain and the lo output leaves PSUM already scaled - one
  Activation pass instead of two.
- engine split per image: Act = scaled lo drains, DVE = hi subs + hi scale
  (a-part), Pool/GPSIMD = hi scale (b-part), PE = matmuls.
- group-batched DMAs (4 images per DMA) to keep the SP sequencer's ~1.1us
  per-DMA issue cost off the critical path.

Sharding: pure data parallel, batch 128 -> 16 per core across 8 cores.
"""

import os
import numpy as np

import concourse.bass as bass
import concourse.bacc as bacc_mod
import concourse.mybir as mybir
import concourse.tile as tile
from concourse.bass_utils import run_bass_kernel_spmd
from concourse.tile_rust import add_dep_helper

H, W = 14, 14
B, N, D = 128, 197, 768
P = 196  # patch tokens
NCORES = 8
BS = B // NCORES  # batches per core

GRP = int(os.environ.get("KRN_GROUP", "4"))        # images per group
BUFO = int(os.environ.get("KRN_BUFO", "2"))        # lo-tile group buffers
F32 = mybir.dt.float32
BF16 = mybir.dt.bfloat16

# x tiles hold tokens 1..196 only (CLS skipped at load): xga = patches
# 0..127 on partitions 0..127, xgb = patches 128..195 on partitions 0..67,
# so matmul operands/outputs and the elementwise hi ops all share base
# partition 0 (the PE requires operand base partition in {0, 32, 64}).
# wtblob column layout (bf16, [128, 400])
WTA0 = 0      # A^T rows 0:128 (K = patches 0..127), cols 0:196
WTB0 = 196    # A^T rows 128:196 (K = patches 128..195), cols 196:392
ONES0 = 392   # [128, 1] column of 1/196
# gblob column layout (bf16, [128, 1748])
GW1, GB1 = 0, 1152
GW20, GW21 = 1344, 1346
GONES = 1348
GALR, GAHR = 1364, 1492
GCNEG = 1620  # -sig(alpha_high)/sig(alpha_low) row for the hi-ratio


def _freq_mask_np(params, low):
    ch, cw, radius, sharp = [np.float64(v) for v in np.asarray(params)]
    y = np.arange(H, dtype=np.float64)
    x = np.arange(W, dtype=np.float64)
    d2 = (y[:, None] - ch) ** 2 + (x[None, :] - cw) ** 2
    dist = np.sqrt(d2 + 1e-12)
    s = np.clip(sharp, 0.5, 10.0)
    r = np.clip(radius, 1.0, min(H, W) / 2.0)
    m = np.exp(-((dist / r) ** s))
    return m if low else 1.0 - m


def _conv_operator(mask):
    """Real 196x196 operator equivalent to ifft2(fft2(img)*mask).real."""
    F_H = np.exp(-2j * np.pi * np.outer(np.arange(H), np.arange(H)) / H)
    F_W = np.exp(-2j * np.pi * np.outer(np.arange(W), np.arange(W)) / W)
    Fi_H = np.conj(F_H) / H
    Fi_W = np.conj(F_W) / W
    op = np.kron(Fi_H, Fi_W) @ np.diag(mask.ravel()) @ np.kron(F_H, F_W)
    return np.real(op)


def _build_program(consts, b2lo, b2hi):
    nc = bacc_mod.Bacc(None)

    xs_h = nc.dram_tensor("xs", [BS, N, D], BF16, kind="ExternalInput")
    lo_h = nc.dram_tensor("lo", [BS, N, D], BF16, kind="ExternalOutput")
    hi_h = nc.dram_tensor("hi", [BS, N, D], BF16, kind="ExternalOutput")

    ch = {k: nc.inline_tensor(v, name=f"c_{k}") for k, v in consts.items()}

    Copy = mybir.ActivationFunctionType.Copy
    Sig = mybir.ActivationFunctionType.Sigmoid

    # small first group (early first stores = DMA ramps sooner) and small
    # last group (short drain+store tail); 4-image groups in the middle
    if os.environ.get("KRN_GPAT"):
        sizes = [int(v) for v in os.environ["KRN_GPAT"].split(",")]
        assert sum(sizes) == BS
    elif BS == 16 and GRP == 4:
        sizes = [2, 3, 4, 4, 3]
    else:
        sizes = [min(GRP, BS - s) for s in range(0, BS, GRP)]
    groups, s = [], 0
    for sz in sizes:
        groups.append(list(range(s, s + sz)))
        s += sz
    n_groups = len(groups)

    with tile.TileContext(nc) as tc:
        with (
            tc.tile_pool(name="consts", bufs=1) as cp,
            tc.tile_pool(name="xp", bufs=n_groups) as xp,
            tc.tile_pool(name="outp", bufs=BUFO) as outp,
            tc.tile_pool(name="gp", bufs=2) as gp,
            tc.tile_pool(name="pm", bufs=2, space="PSUM") as pm,
            tc.tile_pool(name="par", bufs=2, space="PSUM") as par,
        ):
            def cload(key, dtype):
                arr = consts[key]
                t = cp.tile(list(arr.shape), dtype, tag=key)
                nc.sync.dma_start(out=t[:], in_=ch[key][...])
                return t

            wb = cload("wtblob", BF16)      # [128, 400]
            onescol = wb[:, ONES0:ONES0 + 1]

            def load_group(g):
                bs = groups[g]
                b0, gn = bs[0], len(bs)
                xga = xp.tile([128, GRP, D], BF16, tag="xga")
                xgb = xp.tile([68, GRP, D], BF16, tag="xgb")
                nc.sync.dma_start(
                    out=xga[:, 0:gn, :],
                    in_=xs_h[b0:b0 + gn, 1:129, :].rearrange("b t d -> t b d"))
                nc.sync.dma_start(
                    out=xgb[:, 0:gn, :],
                    in_=xs_h[b0:b0 + gn, 129:197, :].rearrange("b t d -> t b d"))
                return xga, xgb

            # first groups' loads go before the remaining consts so their
            # gate chains start as early as possible; all loads precede all
            # stores so SP's in-order sequencer never parks a semaphore-
            # blocked store ahead of a ready load
            xt = {g: load_group(g) for g in range(min(2, n_groups))}

            gb = cload("gblob", BF16)       # [128, 1620]
            eyef = cload("eyef", F32)       # [16, 16] identity
            w1c = gb[:, GW1:GW1 + 1152].rearrange("p (a h b) -> p a h b",
                                                  a=6, h=2)
            w2c0 = gb[0:96, GW20:GW20 + 2]
            w2c1 = gb[0:96, GW21 + 0:GW21 + 2]
            ones1 = gb[0:1, GONES:GONES + 16]
            alr = gb[0:1, GALR:GALR + 128]
            ahr = gb[0:1, GAHR:GAHR + 128]
            cneg = gb[0:1, GCNEG:GCNEG + 128]

            # dummy activation so the act-func table load (~1.3us) runs at
            # t~=1us instead of stalling the first group's gate sigmoid
            warm = gp.tile([1, 16], F32, tag="warm")
            nc.scalar.activation(warm[:], eyef[0:1, 0:16], Sig)

            # CLS passthrough for all batches (DRAM -> DRAM), issued from
            # the near-free GPSIMD DMA queue so SP only handles bulk I/O
            nc.gpsimd.dma_start(out=lo_h[:, 0:1, :], in_=xs_h[:, 0:1, :])
            nc.gpsimd.dma_start(out=hi_h[:, 0:1, :], in_=xs_h[:, 0:1, :])

            for g in range(2, n_groups):
                xt[g] = load_group(g)

            def gate_chain(g, xga, xgb):
                bs = groups[g]
                gn = len(bs)
                # gate for this group, ahead of the main matmuls.
                # gT[d, j] = mean over patch tokens of x (tiny K-contraction
                # matmuls straight into the psum arena, [d, img] layout).
                arena = par.tile([128, 384], F32, tag="arena")
                for j in range(gn):
                    for c in range(6):
                        col = c * 16 + j
                        nc.tensor.matmul(
                            arena[:, col:col + 1],
                            xga[:, j, c * 128:(c + 1) * 128],
                            onescol[:],
                            start=True, stop=False)
                        nc.tensor.matmul(
                            arena[:, col:col + 1],
                            xgb[0:68, j, c * 128:(c + 1) * 128],
                            onescol[0:68],
                            start=False, stop=True)
                gTt = gp.tile([128, 6, 16], BF16, tag="gTt")
                nc.vector.tensor_copy(
                    gTt[:].rearrange("p a b -> p (a b)"), arena[:, 0:96])

                # hidden layer directly in transposed [feature, img] layout
                # (w1 chunks stationary): no transposes, tiny moving dims
                gate_pe = None
                for h in range(2):
                    hps = arena[0:96, 96 + 16 * h:96 + 16 * h + 16]
                    for c in range(6):
                        nc.tensor.matmul(hps[:, 0:gn], w1c[:, c, h, :],
                                         gTt[:, c, 0:gn],
                                         start=(c == 0), stop=False)
                    gate_pe = nc.tensor.matmul(
                        hps[:, 0:gn], gb[0:1, GB1 + 96 * h:GB1 + 96 * h + 96],
                        ones1[0:1, 0:gn], start=False, stop=True)
                hTt = gp.tile([96, 2, 16], BF16, tag="hTt")
                for h in range(2):
                    hps = arena[0:96, 96 + 16 * h:96 + 16 * h + 16]
                    nc.vector.tensor_relu(hTt[:, h, 0:gn], hps[:, 0:gn])

                crows = []
                for col, b2f in ((0, b2lo), (1, b2hi)):
                    g_ps = arena[0:1, 128 + 16 * col:144 + 16 * col]
                    nc.tensor.matmul(g_ps[:, 0:gn], w2c0[:, col:col + 1],
                                     hTt[:, 0, 0:gn], start=True, stop=False)
                    nc.tensor.matmul(g_ps[:, 0:gn], w2c1[:, col:col + 1],
                                     hTt[:, 1, 0:gn], start=False, stop=True)
                    cr = gp.tile([1, 16], BF16, tag=f"crow{col}")
                    nc.scalar.activation(cr[:, 0:gn], g_ps[:, 0:gn], Sig,
                                         bias=b2f)
                    crows.append(cr)
                # replicate the gate rows across partitions; the alpha
                # sigmoid is folded into the alr/ahr weight rows
                for k, (wrow, mov) in enumerate(
                        ((alr, crows[0][0:1, 0:gn]),
                         (ahr, crows[1][0:1, 0:gn]))):
                    nc.tensor.matmul(arena[:, 160 + 16 * k:160 + 16 * k + gn],
                                     wrow[0:1, :], mov, start=True, stop=True)
                crlh = gp.tile([128, 32], F32, tag="crlh")
                nc.vector.tensor_copy(crlh[:], arena[:, 160:192])
                return crlh, gate_pe

            # gates run two groups ahead of their bodies so their small
            # DVE/Act steps never queue behind a full body's engine work
            gates = {g: gate_chain(g, *xt[g]) for g in range(min(2, n_groups))}

            for g, bs in enumerate(groups):
                gn = len(bs)
                b0 = bs[0]
                xga, xgb = xt[g]
                crlh, gate_pe = gates.pop(g)
                crl = crlh[:, 0:16]
                crh = crlh[:, 16:32]

                # ---- main matmuls + scaled drains per image
                lo_ga = outp.tile([128, GRP, D], BF16, tag="lo_ga")
                lo_gb = outp.tile([68, GRP, D], BF16, tag="lo_gb")
                for j in range(gn):
                    # PSUM packing: 3 banks/image (za1 | zb1 | za2+zb2)
                    za1 = pm.tile([128, 512], F32, tag="za1")
                    zb1 = pm.tile([68, 512], F32, tag="zb1")
                    zab2 = pm.tile([128, 512], F32, tag="zab2")
                    za_ch = [(0, 512, za1[:, :]), (512, 768, zab2[:, 0:256])]
                    zb_ch = [(0, 512, zb1[0:68, :]),
                             (512, 768, zab2[0:68, 256:512])]
                    for (n0, n1, zc) in za_ch:
                        mm = nc.tensor.matmul(zc, wb[:, 0:128],
                                              xga[:, j, n0:n1],
                                              start=True, stop=False)
                        add_dep_helper(mm.ins, gate_pe.ins,
                                       reason="gate chain schedules first")
                        nc.tensor.matmul(zc, wb[0:68, WTB0:WTB0 + 128],
                                         xgb[0:68, j, n0:n1],
                                         start=False, stop=True)
                    for (n0, n1, zc) in zb_ch:
                        mm = nc.tensor.matmul(zc, wb[:, 128:196],
                                              xga[:, j, n0:n1],
                                              start=True, stop=False)
                        add_dep_helper(mm.ins, gate_pe.ins,
                                       reason="gate chain schedules first")
                        nc.tensor.matmul(zc, wb[0:68, WTB0 + 128:WTB0 + 196],
                                         xgb[0:68, j, n0:n1],
                                         start=False, stop=True)

                    # GPSIMD cannot touch PSUM, so the legal engine split is:
                    #   Act:  all four scaled lo drains (+ gate sigmoids)
                    #   DVE:  all four hi subtracts
                    #   Pool: both hi scales (SBUF-only bf16)
                    nc.scalar.activation(lo_ga[:, j, 0:512], za1[:, :], Copy,
                                         scale=crl[:, j:j + 1])
                    nc.scalar.activation(lo_ga[:, j, 512:768],
                                         zab2[:, 0:256], Copy,
                                         scale=crl[:, j:j + 1])
                    nc.scalar.activation(lo_gb[0:68, j, 0:512], zb1[0:68, :],
                                         Copy, scale=crl[0:68, j:j + 1])
                    nc.scalar.activation(lo_gb[0:68, j, 512:768],
                                         zab2[0:68, 256:512], Copy,
                                         scale=crl[0:68, j:j + 1])
                    # hi = crh * (x - z)
                    nc.vector.tensor_sub(xga[:, j, 0:512],
                                         xga[:, j, 0:512], za1[:, :])
                    nc.vector.tensor_sub(xga[:, j, 512:768],
                                         xga[:, j, 512:768], zab2[:, 0:256])
                    nc.vector.tensor_sub(xgb[0:68, j, 0:512],
                                         xgb[0:68, j, 0:512], zb1[0:68, :])
                    nc.vector.tensor_sub(xgb[0:68, j, 512:768],
                                         xgb[0:68, j, 512:768],
                                         zab2[0:68, 256:512])
                    nc.gpsimd.tensor_scalar_mul(xga[:, j, :],
                                                xga[:, j, :],
                                                crh[:, j:j + 1])
                    nc.gpsimd.tensor_scalar_mul(xgb[0:68, j, :],
                                                xgb[0:68, j, :],
                                                crh[0:68, j:j + 1])

                # next-next group's gate chain goes ahead of this group's
                # stores (on PE it runs while Act/DVE/Pool finish this group)
                if g + 2 < n_groups:
                    gates[g + 2] = gate_chain(g + 2, *xt[g + 2])

                # all bulk stores from SP: with every load pre-issued, SP's
                # in-order queue matches completion order (no head-of-line
                # blocking), and HWDGE issue keeps the Pool engine free of
                # the ~1us/DMA SWDGE generation cost.  2-image granularity
                # lets each half ship while the next half still drains.
                for k0 in range(0, gn, 2):
                    k1 = min(k0 + 2, gn)
                    c0, c1 = b0 + k0, b0 + k1
                    nc.sync.dma_start(
                        out=lo_h[c0:c1, 1:129, :].rearrange("b t d -> t b d"),
                        in_=lo_ga[:, k0:k1, :])
                    nc.sync.dma_start(
                        out=lo_h[c0:c1, 129:197, :].rearrange("b t d -> t b d"),
                        in_=lo_gb[:, k0:k1, :])
                    nc.sync.dma_start(
                        out=hi_h[c0:c1, 1:129, :].rearrange("b t d -> t b d"),
                        in_=xga[:, k0:k1, :])
                    nc.sync.dma_start(
                        out=hi_h[c0:c1, 129:197, :].rearrange("b t d -> t b d"),
                        in_=xgb[:, k0:k1, :])

    if not nc.is_finalized():
        nc.finalize()
    return nc


def _make_consts(OP, w1, b1, w2, alpha_low, alpha_high):
    import ml_dtypes
    sig = lambda v: 1.0 / (1.0 + np.exp(-np.float64(v)))
    WT = np.ascontiguousarray(np.asarray(OP, np.float64).T)
    wtblob = np.zeros((128, 400), np.float32)
    wtblob[0:128, 0:196] = WT[0:128]
    wtblob[0:68, 196:392] = WT[128:196]
    wtblob[:, ONES0] = 1.0 / P
    gblob = np.zeros((128, 1748), np.float32)
    gblob[:, GW1:GW1 + 1152] = np.asarray(w1, np.float32).reshape(
        6, 128, 192).transpose(1, 0, 2).reshape(128, 1152)
    gblob[0, GB1:GB1 + 192] = np.asarray(b1, np.float32)
    gblob[0:96, GW20:GW20 + 2] = np.asarray(w2, np.float32)[0:96]
    gblob[0:96, GW21:GW21 + 2] = np.asarray(w2, np.float32)[96:192]
    gblob[0, GONES:GONES + 16] = 1.0
    gblob[0, GALR:GALR + 128] = sig(alpha_low)
    gblob[0, GAHR:GAHR + 128] = sig(alpha_high)
    gblob[0, GCNEG:GCNEG + 128] = -sig(alpha_high) / sig(alpha_low)
    return {"wtblob": wtblob.astype(ml_dtypes.bfloat16),
            "gblob": gblob.astype(ml_dtypes.bfloat16),
            "eyef": np.eye(16, dtype=np.float32)}


def build_for_sim():
    """Program instance for cost-model simulation (dummy weights)."""
    import ml_dtypes
    consts = {
        "wtblob": np.zeros((128, 400), ml_dtypes.bfloat16),
        "gblob": np.ones((128, 1748), ml_dtypes.bfloat16),
        "eyef": np.eye(16, dtype=np.float32),
    }
    return _build_program(consts, 0.0, 0.0)


def kernel(x, low_params, high_params, alpha_low, alpha_high,
           w1, b1, w2, b2, cls_token_idx):
    import ml_dtypes
    assert int(cls_token_idx) == 0
    x = np.asarray(x, dtype=np.float32)
    assert x.shape == (B, N, D)

    lm = _freq_mask_np(low_params, True)
    A = _conv_operator(lm)
    share_Y = np.allclose(np.asarray(low_params, np.float32),
                          np.asarray(high_params, np.float32))
    b2v = np.asarray(b2, np.float64).reshape(2)

    xbf = np.ascontiguousarray(x.astype(ml_dtypes.bfloat16))
    xs = xbf.reshape(NCORES, BS, N, D)
    in_maps = [{"xs": np.ascontiguousarray(xs[c])} for c in range(NCORES)]

    def run_once(OP):
        consts = _make_consts(OP, w1, b1, w2, alpha_low, alpha_high)
        nc = _build_program(consts, float(b2v[0]), float(b2v[1]))
        res = run_bass_kernel_spmd(nc, in_maps, core_ids=list(range(NCORES)))
        lo = np.concatenate([np.asarray(r["lo"]) for r in res.results],
                            axis=0).astype(np.float32)
        hi = np.concatenate([np.asarray(r["hi"]) for r in res.results],
                            axis=0).astype(np.float32)
        if getattr(res, "exec_time_ns", None) is not None:
            print(f"HW exec time: {res.exec_time_ns} ns")
        return lo, hi

    if share_Y:
        return run_once(A)
    # generic case (not hit by the reference inputs): hi needs its own
    # operator; run the validated single-operator program twice
    lo, _ = run_once(A)
    Cm = _conv_operator(_freq_mask_np(high_params, True))
    _, hi = run_once(Cm)
    return lo, hi


# revision 72
# speedup vs baseline: 2.5994x; 1.1088x over previous
"""DCT Frequency Splitter — Trainium2 Bass kernel (v3, bf16 end-to-end).

Math: FFT2 -> mask -> IFFT2 -> real is a linear operator on the 196 patch
tokens (per channel): z = A @ patches with A = Re(Finv diag(m) F) (196x196,
real, built on host from the 4 mask params).  With shared mask params the
high path is high = patches - z, so one matmul feeds both outputs:
lo = s_l * z, hi = s_h * (patches - z).

v3 layout decisions (all driven by the TimelineSim cost model):
- bf16 everywhere off-chip: x is converted to bf16 on the host, outputs are
  stored bf16 and upcast on the host.  Halves DMA traffic (the kernel is
  DMA-bound at ~360 GB/s/core); rel-err budget 2e-2 >> bf16's ~4e-3.
- gate-first: per-image token means are computed straight from the x tiles
  with tiny K-contraction matmuls into a PSUM arena (gT in [d, img] layout,
  no PSUM row drain + transpose shuffle), so the gate scales are ready when
  the main matmuls drain and the lo output leaves PSUM already scaled - one
  Activation pass instead of two.
- engine split per image: Act = scaled lo drains, DVE = hi subs + hi scale
  (a-part), Pool/GPSIMD = hi scale (b-part), PE = matmuls.
- group-batched DMAs (4 images per DMA) to keep the SP sequencer's ~1.1us
  per-DMA issue cost off the critical path.

Sharding: pure data parallel, batch 128 -> 16 per core across 8 cores.
"""

import os
import numpy as np

import concourse.bass as bass
import concourse.bacc as bacc_mod
import concourse.mybir as mybir
import concourse.tile as tile
from concourse.bass_utils import run_bass_kernel_spmd
from concourse.tile_rust import add_dep_helper

H, W = 14, 14
B, N, D = 128, 197, 768
P = 196  # patch tokens
NCORES = 8
BS = B // NCORES  # batches per core

GRP = int(os.environ.get("KRN_GROUP", "4"))        # images per group
BUFO = int(os.environ.get("KRN_BUFO", "2"))        # lo-tile group buffers
F32 = mybir.dt.float32
BF16 = mybir.dt.bfloat16

# x tiles hold tokens 1..196 only (CLS skipped at load): xga = patches
# 0..127 on partitions 0..127, xgb = patches 128..195 on partitions 0..67,
# so matmul operands/outputs and the elementwise hi ops all share base
# partition 0 (the PE requires operand base partition in {0, 32, 64}).
# wtblob column layout (bf16, [128, 400])
WTA0 = 0      # A^T rows 0:128 (K = patches 0..127), cols 0:196
WTB0 = 196    # A^T rows 128:196 (K = patches 128..195), cols 196:392
ONES0 = 392   # [128, 1] column of 1/196
# gblob column layout (bf16, [128, 1748])
GW1, GB1 = 0, 1152
GW20, GW21 = 1344, 1346
GONES = 1348
GALR, GAHR = 1364, 1492
GCNEG = 1620  # -sig(alpha_high)/sig(alpha_low) row for the hi-ratio


def _freq_mask_np(params, low):
    ch, cw, radius, sharp = [np.float64(v) for v in np.asarray(params)]
    y = np.arange(H, dtype=np.float64)
    x = np.arange(W, dtype=np.float64)
    d2 = (y[:, None] - ch) ** 2 + (x[None, :] - cw) ** 2
    dist = np.sqrt(d2 + 1e-12)
    s = np.clip(sharp, 0.5, 10.0)
    r = np.clip(radius, 1.0, min(H, W) / 2.0)
    m = np.exp(-((dist / r) ** s))
    return m if low else 1.0 - m


def _conv_operator(mask):
    """Real 196x196 operator equivalent to ifft2(fft2(img)*mask).real."""
    F_H = np.exp(-2j * np.pi * np.outer(np.arange(H), np.arange(H)) / H)
    F_W = np.exp(-2j * np.pi * np.outer(np.arange(W), np.arange(W)) / W)
    Fi_H = np.conj(F_H) / H
    Fi_W = np.conj(F_W) / W
    op = np.kron(Fi_H, Fi_W) @ np.diag(mask.ravel()) @ np.kron(F_H, F_W)
    return np.real(op)


def _build_program(consts, b2lo, b2hi):
    nc = bacc_mod.Bacc(None)

    xs_h = nc.dram_tensor("xs", [BS, N, D], BF16, kind="ExternalInput")
    lo_h = nc.dram_tensor("lo", [BS, N, D], BF16, kind="ExternalOutput")
    hi_h = nc.dram_tensor("hi", [BS, N, D], BF16, kind="ExternalOutput")

    ch = {k: nc.inline_tensor(v, name=f"c_{k}") for k, v in consts.items()}

    Copy = mybir.ActivationFunctionType.Copy
    Sig = mybir.ActivationFunctionType.Sigmoid

    # small first group (early first stores = DMA ramps sooner) and small
    # last group (short drain+store tail); 4-image groups in the middle
    if os.environ.get("KRN_GPAT"):
        sizes = [int(v) for v in os.environ["KRN_GPAT"].split(",")]
        assert sum(sizes) == BS
    elif BS == 16 and GRP == 4:
        sizes = [2, 3, 4, 4, 3]
    else:
        sizes = [min(GRP, BS - s) for s in range(0, BS, GRP)]
    groups, s = [], 0
    for sz in sizes:
        groups.append(list(range(s, s + sz)))
        s += sz
    n_groups = len(groups)

    with tile.TileContext(nc) as tc:
        with (
            tc.tile_pool(name="consts", bufs=1) as cp,
            tc.tile_pool(name="xp", bufs=n_groups) as xp,
            tc.tile_pool(name="outp", bufs=BUFO) as outp,
            tc.tile_pool(name="gp", bufs=2) as gp,
            tc.tile_pool(name="pm", bufs=2, space="PSUM") as pm,
            tc.tile_pool(name="par", bufs=2, space="PSUM") as par,
        ):
            def cload(key, dtype):
                arr = consts[key]
                t = cp.tile(list(arr.shape), dtype, tag=key)
                nc.sync.dma_start(out=t[:], in_=ch[key][...])
                return t

            wb = cload("wtblob", BF16)      # [128, 400]
            onescol = wb[:, ONES0:ONES0 + 1]

            def load_group(g):
                bs = groups[g]
                b0, gn = bs[0], len(bs)
                xga = xp.tile([128, GRP, D], BF16, tag="xga")
                xgb = xp.tile([68, GRP, D], BF16, tag="xgb")
                nc.sync.dma_start(
                    out=xga[:, 0:gn, :],
                    in_=xs_h[b0:b0 + gn, 1:129, :].rearrange("b t d -> t b d"))
                nc.sync.dma_start(
                    out=xgb[:, 0:gn, :],
                    in_=xs_h[b0:b0 + gn, 129:197, :].rearrange("b t d -> t b d"))
                return xga, xgb

            # first groups' loads go before the remaining consts so their
            # gate chains start as early as possible; all loads precede all
            # stores so SP's in-order sequencer never parks a semaphore-
            # blocked store ahead of a ready load
            xt = {g: load_group(g) for g in range(min(2, n_groups))}

            gb = cload("gblob", BF16)       # [128, 1620]
            eyef = cload("eyef", F32)       # [16, 16] identity
            w1c = gb[:, GW1:GW1 + 1152].rearrange("p (a h b) -> p a h b",
                                                  a=6, h=2)
            w2c0 = gb[0:96, GW20:GW20 + 2]
            w2c1 = gb[0:96, GW21 + 0:GW21 + 2]
            ones1 = gb[0:1, GONES:GONES + 16]
            alr = gb[0:1, GALR:GALR + 128]
            ahr = gb[0:1, GAHR:GAHR + 128]
            cneg = gb[0:1, GCNEG:GCNEG + 128]

            # dummy activation so the act-func table load (~1.3us) runs at
            # t~=1us instead of stalling the first group's gate sigmoid
            warm = gp.tile([1, 16], F32, tag="warm")
            nc.scalar.activation(warm[:], eyef[0:1, 0:16], Sig)

            # CLS passthrough for all batches (DRAM -> DRAM), issued from
            # the near-free GPSIMD DMA queue so SP only handles bulk I/O
            nc.gpsimd.dma_start(out=lo_h[:, 0:1, :], in_=xs_h[:, 0:1, :])
            nc.gpsimd.dma_start(out=hi_h[:, 0:1, :], in_=xs_h[:, 0:1, :])

            for g in range(2, n_groups):
                xt[g] = load_group(g)

            def gate_chain(g, xga, xgb):
                bs = groups[g]
                gn = len(bs)
                # gate for this group, ahead of the main matmuls.
                # gT[d, j] = mean over patch tokens of x (tiny K-contraction
                # matmuls straight into the psum arena, [d, img] layout).
                arena = par.tile([128, 384], F32, tag="arena")
                for j in range(gn):
                    for c in range(6):
                        col = c * 16 + j
                        nc.tensor.matmul(
                            arena[:, col:col + 1],
                            xga[:, j, c * 128:(c + 1) * 128],
                            onescol[:],
                            start=True, stop=False)
                        nc.tensor.matmul(
                            arena[:, col:col + 1],
                            xgb[0:68, j, c * 128:(c + 1) * 128],
                            onescol[0:68],
                            start=False, stop=True)
                gTt = gp.tile([128, 6, 16], BF16, tag="gTt")
                nc.vector.tensor_copy(
                    gTt[:].rearrange("p a b -> p (a b)"), arena[:, 0:96])

                # hidden layer directly in transposed [feature, img] layout
                # (w1 chunks stationary): no transposes, tiny moving dims
                gate_pe = None
                for h in range(2):
                    hps = arena[0:96, 96 + 16 * h:96 + 16 * h + 16]
                    for c in range(6):
                        nc.tensor.matmul(hps[:, 0:gn], w1c[:, c, h, :],
                                         gTt[:, c, 0:gn],
                                         start=(c == 0), stop=False)
                    gate_pe = nc.tensor.matmul(
                        hps[:, 0:gn], gb[0:1, GB1 + 96 * h:GB1 + 96 * h + 96],
                        ones1[0:1, 0:gn], start=False, stop=True)
                hTt = gp.tile([96, 2, 16], BF16, tag="hTt")
                for h in range(2):
                    hps = arena[0:96, 96 + 16 * h:96 + 16 * h + 16]
                    nc.vector.tensor_relu(hTt[:, h, 0:gn], hps[:, 0:gn])

                crows = []
                for col, b2f in ((0, b2lo), (1, b2hi)):
                    g_ps = arena[0:1, 128 + 16 * col:144 + 16 * col]
                    nc.tensor.matmul(g_ps[:, 0:gn], w2c0[:, col:col + 1],
                                     hTt[:, 0, 0:gn], start=True, stop=False)
                    nc.tensor.matmul(g_ps[:, 0:gn], w2c1[:, col:col + 1],
                                     hTt[:, 1, 0:gn], start=False, stop=True)
                    cr = gp.tile([1, 16], BF16, tag=f"crow{col}")
                    nc.scalar.activation(cr[:, 0:gn], g_ps[:, 0:gn], Sig,
                                         bias=b2f)
                    crows.append(cr)
                # per-image hi/lo gate ratio (the hi path is reconstructed
                # from the already-scaled lo tile: hi = crh*x - r*lo with
                # r = crh/crl; the -alpha ratio constant lives in cneg)
                rcp = gp.tile([1, 16], F32, tag="rcp")
                nc.vector.reciprocal(rcp[:, 0:gn], crows[0][0:1, 0:gn])
                rrow = gp.tile([1, 16], BF16, tag="rrow")
                nc.vector.tensor_mul(rrow[:, 0:gn], rcp[:, 0:gn],
                                     crows[1][0:1, 0:gn])
                # replicate the gate rows across partitions; the alpha
                # sigmoid is folded into the alr/ahr/cneg weight rows
                for k, (wrow, mov) in enumerate(
                        ((alr, crows[0][0:1, 0:gn]),
                         (ahr, crows[1][0:1, 0:gn]),
                         (cneg, rrow[0:1, 0:gn]))):
                    nc.tensor.matmul(arena[:, 160 + 16 * k:160 + 16 * k + gn],
                                     wrow[0:1, :], mov, start=True, stop=True)
                crlh = gp.tile([128, 48], F32, tag="crlh")
                nc.vector.tensor_copy(crlh[:], arena[:, 160:208])
                return crlh, gate_pe

            # gates run two groups ahead of their bodies so their small
            # DVE/Act steps never queue behind a full body's engine work
            gates = {g: gate_chain(g, *xt[g]) for g in range(min(2, n_groups))}

            for g, bs in enumerate(groups):
                gn = len(bs)
                b0 = bs[0]
                xga, xgb = xt[g]
                crlh, gate_pe = gates.pop(g)
                crl = crlh[:, 0:16]
                crh = crlh[:, 16:32]
                rneg = crlh[:, 32:48]

                # ---- main matmuls + scaled drains per image
                lo_ga = outp.tile([128, GRP, D], BF16, tag="lo_ga")
                lo_gb = outp.tile([68, GRP, D], BF16, tag="lo_gb")
                for j in range(gn):
                    # PSUM packing: 3 banks/image (za1 | zb1 | za2+zb2)
                    za1 = pm.tile([128, 512], F32, tag="za1")
                    zb1 = pm.tile([68, 512], F32, tag="zb1")
                    zab2 = pm.tile([128, 512], F32, tag="zab2")
                    za_ch = [(0, 512, za1[:, :]), (512, 768, zab2[:, 0:256])]
                    zb_ch = [(0, 512, zb1[0:68, :]),
                             (512, 768, zab2[0:68, 256:512])]
                    for (n0, n1, zc) in za_ch:
                        mm = nc.tensor.matmul(zc, wb[:, 0:128],
                                              xga[:, j, n0:n1],
                                              start=True, stop=False)
                        add_dep_helper(mm.ins, gate_pe.ins,
                                       reason="gate chain schedules first")
                        nc.tensor.matmul(zc, wb[0:68, WTB0:WTB0 + 128],
                                         xgb[0:68, j, n0:n1],
                                         start=False, stop=True)
                    for (n0, n1, zc) in zb_ch:
                        mm = nc.tensor.matmul(zc, wb[:, 128:196],
                                              xga[:, j, n0:n1],
                                              start=True, stop=False)
                        add_dep_helper(mm.ins, gate_pe.ins,
                                       reason="gate chain schedules first")
                        nc.tensor.matmul(zc, wb[0:68, WTB0 + 128:WTB0 + 196],
                                         xgb[0:68, j, n0:n1],
                                         start=False, stop=True)

                    # GPSIMD cannot touch PSUM, so the legal engine split is:
                    #   Act:  all four scaled lo drains (+ gate sigmoids)
                    #   DVE:  all four hi subtracts
                    #   Pool: both hi scales (SBUF-only bf16)
                    nc.scalar.activation(lo_ga[:, j, 0:512], za1[:, :], Copy,
                                         scale=crl[:, j:j + 1])
                    nc.scalar.activation(lo_ga[:, j, 512:768],
                                         zab2[:, 0:256], Copy,
                                         scale=crl[:, j:j + 1])
                    nc.scalar.activation(lo_gb[0:68, j, 0:512], zb1[0:68, :],
                                         Copy, scale=crl[0:68, j:j + 1])
                    nc.scalar.activation(lo_gb[0:68, j, 512:768],
                                         zab2[0:68, 256:512], Copy,
                                         scale=crl[0:68, j:j + 1])
                    # hi = crh*x - (crh/crl)*lo: reconstructed from the
                    # drained lo tiles, entirely off PSUM (z has a single
                    # reader).  b-scale on GPSIMD, a-scale + both fused
                    # multiply-adds on DVE.
                    nc.gpsimd.tensor_scalar_mul(xgb[0:68, j, :],
                                                xgb[0:68, j, :],
                                                crh[0:68, j:j + 1])
                    nc.vector.tensor_scalar_mul(xga[:, j, :],
                                                xga[:, j, :],
                                                crh[:, j:j + 1])
                    nc.vector.scalar_tensor_tensor(
                        xga[:, j, :], lo_ga[:, j, :], rneg[:, j:j + 1],
                        xga[:, j, :], op0=mybir.AluOpType.mult,
                        op1=mybir.AluOpType.add)
                    nc.vector.scalar_tensor_tensor(
                        xgb[0:68, j, :], lo_gb[0:68, j, :],
                        rneg[0:68, j:j + 1], xgb[0:68, j, :],
                        op0=mybir.AluOpType.mult, op1=mybir.AluOpType.add)

                # next-next group's gate chain goes ahead of this group's
                # stores (on PE it runs while Act/DVE/Pool finish this group)
                if g + 2 < n_groups:
                    gates[g + 2] = gate_chain(g + 2, *xt[g + 2])

                # all bulk stores from SP: with every load pre-issued, SP's
                # in-order queue matches completion order (no head-of-line
                # blocking), and HWDGE issue keeps the Pool engine free of
                # the ~1us/DMA SWDGE generation cost.  2-image granularity
                # lets each half ship while the next half still drains.
                for k0 in range(0, gn, 2):
                    k1 = min(k0 + 2, gn)
                    c0, c1 = b0 + k0, b0 + k1
                    nc.sync.dma_start(
                        out=lo_h[c0:c1, 1:129, :].rearrange("b t d -> t b d"),
                        in_=lo_ga[:, k0:k1, :])
                    nc.sync.dma_start(
                        out=lo_h[c0:c1, 129:197, :].rearrange("b t d -> t b d"),
                        in_=lo_gb[:, k0:k1, :])
                    nc.sync.dma_start(
                        out=hi_h[c0:c1, 1:129, :].rearrange("b t d -> t b d"),
                        in_=xga[:, k0:k1, :])
                    nc.sync.dma_start(
                        out=hi_h[c0:c1, 129:197, :].rearrange("b t d -> t b d"),
                        in_=xgb[:, k0:k1, :])

    if not nc.is_finalized():
        nc.finalize()
    return nc


def _make_consts(OP, w1, b1, w2, alpha_low, alpha_high):
    import ml_dtypes
    sig = lambda v: 1.0 / (1.0 + np.exp(-np.float64(v)))
    WT = np.ascontiguousarray(np.asarray(OP, np.float64).T)
    wtblob = np.zeros((128, 400), np.float32)
    wtblob[0:128, 0:196] = WT[0:128]
    wtblob[0:68, 196:392] = WT[128:196]
    wtblob[:, ONES0] = 1.0 / P
    gblob = np.zeros((128, 1748), np.float32)
    gblob[:, GW1:GW1 + 1152] = np.asarray(w1, np.float32).reshape(
        6, 128, 192).transpose(1, 0, 2).reshape(128, 1152)
    gblob[0, GB1:GB1 + 192] = np.asarray(b1, np.float32)
    gblob[0:96, GW20:GW20 + 2] = np.asarray(w2, np.float32)[0:96]
    gblob[0:96, GW21:GW21 + 2] = np.asarray(w2, np.float32)[96:192]
    gblob[0, GONES:GONES + 16] = 1.0
    gblob[0, GALR:GALR + 128] = sig(alpha_low)
    gblob[0, GAHR:GAHR + 128] = sig(alpha_high)
    gblob[0, GCNEG:GCNEG + 128] = -sig(alpha_high) / sig(alpha_low)
    return {"wtblob": wtblob.astype(ml_dtypes.bfloat16),
            "gblob": gblob.astype(ml_dtypes.bfloat16),
            "eyef": np.eye(16, dtype=np.float32)}


def build_for_sim():
    """Program instance for cost-model simulation (dummy weights)."""
    import ml_dtypes
    consts = {
        "wtblob": np.zeros((128, 400), ml_dtypes.bfloat16),
        "gblob": np.ones((128, 1748), ml_dtypes.bfloat16),
        "eyef": np.eye(16, dtype=np.float32),
    }
    return _build_program(consts, 0.0, 0.0)


def kernel(x, low_params, high_params, alpha_low, alpha_high,
           w1, b1, w2, b2, cls_token_idx):
    import ml_dtypes
    assert int(cls_token_idx) == 0
    x = np.asarray(x, dtype=np.float32)
    assert x.shape == (B, N, D)

    lm = _freq_mask_np(low_params, True)
    A = _conv_operator(lm)
    share_Y = np.allclose(np.asarray(low_params, np.float32),
                          np.asarray(high_params, np.float32))
    b2v = np.asarray(b2, np.float64).reshape(2)

    xbf = np.ascontiguousarray(x.astype(ml_dtypes.bfloat16))
    xs = xbf.reshape(NCORES, BS, N, D)
    in_maps = [{"xs": np.ascontiguousarray(xs[c])} for c in range(NCORES)]

    def run_once(OP):
        consts = _make_consts(OP, w1, b1, w2, alpha_low, alpha_high)
        nc = _build_program(consts, float(b2v[0]), float(b2v[1]))
        res = run_bass_kernel_spmd(nc, in_maps, core_ids=list(range(NCORES)))
        lo = np.concatenate([np.asarray(r["lo"]) for r in res.results],
                            axis=0).astype(np.float32)
        hi = np.concatenate([np.asarray(r["hi"]) for r in res.results],
                            axis=0).astype(np.float32)
        if getattr(res, "exec_time_ns", None) is not None:
            print(f"HW exec time: {res.exec_time_ns} ns")
        return lo, hi

    if share_Y:
        return run_once(A)
    # generic case (not hit by the reference inputs): hi needs its own
    # operator; run the validated single-operator program twice
    lo, _ = run_once(A)
    Cm = _conv_operator(_freq_mask_np(high_params, True))
    _, hi = run_once(Cm)
    return lo, hi


# revision 76
# speedup vs baseline: 2.6515x; 1.0200x over previous
"""DCT Frequency Splitter — Trainium2 Bass kernel (v3, bf16 end-to-end).

Math: FFT2 -> mask -> IFFT2 -> real is a linear operator on the 196 patch
tokens (per channel): z = A @ patches with A = Re(Finv diag(m) F) (196x196,
real, built on host from the 4 mask params).  With shared mask params the
high path is high = patches - z, so one matmul feeds both outputs:
lo = s_l * z, hi = s_h * (patches - z).

v3 layout decisions (all driven by the TimelineSim cost model):
- bf16 everywhere off-chip: x is converted to bf16 on the host, outputs are
  stored bf16 and upcast on the host.  Halves DMA traffic (the kernel is
  DMA-bound at ~360 GB/s/core); rel-err budget 2e-2 >> bf16's ~4e-3.
- gate-first: per-image token means are computed straight from the x tiles
  with tiny K-contraction matmuls into a PSUM arena (gT in [d, img] layout,
  no PSUM row drain + transpose shuffle), so the gate scales are ready when
  the main matmuls drain and the lo output leaves PSUM already scaled - one
  Activation pass instead of two.
- engine split per image: Act = scaled lo drains, DVE = hi subs + hi scale
  (a-part), Pool/GPSIMD = hi scale (b-part), PE = matmuls.
- group-batched DMAs (4 images per DMA) to keep the SP sequencer's ~1.1us
  per-DMA issue cost off the critical path.

Sharding: pure data parallel, batch 128 -> 16 per core across 8 cores.
"""

import os
import numpy as np

import concourse.bass as bass
import concourse.bacc as bacc_mod
import concourse.mybir as mybir
import concourse.tile as tile
from concourse.bass_utils import run_bass_kernel_spmd
from concourse.tile_rust import add_dep_helper

H, W = 14, 14
B, N, D = 128, 197, 768
P = 196  # patch tokens
NCORES = 8
BS = B // NCORES  # batches per core

GRP = int(os.environ.get("KRN_GROUP", "4"))        # images per group
BUFO = int(os.environ.get("KRN_BUFO", "2"))        # lo-tile group buffers
F32 = mybir.dt.float32
BF16 = mybir.dt.bfloat16

# x tiles hold tokens 1..196 only (CLS skipped at load): xga = patches
# 0..127 on partitions 0..127, xgb = patches 128..195 on partitions 0..67,
# so matmul operands/outputs and the elementwise hi ops all share base
# partition 0 (the PE requires operand base partition in {0, 32, 64}).
# wtblob column layout (bf16, [128, 400])
WTA0 = 0      # A^T rows 0:128 (K = patches 0..127), cols 0:196
WTB0 = 196    # A^T rows 128:196 (K = patches 128..195), cols 196:392
ONES0 = 392   # [128, 1] column of 1/196
# gblob column layout (bf16, [128, 1748])
GW1, GB1 = 0, 1152
GW20, GW21 = 1344, 1346
GONES = 1348
GALR, GAHR = 1364, 1492
GCNEG = 1620  # -sig(alpha_high)/sig(alpha_low) row for the hi-ratio


def _freq_mask_np(params, low):
    ch, cw, radius, sharp = [np.float64(v) for v in np.asarray(params)]
    y = np.arange(H, dtype=np.float64)
    x = np.arange(W, dtype=np.float64)
    d2 = (y[:, None] - ch) ** 2 + (x[None, :] - cw) ** 2
    dist = np.sqrt(d2 + 1e-12)
    s = np.clip(sharp, 0.5, 10.0)
    r = np.clip(radius, 1.0, min(H, W) / 2.0)
    m = np.exp(-((dist / r) ** s))
    return m if low else 1.0 - m


def _conv_operator(mask):
    """Real 196x196 operator equivalent to ifft2(fft2(img)*mask).real."""
    F_H = np.exp(-2j * np.pi * np.outer(np.arange(H), np.arange(H)) / H)
    F_W = np.exp(-2j * np.pi * np.outer(np.arange(W), np.arange(W)) / W)
    Fi_H = np.conj(F_H) / H
    Fi_W = np.conj(F_W) / W
    op = np.kron(Fi_H, Fi_W) @ np.diag(mask.ravel()) @ np.kron(F_H, F_W)
    return np.real(op)


def _build_program(consts, b2lo, b2hi):
    nc = bacc_mod.Bacc(None)

    xs_h = nc.dram_tensor("xs", [BS, N, D], BF16, kind="ExternalInput")
    lo_h = nc.dram_tensor("lo", [BS, N, D], BF16, kind="ExternalOutput")
    hi_h = nc.dram_tensor("hi", [BS, N, D], BF16, kind="ExternalOutput")

    ch = {k: nc.inline_tensor(v, name=f"c_{k}") for k, v in consts.items()}

    Copy = mybir.ActivationFunctionType.Copy
    Sig = mybir.ActivationFunctionType.Sigmoid

    # small first group (early first stores = DMA ramps sooner) and small
    # last group (short drain+store tail); 4-image groups in the middle
    if os.environ.get("KRN_GPAT"):
        sizes = [int(v) for v in os.environ["KRN_GPAT"].split(",")]
        assert sum(sizes) == BS
    else:
        sizes = [min(GRP, BS - s) for s in range(0, BS, GRP)]
    groups, s = [], 0
    for sz in sizes:
        groups.append(list(range(s, s + sz)))
        s += sz
    n_groups = len(groups)

    with tile.TileContext(nc) as tc:
        with (
            tc.tile_pool(name="consts", bufs=1) as cp,
            tc.tile_pool(name="xp", bufs=n_groups) as xp,
            tc.tile_pool(name="outp", bufs=BUFO) as outp,
            tc.tile_pool(name="scr", bufs=2) as scr,
            tc.tile_pool(name="gp", bufs=2) as gp,
            tc.tile_pool(name="pm", bufs=2, space="PSUM") as pm,
            tc.tile_pool(name="par", bufs=2, space="PSUM") as par,
        ):
            def cload(key, dtype):
                arr = consts[key]
                t = cp.tile(list(arr.shape), dtype, tag=key)
                nc.sync.dma_start(out=t[:], in_=ch[key][...])
                return t

            wb = cload("wtblob", BF16)      # [128, 400]
            onescol = wb[:, ONES0:ONES0 + 1]

            def load_group(g):
                bs = groups[g]
                b0, gn = bs[0], len(bs)
                xga = xp.tile([128, GRP, D], BF16, tag="xga")
                xgb = xp.tile([68, GRP, D], BF16, tag="xgb")
                nc.sync.dma_start(
                    out=xga[:, 0:gn, :],
                    in_=xs_h[b0:b0 + gn, 1:129, :].rearrange("b t d -> t b d"))
                nc.sync.dma_start(
                    out=xgb[:, 0:gn, :],
                    in_=xs_h[b0:b0 + gn, 129:197, :].rearrange("b t d -> t b d"))
                return xga, xgb

            # first groups' loads go before the remaining consts so their
            # gate chains start as early as possible; all loads precede all
            # stores so SP's in-order sequencer never parks a semaphore-
            # blocked store ahead of a ready load
            xt = {g: load_group(g) for g in range(min(2, n_groups))}

            gb = cload("gblob", BF16)       # [128, 1620]
            eyef = cload("eyef", F32)       # [16, 16] identity
            w1c = gb[:, GW1:GW1 + 1152].rearrange("p (a h b) -> p a h b",
                                                  a=6, h=2)
            w2c0 = gb[0:96, GW20:GW20 + 2]
            w2c1 = gb[0:96, GW21 + 0:GW21 + 2]
            ones1 = gb[0:1, GONES:GONES + 16]
            alr = gb[0:1, GALR:GALR + 128]
            ahr = gb[0:1, GAHR:GAHR + 128]
            cneg = gb[0:1, GCNEG:GCNEG + 128]

            # dummy activation so the act-func table load (~1.3us) runs at
            # t~=1us instead of stalling the first group's gate sigmoid
            warm = gp.tile([1, 16], F32, tag="warm")
            nc.scalar.activation(warm[:], eyef[0:1, 0:16], Sig)

            # CLS passthrough for all batches (DRAM -> DRAM), issued from
            # the near-free GPSIMD DMA queue so SP only handles bulk I/O
            nc.gpsimd.dma_start(out=lo_h[:, 0:1, :], in_=xs_h[:, 0:1, :])
            nc.gpsimd.dma_start(out=hi_h[:, 0:1, :], in_=xs_h[:, 0:1, :])

            for g in range(2, n_groups):
                xt[g] = load_group(g)

            def gate_chain(g, xga, xgb):
                bs = groups[g]
                gn = len(bs)
                # gate for this group, ahead of the main matmuls.
                # gT[d, j] = mean over patch tokens of x (tiny K-contraction
                # matmuls straight into the psum arena, [d, img] layout).
                arena = par.tile([128, 384], F32, tag="arena")
                for j in range(gn):
                    for c in range(6):
                        col = c * 16 + j
                        nc.tensor.matmul(
                            arena[:, col:col + 1],
                            xga[:, j, c * 128:(c + 1) * 128],
                            onescol[:],
                            start=True, stop=False)
                        nc.tensor.matmul(
                            arena[:, col:col + 1],
                            xgb[0:68, j, c * 128:(c + 1) * 128],
                            onescol[0:68],
                            start=False, stop=True)
                gTt = gp.tile([128, 6, 16], BF16, tag="gTt")
                nc.vector.tensor_copy(
                    gTt[:].rearrange("p a b -> p (a b)"), arena[:, 0:96])

                # hidden layer directly in transposed [feature, img] layout
                # (w1 chunks stationary): no transposes, tiny moving dims
                gate_pe = None
                for h in range(2):
                    hps = arena[0:96, 96 + 16 * h:96 + 16 * h + 16]
                    for c in range(6):
                        nc.tensor.matmul(hps[:, 0:gn], w1c[:, c, h, :],
                                         gTt[:, c, 0:gn],
                                         start=(c == 0), stop=False)
                    gate_pe = nc.tensor.matmul(
                        hps[:, 0:gn], gb[0:1, GB1 + 96 * h:GB1 + 96 * h + 96],
                        ones1[0:1, 0:gn], start=False, stop=True)
                hTt = gp.tile([96, 2, 16], BF16, tag="hTt")
                for h in range(2):
                    hps = arena[0:96, 96 + 16 * h:96 + 16 * h + 16]
                    nc.vector.tensor_relu(hTt[:, h, 0:gn], hps[:, 0:gn])

                crows = []
                for col, b2f in ((0, b2lo), (1, b2hi)):
                    g_ps = arena[0:1, 128 + 16 * col:144 + 16 * col]
                    nc.tensor.matmul(g_ps[:, 0:gn], w2c0[:, col:col + 1],
                                     hTt[:, 0, 0:gn], start=True, stop=False)
                    nc.tensor.matmul(g_ps[:, 0:gn], w2c1[:, col:col + 1],
                                     hTt[:, 1, 0:gn], start=False, stop=True)
                    cr = gp.tile([1, 16], BF16, tag=f"crow{col}")
                    nc.scalar.activation(cr[:, 0:gn], g_ps[:, 0:gn], Sig,
                                         bias=b2f)
                    crows.append(cr)
                # per-image hi/lo gate ratio (the hi path is reconstructed
                # from the already-scaled lo tile: hi = crh*x - r*lo with
                # r = crh/crl; the -alpha ratio constant lives in cneg)
                rcp = gp.tile([1, 16], F32, tag="rcp")
                nc.vector.reciprocal(rcp[:, 0:gn], crows[0][0:1, 0:gn])
                rrow = gp.tile([1, 16], BF16, tag="rrow")
                nc.vector.tensor_mul(rrow[:, 0:gn], rcp[:, 0:gn],
                                     crows[1][0:1, 0:gn])
                # replicate the gate rows across partitions; the alpha
                # sigmoid is folded into the alr/ahr/cneg weight rows
                for k, (wrow, mov) in enumerate(
                        ((alr, crows[0][0:1, 0:gn]),
                         (ahr, crows[1][0:1, 0:gn]),
                         (cneg, rrow[0:1, 0:gn]))):
                    nc.tensor.matmul(arena[:, 160 + 16 * k:160 + 16 * k + gn],
                                     wrow[0:1, :], mov, start=True, stop=True)
                crlh = gp.tile([128, 48], F32, tag="crlh")
                nc.vector.tensor_copy(crlh[:], arena[:, 160:208])
                return crlh, gate_pe

            # gates run two groups ahead of their bodies so their small
            # DVE/Act steps never queue behind a full body's engine work
            gates = {g: gate_chain(g, *xt[g]) for g in range(min(2, n_groups))}

            for g, bs in enumerate(groups):
                gn = len(bs)
                b0 = bs[0]
                xga, xgb = xt[g]
                crlh, gate_pe = gates.pop(g)
                crl = crlh[:, 0:16]
                crh = crlh[:, 16:32]
                rneg = crlh[:, 32:48]

                # ---- main matmuls + scaled drains per image
                lo_ga = outp.tile([128, GRP, D], BF16, tag="lo_ga")
                lo_gb = outp.tile([68, GRP, D], BF16, tag="lo_gb")
                for j in range(gn):
                    # PSUM packing: 3 banks/image (za1 | zb1 | za2+zb2)
                    za1 = pm.tile([128, 512], F32, tag="za1")
                    zb1 = pm.tile([68, 512], F32, tag="zb1")
                    zab2 = pm.tile([128, 512], F32, tag="zab2")
                    za_ch = [(0, 512, za1[:, :]), (512, 768, zab2[:, 0:256])]
                    zb_ch = [(0, 512, zb1[0:68, :]),
                             (512, 768, zab2[0:68, 256:512])]
                    for (n0, n1, zc) in za_ch:
                        mm = nc.tensor.matmul(zc, wb[:, 0:128],
                                              xga[:, j, n0:n1],
                                              start=True, stop=False)
                        if g > 0:
                            add_dep_helper(mm.ins, gate_pe.ins,
                                           reason="gate chain schedules first")
                        nc.tensor.matmul(zc, wb[0:68, WTB0:WTB0 + 128],
                                         xgb[0:68, j, n0:n1],
                                         start=False, stop=True)
                    for (n0, n1, zc) in zb_ch:
                        mm = nc.tensor.matmul(zc, wb[:, 128:196],
                                              xga[:, j, n0:n1],
                                              start=True, stop=False)
                        if g > 0:
                            add_dep_helper(mm.ins, gate_pe.ins,
                                           reason="gate chain schedules first")
                        nc.tensor.matmul(zc, wb[0:68, WTB0 + 128:WTB0 + 196],
                                         xgb[0:68, j, n0:n1],
                                         start=False, stop=True)

                    # GPSIMD cannot touch PSUM, so the legal engine split is:
                    #   Act:  all four scaled lo drains (+ gate sigmoids)
                    #   DVE:  all four hi subtracts
                    #   Pool: both hi scales (SBUF-only bf16)
                    nc.scalar.activation(lo_ga[:, j, 0:512], za1[:, :], Copy,
                                         scale=crl[:, j:j + 1])
                    nc.scalar.activation(lo_ga[:, j, 512:768],
                                         zab2[:, 0:256], Copy,
                                         scale=crl[:, j:j + 1])
                    nc.scalar.activation(lo_gb[0:68, j, 0:512], zb1[0:68, :],
                                         Copy, scale=crl[0:68, j:j + 1])
                    if j % 2 == 0:
                        nc.scalar.activation(lo_gb[0:68, j, 512:768],
                                             zab2[0:68, 256:512], Copy,
                                             scale=crl[0:68, j:j + 1])
                    else:
                        nc.vector.tensor_scalar_mul(lo_gb[0:68, j, 512:768],
                                                    zab2[0:68, 256:512],
                                                    crl[0:68, j:j + 1])
                    # hi = crh*x - (crh/crl)*lo: reconstructed from the
                    # drained lo tiles, entirely off PSUM (z has a single
                    # reader).  All-bf16-SBUF muls/adds hit DVE 2x/4x modes;
                    # the b-side x-scale rides the otherwise idle GPSIMD.
                    nc.gpsimd.tensor_scalar_mul(xgb[0:68, j, :],
                                                xgb[0:68, j, :],
                                                crh[0:68, j:j + 1])
                    nc.vector.tensor_scalar_mul(xga[:, j, :],
                                                xga[:, j, :],
                                                crh[:, j:j + 1])
                    ta = scr.tile([128, D], BF16, tag="ta")
                    tb = scr.tile([68, D], BF16, tag="tb")
                    nc.vector.tensor_scalar_mul(ta[:], lo_ga[:, j, :],
                                                rneg[:, j:j + 1])
                    nc.vector.tensor_add(xga[:, j, :], xga[:, j, :], ta[:])
                    nc.vector.tensor_scalar_mul(tb[0:68, :],
                                                lo_gb[0:68, j, :],
                                                rneg[0:68, j:j + 1])
                    nc.vector.tensor_add(xgb[0:68, j, :], xgb[0:68, j, :],
                                         tb[0:68, :])

                # next-next group's gate chain goes ahead of this group's
                # stores (on PE it runs while Act/DVE/Pool finish this group)
                if g + 2 < n_groups:
                    gates[g + 2] = gate_chain(g + 2, *xt[g + 2])

                # all bulk stores from SP: with every load pre-issued, SP's
                # in-order queue matches completion order (no head-of-line
                # blocking), and HWDGE issue keeps the Pool engine free of
                # the ~1us/DMA SWDGE generation cost.  2-image granularity
                # lets each half ship while the next half still drains.
                for k0 in range(0, gn, 2):
                    k1 = min(k0 + 2, gn)
                    c0, c1 = b0 + k0, b0 + k1
                    nc.sync.dma_start(
                        out=lo_h[c0:c1, 1:129, :].rearrange("b t d -> t b d"),
                        in_=lo_ga[:, k0:k1, :])
                    nc.sync.dma_start(
                        out=lo_h[c0:c1, 129:197, :].rearrange("b t d -> t b d"),
                        in_=lo_gb[:, k0:k1, :])
                    nc.sync.dma_start(
                        out=hi_h[c0:c1, 1:129, :].rearrange("b t d -> t b d"),
                        in_=xga[:, k0:k1, :])
                    nc.sync.dma_start(
                        out=hi_h[c0:c1, 129:197, :].rearrange("b t d -> t b d"),
                        in_=xgb[:, k0:k1, :])

    if not nc.is_finalized():
        nc.finalize()
    return nc


def _make_consts(OP, w1, b1, w2, alpha_low, alpha_high):
    import ml_dtypes
    sig = lambda v: 1.0 / (1.0 + np.exp(-np.float64(v)))
    WT = np.ascontiguousarray(np.asarray(OP, np.float64).T)
    wtblob = np.zeros((128, 400), np.float32)
    wtblob[0:128, 0:196] = WT[0:128]
    wtblob[0:68, 196:392] = WT[128:196]
    wtblob[:, ONES0] = 1.0 / P
    gblob = np.zeros((128, 1748), np.float32)
    gblob[:, GW1:GW1 + 1152] = np.asarray(w1, np.float32).reshape(
        6, 128, 192).transpose(1, 0, 2).reshape(128, 1152)
    gblob[0, GB1:GB1 + 192] = np.asarray(b1, np.float32)
    gblob[0:96, GW20:GW20 + 2] = np.asarray(w2, np.float32)[0:96]
    gblob[0:96, GW21:GW21 + 2] = np.asarray(w2, np.float32)[96:192]
    gblob[0, GONES:GONES + 16] = 1.0
    gblob[0, GALR:GALR + 128] = sig(alpha_low)
    gblob[0, GAHR:GAHR + 128] = sig(alpha_high)
    gblob[0, GCNEG:GCNEG + 128] = -sig(alpha_high) / sig(alpha_low)
    return {"wtblob": wtblob.astype(ml_dtypes.bfloat16),
            "gblob": gblob.astype(ml_dtypes.bfloat16),
            "eyef": np.eye(16, dtype=np.float32)}


def build_for_sim():
    """Program instance for cost-model simulation (dummy weights)."""
    import ml_dtypes
    consts = {
        "wtblob": np.zeros((128, 400), ml_dtypes.bfloat16),
        "gblob": np.ones((128, 1748), ml_dtypes.bfloat16),
        "eyef": np.eye(16, dtype=np.float32),
    }
    return _build_program(consts, 0.0, 0.0)


def kernel(x, low_params, high_params, alpha_low, alpha_high,
           w1, b1, w2, b2, cls_token_idx):
    import ml_dtypes
    assert int(cls_token_idx) == 0
    x = np.asarray(x, dtype=np.float32)
    assert x.shape == (B, N, D)

    lm = _freq_mask_np(low_params, True)
    A = _conv_operator(lm)
    share_Y = np.allclose(np.asarray(low_params, np.float32),
                          np.asarray(high_params, np.float32))
    b2v = np.asarray(b2, np.float64).reshape(2)

    xbf = np.ascontiguousarray(x.astype(ml_dtypes.bfloat16))
    xs = xbf.reshape(NCORES, BS, N, D)
    in_maps = [{"xs": np.ascontiguousarray(xs[c])} for c in range(NCORES)]

    def run_once(OP):
        consts = _make_consts(OP, w1, b1, w2, alpha_low, alpha_high)
        nc = _build_program(consts, float(b2v[0]), float(b2v[1]))
        res = run_bass_kernel_spmd(nc, in_maps, core_ids=list(range(NCORES)))
        lo = np.concatenate([np.asarray(r["lo"]) for r in res.results],
                            axis=0).astype(np.float32)
        hi = np.concatenate([np.asarray(r["hi"]) for r in res.results],
                            axis=0).astype(np.float32)
        if getattr(res, "exec_time_ns", None) is not None:
            print(f"HW exec time: {res.exec_time_ns} ns")
        return lo, hi

    if share_Y:
        return run_once(A)
    # generic case (not hit by the reference inputs): hi needs its own
    # operator; run the validated single-operator program twice
    lo, _ = run_once(A)
    Cm = _conv_operator(_freq_mask_np(high_params, True))
    _, hi = run_once(Cm)
    return lo, hi


# revision 97
# speedup vs baseline: 2.6991x; 1.0180x over previous
"""DCT Frequency Splitter — Trainium2 Bass kernel (v3, bf16 end-to-end).

Math: FFT2 -> mask -> IFFT2 -> real is a linear operator on the 196 patch
tokens (per channel): z = A @ patches with A = Re(Finv diag(m) F) (196x196,
real, built on host from the 4 mask params).  With shared mask params the
high path is high = patches - z, so one matmul feeds both outputs:
lo = s_l * z, hi = s_h * (patches - z).

v3 layout decisions (all driven by the TimelineSim cost model):
- bf16 everywhere off-chip: x is converted to bf16 on the host, outputs are
  stored bf16 and upcast on the host.  Halves DMA traffic (the kernel is
  DMA-bound at ~360 GB/s/core); rel-err budget 2e-2 >> bf16's ~4e-3.
- gate-first: per-image token means are computed straight from the x tiles
  with tiny K-contraction matmuls into a PSUM arena (gT in [d, img] layout,
  no PSUM row drain + transpose shuffle), so the gate scales are ready when
  the main matmuls drain and the lo output leaves PSUM already scaled - one
  Activation pass instead of two.
- engine split per image: Act = scaled lo drains, DVE = hi subs + hi scale
  (a-part), Pool/GPSIMD = hi scale (b-part), PE = matmuls.
- group-batched DMAs (4 images per DMA) to keep the SP sequencer's ~1.1us
  per-DMA issue cost off the critical path.

Sharding: pure data parallel, batch 128 -> 16 per core across 8 cores.
"""

import os
import numpy as np

import concourse.bass as bass
import concourse.bacc as bacc_mod
import concourse.mybir as mybir
import concourse.tile as tile
from concourse.bass_utils import run_bass_kernel_spmd
from concourse.tile_rust import add_dep_helper

H, W = 14, 14
B, N, D = 128, 197, 768
P = 196  # patch tokens
NCORES = 8
BS = B // NCORES  # batches per core

GRP = int(os.environ.get("KRN_GROUP", "4"))        # images per group
BUFO = int(os.environ.get("KRN_BUFO", "3"))        # output half-tile buffers
F32 = mybir.dt.float32
BF16 = mybir.dt.bfloat16

# x tiles hold tokens 1..196 only (CLS skipped at load): xga = patches
# 0..127 on partitions 0..127, xgb = patches 128..195 on partitions 0..67,
# so matmul operands/outputs and the elementwise hi ops all share base
# partition 0 (the PE requires operand base partition in {0, 32, 64}).
# wtblob column layout (bf16, [128, 400])
WTA0 = 0      # A^T rows 0:128 (K = patches 0..127), cols 0:196
WTB0 = 196    # A^T rows 128:196 (K = patches 128..195), cols 196:392
ONES0 = 392   # [128, 1] column of 1/196
# gblob column layout (bf16, [128, 1748])
GW1, GB1 = 0, 1152
GW20, GW21 = 1344, 1346
GONES = 1348
GALR, GAHR = 1364, 1492
GCNEG = 1620  # -sig(alpha_high)/sig(alpha_low) row for the hi-ratio


def _freq_mask_np(params, low):
    ch, cw, radius, sharp = [np.float64(v) for v in np.asarray(params)]
    y = np.arange(H, dtype=np.float64)
    x = np.arange(W, dtype=np.float64)
    d2 = (y[:, None] - ch) ** 2 + (x[None, :] - cw) ** 2
    dist = np.sqrt(d2 + 1e-12)
    s = np.clip(sharp, 0.5, 10.0)
    r = np.clip(radius, 1.0, min(H, W) / 2.0)
    m = np.exp(-((dist / r) ** s))
    return m if low else 1.0 - m


def _conv_operator(mask):
    """Real 196x196 operator equivalent to ifft2(fft2(img)*mask).real."""
    F_H = np.exp(-2j * np.pi * np.outer(np.arange(H), np.arange(H)) / H)
    F_W = np.exp(-2j * np.pi * np.outer(np.arange(W), np.arange(W)) / W)
    Fi_H = np.conj(F_H) / H
    Fi_W = np.conj(F_W) / W
    op = np.kron(Fi_H, Fi_W) @ np.diag(mask.ravel()) @ np.kron(F_H, F_W)
    return np.real(op)


def _build_program(consts, b2lo, b2hi):
    nc = bacc_mod.Bacc(None)

    xs_h = nc.dram_tensor("xs", [BS, N, D], BF16, kind="ExternalInput")
    lo_h = nc.dram_tensor("lo", [BS, N, D], BF16, kind="ExternalOutput")
    hi_h = nc.dram_tensor("hi", [BS, N, D], BF16, kind="ExternalOutput")

    ch = {k: nc.inline_tensor(v, name=f"c_{k}") for k, v in consts.items()}

    Copy = mybir.ActivationFunctionType.Copy
    Sig = mybir.ActivationFunctionType.Sigmoid

    # small first group (early first stores = DMA ramps sooner) and small
    # last group (short drain+store tail); 4-image groups in the middle
    if os.environ.get("KRN_GPAT"):
        sizes = [int(v) for v in os.environ["KRN_GPAT"].split(",")]
        assert sum(sizes) == BS
    else:
        sizes = [min(GRP, BS - s) for s in range(0, BS, GRP)]
    groups, s = [], 0
    for sz in sizes:
        groups.append(list(range(s, s + sz)))
        s += sz
    n_groups = len(groups)

    with tile.TileContext(nc) as tc:
        with (
            tc.tile_pool(name="consts", bufs=1) as cp,
            tc.tile_pool(name="xp", bufs=n_groups) as xp,
            tc.tile_pool(name="outp", bufs=BUFO) as outp,
            tc.tile_pool(name="scr", bufs=2) as scr,
            tc.tile_pool(name="gp", bufs=2) as gp,
            tc.tile_pool(name="pm", bufs=2, space="PSUM") as pm,
            tc.tile_pool(name="par", bufs=2, space="PSUM") as par,
        ):
            def cload(key, dtype):
                arr = consts[key]
                t = cp.tile(list(arr.shape), dtype, tag=key)
                nc.sync.dma_start(out=t[:], in_=ch[key][...])
                return t

            wb = cload("wtblob", BF16)      # [128, 400]
            onescol = wb[:, ONES0:ONES0 + 1]

            def load_group(g):
                bs = groups[g]
                b0, gn = bs[0], len(bs)
                xga = xp.tile([128, GRP, D], BF16, tag="xga")
                xgb = xp.tile([68, GRP, D], BF16, tag="xgb")
                nc.sync.dma_start(
                    out=xga[:, 0:gn, :],
                    in_=xs_h[b0:b0 + gn, 1:129, :].rearrange("b t d -> t b d"))
                nc.sync.dma_start(
                    out=xgb[:, 0:gn, :],
                    in_=xs_h[b0:b0 + gn, 129:197, :].rearrange("b t d -> t b d"))
                return xga, xgb

            # group 0's loads go before the remaining consts so its gate
            # chain starts as early as possible; all loads precede all
            # stores so SP's in-order sequencer never parks a semaphore-
            # blocked store ahead of a ready load
            xt = {0: load_group(0)}

            gb = cload("gblob", BF16)       # [128, 1748]
            w1c = gb[:, GW1:GW1 + 1152].rearrange("p (a h b) -> p a h b",
                                                  a=6, h=2)
            w2c0 = gb[0:96, GW20:GW20 + 2]
            w2c1 = gb[0:96, GW21 + 0:GW21 + 2]
            ones1 = gb[0:1, GONES:GONES + 16]
            alr = gb[0:1, GALR:GALR + 128]
            ahr = gb[0:1, GAHR:GAHR + 128]
            cneg = gb[0:1, GCNEG:GCNEG + 128]

            # dummy activation so the act-func table load (~1.3us) runs at
            # t~=1us instead of stalling the first group's gate sigmoid;
            # reads the earliest const so it never blocks Act's queue
            warm = gp.tile([1, 16], F32, tag="warm")
            nc.scalar.activation(warm[:], wb[0:1, 0:16], Sig)

            # CLS passthrough for all batches (DRAM -> DRAM), issued from
            # the near-free GPSIMD DMA queue so SP only handles bulk I/O
            nc.gpsimd.dma_start(out=lo_h[:, 0:1, :], in_=xs_h[:, 0:1, :])
            nc.gpsimd.dma_start(out=hi_h[:, 0:1, :], in_=xs_h[:, 0:1, :])

            for g in range(1, n_groups):
                xt[g] = load_group(g)

            def gate_chain(g, xga, xgb):
                bs = groups[g]
                gn = len(bs)
                # gate for this group, ahead of the main matmuls.
                # gT[d, j] = mean over patch tokens of x (tiny K-contraction
                # matmuls straight into the psum arena, [d, img] layout).
                arena = par.tile([128, 384], F32, tag="arena")
                for j in range(gn):
                    for c in range(6):
                        col = c * 16 + j
                        nc.tensor.matmul(
                            arena[:, col:col + 1],
                            xga[:, j, c * 128:(c + 1) * 128],
                            onescol[:],
                            start=True, stop=False)
                        nc.tensor.matmul(
                            arena[:, col:col + 1],
                            xgb[0:68, j, c * 128:(c + 1) * 128],
                            onescol[0:68],
                            start=False, stop=True)
                gTt = gp.tile([128, 6, 16], BF16, tag="gTt")
                nc.vector.tensor_copy(
                    gTt[:].rearrange("p a b -> p (a b)"), arena[:, 0:96])

                # hidden layer directly in transposed [feature, img] layout
                # (w1 chunks stationary): no transposes, tiny moving dims
                gate_pe = None
                for h in range(2):
                    hps = arena[0:96, 96 + 16 * h:96 + 16 * h + 16]
                    for c in range(6):
                        nc.tensor.matmul(hps[:, 0:gn], w1c[:, c, h, :],
                                         gTt[:, c, 0:gn],
                                         start=(c == 0), stop=False)
                    gate_pe = nc.tensor.matmul(
                        hps[:, 0:gn], gb[0:1, GB1 + 96 * h:GB1 + 96 * h + 96],
                        ones1[0:1, 0:gn], start=False, stop=True)
                hTt = gp.tile([96, 2, 16], BF16, tag="hTt")
                for h in range(2):
                    hps = arena[0:96, 96 + 16 * h:96 + 16 * h + 16]
                    nc.vector.tensor_relu(hTt[:, h, 0:gn], hps[:, 0:gn])

                crows = []
                for col, b2f in ((0, b2lo), (1, b2hi)):
                    g_ps = arena[0:1, 128 + 16 * col:144 + 16 * col]
                    nc.tensor.matmul(g_ps[:, 0:gn], w2c0[:, col:col + 1],
                                     hTt[:, 0, 0:gn], start=True, stop=False)
                    nc.tensor.matmul(g_ps[:, 0:gn], w2c1[:, col:col + 1],
                                     hTt[:, 1, 0:gn], start=False, stop=True)
                    cr = gp.tile([1, 16], BF16, tag=f"crow{col}")
                    nc.scalar.activation(cr[:, 0:gn], g_ps[:, 0:gn], Sig,
                                         bias=b2f)
                    crows.append(cr)
                # per-image hi/lo gate ratio (the hi path is reconstructed
                # from the already-scaled lo tile: hi = crh*x - r*lo with
                # r = crh/crl; the -alpha ratio constant lives in cneg)
                rcp = gp.tile([1, 16], F32, tag="rcp")
                nc.vector.reciprocal(rcp[:, 0:gn], crows[0][0:1, 0:gn])
                rrow = gp.tile([1, 16], BF16, tag="rrow")
                nc.vector.tensor_mul(rrow[:, 0:gn], rcp[:, 0:gn],
                                     crows[1][0:1, 0:gn])
                # replicate the gate rows across partitions; the alpha
                # sigmoid is folded into the alr/ahr/cneg weight rows
                for k, (wrow, mov) in enumerate(
                        ((alr, crows[0][0:1, 0:gn]),
                         (ahr, crows[1][0:1, 0:gn]),
                         (cneg, rrow[0:1, 0:gn]))):
                    nc.tensor.matmul(arena[:, 160 + 16 * k:160 + 16 * k + gn],
                                     wrow[0:1, :], mov, start=True, stop=True)
                crlh = gp.tile([128, 48], F32, tag="crlh")
                nc.vector.tensor_copy(crlh[:], arena[:, 160:208])
                return crlh, gate_pe

            # gates run two groups ahead of their bodies so their small
            # DVE/Act steps never queue behind a full body's engine work
            gates = {g: gate_chain(g, *xt[g]) for g in range(min(2, n_groups))}

            for g, bs in enumerate(groups):
                gn = len(bs)
                b0 = bs[0]
                xga, xgb = xt[g]
                crlh, gate_pe = gates.pop(g)
                crl = crlh[:, 0:16]
                crh = crlh[:, 16:32]
                rneg = crlh[:, 32:48]

                # ---- main matmuls + scaled drains per image
                for j in range(gn):
                    # per-2-image output tiles: a store only waits on its
                    # own half's drains/adds (tile-granular dependency
                    # tracking would otherwise park it behind the whole
                    # group), and dedicated hi tiles release the x tiles to
                    # the pool at the final add rather than at the store
                    jj = j % 2
                    if jj == 0:
                        lo_ga = outp.tile([128, 2, D], BF16, tag="lo_ga")
                        lo_gb = outp.tile([68, 2, D], BF16, tag="lo_gb")
                        ho_ga = outp.tile([128, 2, D], BF16, tag="ho_ga")
                        ho_gb = outp.tile([68, 2, D], BF16, tag="ho_gb")
                    # PSUM packing: 3 banks/image — za_t[128,1024] holds
                    # za(0:768) + zb's tail chunk (768:1024), zb1 the rest.
                    # The a-side then drains in ONE 768-col Act pass.
                    za_t = pm.tile([128, 1024], F32, tag="za_t")
                    zb1 = pm.tile([68, 512], F32, tag="zb1")
                    za_ch = [(0, 512, za_t[:, 0:512]),
                             (512, 768, za_t[:, 512:768])]
                    zb_ch = [(0, 512, zb1[0:68, :]),
                             (512, 768, za_t[0:68, 768:1024])]
                    for (n0, n1, zc) in za_ch:
                        mm = nc.tensor.matmul(zc, wb[:, 0:128],
                                              xga[:, j, n0:n1],
                                              start=True, stop=False)
                        add_dep_helper(mm.ins, gate_pe.ins,
                                       reason="gate chain schedules first")
                        nc.tensor.matmul(zc, wb[0:68, WTB0:WTB0 + 128],
                                         xgb[0:68, j, n0:n1],
                                         start=False, stop=True)
                    for (n0, n1, zc) in zb_ch:
                        mm = nc.tensor.matmul(zc, wb[:, 128:196],
                                              xga[:, j, n0:n1],
                                              start=True, stop=False)
                        add_dep_helper(mm.ins, gate_pe.ins,
                                       reason="gate chain schedules first")
                        nc.tensor.matmul(zc, wb[0:68, WTB0 + 128:WTB0 + 196],
                                         xgb[0:68, j, n0:n1],
                                         start=False, stop=True)

                    # GPSIMD cannot touch PSUM, so the legal engine split is:
                    #   Act:  scaled lo drains (+ gate sigmoids)
                    #   DVE:  hi reconstruction + every other b2 drain
                    #   Pool: b-side hi scale (SBUF-only bf16)
                    nc.scalar.activation(lo_ga[:, jj, :], za_t[:, 0:768],
                                         Copy, scale=crl[:, j:j + 1])
                    nc.scalar.activation(lo_gb[0:68, jj, 0:512], zb1[0:68, :],
                                         Copy, scale=crl[0:68, j:j + 1])
                    nc.scalar.activation(lo_gb[0:68, jj, 512:768],
                                         za_t[0:68, 768:1024], Copy,
                                         scale=crl[0:68, j:j + 1])
                    # hi = crh*x - (crh/crl)*lo: reconstructed from the
                    # drained lo tiles, entirely off PSUM (z has a single
                    # reader).  All-bf16-SBUF muls/adds hit DVE 2x/4x modes;
                    # the b-side x-scale rides the otherwise idle GPSIMD.
                    nc.gpsimd.tensor_scalar_mul(xgb[0:68, j, :],
                                                xgb[0:68, j, :],
                                                crh[0:68, j:j + 1])
                    nc.vector.tensor_scalar_mul(xga[:, j, :],
                                                xga[:, j, :],
                                                crh[:, j:j + 1])
                    ta = scr.tile([128, D], BF16, tag="ta")
                    tb = scr.tile([68, D], BF16, tag="tb")
                    nc.vector.tensor_scalar_mul(ta[:], lo_ga[:, jj, :],
                                                rneg[:, j:j + 1])
                    nc.vector.tensor_add(ho_ga[:, jj, :], xga[:, j, :],
                                         ta[:])
                    nc.vector.tensor_scalar_mul(tb[0:68, :],
                                                lo_gb[0:68, jj, :],
                                                rneg[0:68, j:j + 1])
                    nc.vector.tensor_add(ho_gb[0:68, jj, :], xgb[0:68, j, :],
                                         tb[0:68, :])

                    # ship each completed half immediately (stores from SP:
                    # all loads were pre-issued, so the in-order queue never
                    # parks a blocked store ahead of a ready load)
                    if jj == 1 or j == gn - 1:
                        k0 = j - jj
                        c0, c1 = b0 + k0, b0 + j + 1
                        kn = j + 1 - k0
                        nc.sync.dma_start(
                            out=lo_h[c0:c1, 1:129, :].rearrange(
                                "b t d -> t b d"),
                            in_=lo_ga[:, 0:kn, :])
                        nc.sync.dma_start(
                            out=lo_h[c0:c1, 129:197, :].rearrange(
                                "b t d -> t b d"),
                            in_=lo_gb[:, 0:kn, :])
                        nc.sync.dma_start(
                            out=hi_h[c0:c1, 1:129, :].rearrange(
                                "b t d -> t b d"),
                            in_=ho_ga[:, 0:kn, :])
                        nc.sync.dma_start(
                            out=hi_h[c0:c1, 129:197, :].rearrange(
                                "b t d -> t b d"),
                            in_=ho_gb[:, 0:kn, :])

                # next-next group's gate chain goes ahead of this group's
                # last stores (on PE it runs while Act/DVE/Pool finish here)
                if g + 2 < n_groups:
                    gates[g + 2] = gate_chain(g + 2, *xt[g + 2])

    if not nc.is_finalized():
        nc.finalize()
    return nc


def _make_consts(OP, w1, b1, w2, alpha_low, alpha_high):
    import ml_dtypes
    sig = lambda v: 1.0 / (1.0 + np.exp(-np.float64(v)))
    WT = np.ascontiguousarray(np.asarray(OP, np.float64).T)
    wtblob = np.zeros((128, 400), np.float32)
    wtblob[0:128, 0:196] = WT[0:128]
    wtblob[0:68, 196:392] = WT[128:196]
    wtblob[:, ONES0] = 1.0 / P
    gblob = np.zeros((128, 1748), np.float32)
    gblob[:, GW1:GW1 + 1152] = np.asarray(w1, np.float32).reshape(
        6, 128, 192).transpose(1, 0, 2).reshape(128, 1152)
    gblob[0, GB1:GB1 + 192] = np.asarray(b1, np.float32)
    gblob[0:96, GW20:GW20 + 2] = np.asarray(w2, np.float32)[0:96]
    gblob[0:96, GW21:GW21 + 2] = np.asarray(w2, np.float32)[96:192]
    gblob[0, GONES:GONES + 16] = 1.0
    gblob[0, GALR:GALR + 128] = sig(alpha_low)
    gblob[0, GAHR:GAHR + 128] = sig(alpha_high)
    gblob[0, GCNEG:GCNEG + 128] = -sig(alpha_high) / sig(alpha_low)
    return {"wtblob": wtblob.astype(ml_dtypes.bfloat16),
            "gblob": gblob.astype(ml_dtypes.bfloat16)}


def build_for_sim():
    """Program instance for cost-model simulation (dummy weights)."""
    import ml_dtypes
    consts = {
        "wtblob": np.zeros((128, 400), ml_dtypes.bfloat16),
        "gblob": np.ones((128, 1748), ml_dtypes.bfloat16),
    }
    return _build_program(consts, 0.0, 0.0)


def kernel(x, low_params, high_params, alpha_low, alpha_high,
           w1, b1, w2, b2, cls_token_idx):
    import ml_dtypes
    assert int(cls_token_idx) == 0
    x = np.asarray(x, dtype=np.float32)
    assert x.shape == (B, N, D)

    lm = _freq_mask_np(low_params, True)
    A = _conv_operator(lm)
    share_Y = np.allclose(np.asarray(low_params, np.float32),
                          np.asarray(high_params, np.float32))
    b2v = np.asarray(b2, np.float64).reshape(2)

    xbf = np.ascontiguousarray(x.astype(ml_dtypes.bfloat16))
    xs = xbf.reshape(NCORES, BS, N, D)
    in_maps = [{"xs": np.ascontiguousarray(xs[c])} for c in range(NCORES)]

    def run_once(OP):
        consts = _make_consts(OP, w1, b1, w2, alpha_low, alpha_high)
        nc = _build_program(consts, float(b2v[0]), float(b2v[1]))
        res = run_bass_kernel_spmd(nc, in_maps, core_ids=list(range(NCORES)))
        lo = np.concatenate([np.asarray(r["lo"]) for r in res.results],
                            axis=0).astype(np.float32)
        hi = np.concatenate([np.asarray(r["hi"]) for r in res.results],
                            axis=0).astype(np.float32)
        if getattr(res, "exec_time_ns", None) is not None:
            print(f"HW exec time: {res.exec_time_ns} ns")
        return lo, hi

    if share_Y:
        return run_once(A)
    # generic case (not hit by the reference inputs): hi needs its own
    # operator; run the validated single-operator program twice
    lo, _ = run_once(A)
    Cm = _conv_operator(_freq_mask_np(high_params, True))
    _, hi = run_once(Cm)
    return lo, hi


# revision 104
# speedup vs baseline: 2.7601x; 1.0226x over previous
"""DCT Frequency Splitter — Trainium2 Bass kernel (v3, bf16 end-to-end).

Math: FFT2 -> mask -> IFFT2 -> real is a linear operator on the 196 patch
tokens (per channel): z = A @ patches with A = Re(Finv diag(m) F) (196x196,
real, built on host from the 4 mask params).  With shared mask params the
high path is high = patches - z, so one matmul feeds both outputs:
lo = s_l * z, hi = s_h * (patches - z).

v3 layout decisions (all driven by the TimelineSim cost model):
- bf16 everywhere off-chip: x is converted to bf16 on the host, outputs are
  stored bf16 and upcast on the host.  Halves DMA traffic (the kernel is
  DMA-bound at ~360 GB/s/core); rel-err budget 2e-2 >> bf16's ~4e-3.
- gate-first: per-image token means are computed straight from the x tiles
  with tiny K-contraction matmuls into a PSUM arena (gT in [d, img] layout,
  no PSUM row drain + transpose shuffle), so the gate scales are ready when
  the main matmuls drain and the lo output leaves PSUM already scaled - one
  Activation pass instead of two.
- engine split per image: Act = scaled lo drains, DVE = hi subs + hi scale
  (a-part), Pool/GPSIMD = hi scale (b-part), PE = matmuls.
- group-batched DMAs (4 images per DMA) to keep the SP sequencer's ~1.1us
  per-DMA issue cost off the critical path.

Sharding: pure data parallel, batch 128 -> 16 per core across 8 cores.
"""

import os
import numpy as np

import concourse.bass as bass
import concourse.bacc as bacc_mod
import concourse.mybir as mybir
import concourse.tile as tile
from concourse.bass_utils import run_bass_kernel_spmd
from concourse.tile_rust import add_dep_helper

H, W = 14, 14
B, N, D = 128, 197, 768
P = 196  # patch tokens
NCORES = 8
BS = B // NCORES  # batches per core

GRP = int(os.environ.get("KRN_GROUP", "8"))        # images per group
BUFO = int(os.environ.get("KRN_BUFO", "4"))        # output half-tile buffers
F32 = mybir.dt.float32
BF16 = mybir.dt.bfloat16

# x tiles hold tokens 1..196 only (CLS skipped at load): xga = patches
# 0..127 on partitions 0..127, xgb = patches 128..195 on partitions 0..67,
# so matmul operands/outputs and the elementwise hi ops all share base
# partition 0 (the PE requires operand base partition in {0, 32, 64}).
# wtblob column layout (bf16, [128, 400])
WTA0 = 0      # A^T rows 0:128 (K = patches 0..127), cols 0:196
WTB0 = 196    # A^T rows 128:196 (K = patches 128..195), cols 196:392
ONES0 = 392   # [128, 1] column of 1/196
# gblob column layout (bf16, [128, 1748])
GW1, GB1 = 0, 1152
GW20, GW21 = 1344, 1346
GONES = 1348
GALR, GAHR = 1364, 1492
GCNEG = 1620  # -sig(alpha_high)/sig(alpha_low) row for the hi-ratio


def _freq_mask_np(params, low):
    ch, cw, radius, sharp = [np.float64(v) for v in np.asarray(params)]
    y = np.arange(H, dtype=np.float64)
    x = np.arange(W, dtype=np.float64)
    d2 = (y[:, None] - ch) ** 2 + (x[None, :] - cw) ** 2
    dist = np.sqrt(d2 + 1e-12)
    s = np.clip(sharp, 0.5, 10.0)
    r = np.clip(radius, 1.0, min(H, W) / 2.0)
    m = np.exp(-((dist / r) ** s))
    return m if low else 1.0 - m


def _conv_operator(mask):
    """Real 196x196 operator equivalent to ifft2(fft2(img)*mask).real."""
    F_H = np.exp(-2j * np.pi * np.outer(np.arange(H), np.arange(H)) / H)
    F_W = np.exp(-2j * np.pi * np.outer(np.arange(W), np.arange(W)) / W)
    Fi_H = np.conj(F_H) / H
    Fi_W = np.conj(F_W) / W
    op = np.kron(Fi_H, Fi_W) @ np.diag(mask.ravel()) @ np.kron(F_H, F_W)
    return np.real(op)


def _build_program(consts, b2lo, b2hi):
    nc = bacc_mod.Bacc(None)

    xs_h = nc.dram_tensor("xs", [BS, N, D], BF16, kind="ExternalInput")
    lo_h = nc.dram_tensor("lo", [BS, N, D], BF16, kind="ExternalOutput")
    hi_h = nc.dram_tensor("hi", [BS, N, D], BF16, kind="ExternalOutput")

    ch = {k: nc.inline_tensor(v, name=f"c_{k}") for k, v in consts.items()}

    Copy = mybir.ActivationFunctionType.Copy
    Sig = mybir.ActivationFunctionType.Sigmoid

    # small first group (early first stores = DMA ramps sooner) and small
    # last group (short drain+store tail); 4-image groups in the middle
    if os.environ.get("KRN_GPAT"):
        sizes = [int(v) for v in os.environ["KRN_GPAT"].split(",")]
        assert sum(sizes) == BS
    else:
        sizes = [min(GRP, BS - s) for s in range(0, BS, GRP)]
    groups, s = [], 0
    for sz in sizes:
        groups.append(list(range(s, s + sz)))
        s += sz
    n_groups = len(groups)

    with tile.TileContext(nc) as tc:
        with (
            tc.tile_pool(name="consts", bufs=1) as cp,
            tc.tile_pool(name="xp", bufs=n_groups) as xp,
            tc.tile_pool(name="outp", bufs=BUFO) as outp,
            tc.tile_pool(name="scr", bufs=2) as scr,
            tc.tile_pool(name="gp", bufs=2) as gp,
            tc.tile_pool(name="pm", bufs=2, space="PSUM") as pm,
            tc.tile_pool(name="par", bufs=2, space="PSUM") as par,
        ):
            def cload(key, dtype):
                arr = consts[key]
                t = cp.tile(list(arr.shape), dtype, tag=key)
                nc.sync.dma_start(out=t[:], in_=ch[key][...])
                return t

            wb = cload("wtblob", BF16)      # [128, 400]
            onescol = wb[:, ONES0:ONES0 + 1]

            def load_group(g):
                bs = groups[g]
                b0, gn = bs[0], len(bs)
                xga = xp.tile([128, GRP, D], BF16, tag="xga")
                xgb = xp.tile([68, GRP, D], BF16, tag="xgb")
                nc.sync.dma_start(
                    out=xga[:, 0:gn, :],
                    in_=xs_h[b0:b0 + gn, 1:129, :].rearrange("b t d -> t b d"))
                nc.sync.dma_start(
                    out=xgb[:, 0:gn, :],
                    in_=xs_h[b0:b0 + gn, 129:197, :].rearrange("b t d -> t b d"))
                return xga, xgb

            # group 0's loads go before the remaining consts so its gate
            # chain starts as early as possible; all loads precede all
            # stores so SP's in-order sequencer never parks a semaphore-
            # blocked store ahead of a ready load
            xt = {0: load_group(0)}

            gb = cload("gblob", BF16)       # [128, 1748]
            w1c = gb[:, GW1:GW1 + 1152].rearrange("p (a h b) -> p a h b",
                                                  a=6, h=2)
            w2c0 = gb[0:96, GW20:GW20 + 2]
            w2c1 = gb[0:96, GW21 + 0:GW21 + 2]
            ones1 = gb[0:1, GONES:GONES + 16]
            alr = gb[0:1, GALR:GALR + 128]
            ahr = gb[0:1, GAHR:GAHR + 128]
            cneg = gb[0:1, GCNEG:GCNEG + 128]

            # dummy activation so the act-func table load (~1.3us) runs at
            # t~=1us instead of stalling the first group's gate sigmoid;
            # reads the earliest const so it never blocks Act's queue
            warm = gp.tile([1, 16], F32, tag="warm")
            nc.scalar.activation(warm[:], wb[0:1, 0:16], Sig)

            # CLS passthrough for all batches (DRAM -> DRAM), issued from
            # the near-free GPSIMD DMA queue so SP only handles bulk I/O
            nc.gpsimd.dma_start(out=lo_h[:, 0:1, :], in_=xs_h[:, 0:1, :])
            nc.gpsimd.dma_start(out=hi_h[:, 0:1, :], in_=xs_h[:, 0:1, :])

            for g in range(1, n_groups):
                xt[g] = load_group(g)

            def gate_chain(g, xga, xgb):
                bs = groups[g]
                gn = len(bs)
                # gate for this group, ahead of the main matmuls.
                # gT[d, j] = mean over patch tokens of x (tiny K-contraction
                # matmuls straight into the psum arena, [d, img] layout).
                arena = par.tile([128, 384], F32, tag="arena")
                for j in range(gn):
                    for c in range(6):
                        col = c * 16 + j
                        nc.tensor.matmul(
                            arena[:, col:col + 1],
                            xga[:, j, c * 128:(c + 1) * 128],
                            onescol[:],
                            start=True, stop=False)
                        nc.tensor.matmul(
                            arena[:, col:col + 1],
                            xgb[0:68, j, c * 128:(c + 1) * 128],
                            onescol[0:68],
                            start=False, stop=True)
                gTt = gp.tile([128, 6, 16], BF16, tag="gTt")
                nc.vector.tensor_copy(
                    gTt[:].rearrange("p a b -> p (a b)"), arena[:, 0:96])

                # hidden layer directly in transposed [feature, img] layout
                # (w1 chunks stationary): no transposes, tiny moving dims
                gate_pe = None
                for h in range(2):
                    hps = arena[0:96, 96 + 16 * h:96 + 16 * h + 16]
                    for c in range(6):
                        nc.tensor.matmul(hps[:, 0:gn], w1c[:, c, h, :],
                                         gTt[:, c, 0:gn],
                                         start=(c == 0), stop=False)
                    gate_pe = nc.tensor.matmul(
                        hps[:, 0:gn], gb[0:1, GB1 + 96 * h:GB1 + 96 * h + 96],
                        ones1[0:1, 0:gn], start=False, stop=True)
                hTt = gp.tile([96, 2, 16], BF16, tag="hTt")
                for h in range(2):
                    hps = arena[0:96, 96 + 16 * h:96 + 16 * h + 16]
                    nc.vector.tensor_relu(hTt[:, h, 0:gn], hps[:, 0:gn])

                crows = []
                for col, b2f in ((0, b2lo), (1, b2hi)):
                    g_ps = arena[0:1, 128 + 16 * col:144 + 16 * col]
                    nc.tensor.matmul(g_ps[:, 0:gn], w2c0[:, col:col + 1],
                                     hTt[:, 0, 0:gn], start=True, stop=False)
                    nc.tensor.matmul(g_ps[:, 0:gn], w2c1[:, col:col + 1],
                                     hTt[:, 1, 0:gn], start=False, stop=True)
                    cr = gp.tile([1, 16], BF16, tag=f"crow{col}")
                    nc.scalar.activation(cr[:, 0:gn], g_ps[:, 0:gn], Sig,
                                         bias=b2f)
                    crows.append(cr)
                # per-image hi/lo gate ratio (the hi path is reconstructed
                # from the already-scaled lo tile: hi = crh*x - r*lo with
                # r = crh/crl; the -alpha ratio constant lives in cneg)
                rcp = gp.tile([1, 16], F32, tag="rcp")
                nc.vector.reciprocal(rcp[:, 0:gn], crows[0][0:1, 0:gn])
                rrow = gp.tile([1, 16], BF16, tag="rrow")
                nc.vector.tensor_mul(rrow[:, 0:gn], rcp[:, 0:gn],
                                     crows[1][0:1, 0:gn])
                # replicate the gate rows across partitions; the alpha
                # sigmoid is folded into the alr/ahr/cneg weight rows
                for k, (wrow, mov) in enumerate(
                        ((alr, crows[0][0:1, 0:gn]),
                         (ahr, crows[1][0:1, 0:gn]),
                         (cneg, rrow[0:1, 0:gn]))):
                    nc.tensor.matmul(arena[:, 160 + 16 * k:160 + 16 * k + gn],
                                     wrow[0:1, :], mov, start=True, stop=True)
                crlh = gp.tile([128, 48], F32, tag="crlh")
                nc.vector.tensor_copy(crlh[:], arena[:, 160:208])
                return crlh, gate_pe

            # gates run two groups ahead of their bodies so their small
            # DVE/Act steps never queue behind a full body's engine work
            gates = {g: gate_chain(g, *xt[g]) for g in range(min(2, n_groups))}

            for g, bs in enumerate(groups):
                gn = len(bs)
                b0 = bs[0]
                xga, xgb = xt[g]
                crlh, gate_pe = gates.pop(g)
                crl = crlh[:, 0:16]
                crh = crlh[:, 16:32]
                rneg = crlh[:, 32:48]

                # ---- main matmuls + scaled drains per image
                for j in range(gn):
                    # per-2-image output tiles: a store only waits on its
                    # own half's drains/adds (tile-granular dependency
                    # tracking would otherwise park it behind the whole
                    # group), and dedicated hi tiles release the x tiles to
                    # the pool at the final add rather than at the store
                    jj = j % 2
                    if jj == 0:
                        lo_ga = outp.tile([128, 2, D], BF16, tag="lo_ga")
                        lo_gb = outp.tile([68, 2, D], BF16, tag="lo_gb")
                        ho_ga = outp.tile([128, 2, D], BF16, tag="ho_ga")
                        ho_gb = outp.tile([68, 2, D], BF16, tag="ho_gb")
                    # PSUM packing: 3 banks/image — za_t[128,1024] holds
                    # za(0:768) + zb's tail chunk (768:1024), zb1 the rest.
                    # The a-side then drains in ONE 768-col Act pass; the
                    # two tiles free independently (a-side earlier), which
                    # beats a fully-merged 3-bank tile.
                    za_t = pm.tile([128, 1024], F32, tag="za_t")
                    zb1 = pm.tile([68, 512], F32, tag="zb1")
                    za_ch = [(0, 512, za_t[:, 0:512]),
                             (512, 768, za_t[:, 512:768])]
                    zb_ch = [(0, 512, zb1[0:68, :]),
                             (512, 768, za_t[0:68, 768:1024])]
                    for (n0, n1, zc) in za_ch:
                        mm = nc.tensor.matmul(zc, wb[:, 0:128],
                                              xga[:, j, n0:n1],
                                              start=True, stop=False)
                        add_dep_helper(mm.ins, gate_pe.ins,
                                       reason="gate chain schedules first")
                        nc.tensor.matmul(zc, wb[0:68, WTB0:WTB0 + 128],
                                         xgb[0:68, j, n0:n1],
                                         start=False, stop=True)
                    for (n0, n1, zc) in zb_ch:
                        mm = nc.tensor.matmul(zc, wb[:, 128:196],
                                              xga[:, j, n0:n1],
                                              start=True, stop=False)
                        add_dep_helper(mm.ins, gate_pe.ins,
                                       reason="gate chain schedules first")
                        nc.tensor.matmul(zc, wb[0:68, WTB0 + 128:WTB0 + 196],
                                         xgb[0:68, j, n0:n1],
                                         start=False, stop=True)

                    # GPSIMD cannot touch PSUM, so the legal engine split is:
                    #   Act:  scaled lo drains (+ gate sigmoids)
                    #   DVE:  hi reconstruction + every other b2 drain
                    #   Pool: b-side hi scale (SBUF-only bf16)
                    nc.scalar.activation(lo_ga[:, jj, :], za_t[:, 0:768],
                                         Copy, scale=crl[:, j:j + 1])
                    nc.scalar.activation(lo_gb[0:68, jj, 0:512], zb1[0:68, :],
                                         Copy, scale=crl[0:68, j:j + 1])
                    nc.scalar.activation(lo_gb[0:68, jj, 512:768],
                                         za_t[0:68, 768:1024], Copy,
                                         scale=crl[0:68, j:j + 1])
                    # hi = crh*x - (crh/crl)*lo: reconstructed from the
                    # drained lo tiles, entirely off PSUM (z has a single
                    # reader).  All-bf16-SBUF muls/adds hit DVE 2x/4x modes;
                    # the b-side x-scale rides the otherwise idle GPSIMD.
                    nc.gpsimd.tensor_scalar_mul(xgb[0:68, j, :],
                                                xgb[0:68, j, :],
                                                crh[0:68, j:j + 1])
                    nc.vector.tensor_scalar_mul(xga[:, j, :],
                                                xga[:, j, :],
                                                crh[:, j:j + 1])
                    ta = scr.tile([128, D], BF16, tag="ta")
                    tb = scr.tile([68, D], BF16, tag="tb")
                    nc.vector.tensor_scalar_mul(ta[:], lo_ga[:, jj, :],
                                                rneg[:, j:j + 1])
                    nc.vector.tensor_add(ho_ga[:, jj, :], xga[:, j, :],
                                         ta[:])
                    nc.vector.tensor_scalar_mul(tb[0:68, :],
                                                lo_gb[0:68, jj, :],
                                                rneg[0:68, j:j + 1])
                    nc.vector.tensor_add(ho_gb[0:68, jj, :], xgb[0:68, j, :],
                                         tb[0:68, :])

                    # ship each completed half immediately (stores from SP:
                    # all loads were pre-issued, so the in-order queue never
                    # parks a blocked store ahead of a ready load)
                    if jj == 1 or j == gn - 1:
                        k0 = j - jj
                        c0, c1 = b0 + k0, b0 + j + 1
                        kn = j + 1 - k0
                        nc.sync.dma_start(
                            out=lo_h[c0:c1, 1:129, :].rearrange(
                                "b t d -> t b d"),
                            in_=lo_ga[:, 0:kn, :])
                        nc.sync.dma_start(
                            out=lo_h[c0:c1, 129:197, :].rearrange(
                                "b t d -> t b d"),
                            in_=lo_gb[:, 0:kn, :])
                        nc.sync.dma_start(
                            out=hi_h[c0:c1, 1:129, :].rearrange(
                                "b t d -> t b d"),
                            in_=ho_ga[:, 0:kn, :])
                        nc.sync.dma_start(
                            out=hi_h[c0:c1, 129:197, :].rearrange(
                                "b t d -> t b d"),
                            in_=ho_gb[:, 0:kn, :])

                # next-next group's gate chain goes ahead of this group's
                # last stores (on PE it runs while Act/DVE/Pool finish here)
                if g + 2 < n_groups:
                    gates[g + 2] = gate_chain(g + 2, *xt[g + 2])

    if not nc.is_finalized():
        nc.finalize()
    return nc


def _make_consts(OP, w1, b1, w2, alpha_low, alpha_high):
    import ml_dtypes
    sig = lambda v: 1.0 / (1.0 + np.exp(-np.float64(v)))
    WT = np.ascontiguousarray(np.asarray(OP, np.float64).T)
    wtblob = np.zeros((128, 400), np.float32)
    wtblob[0:128, 0:196] = WT[0:128]
    wtblob[0:68, 196:392] = WT[128:196]
    wtblob[:, ONES0] = 1.0 / P
    gblob = np.zeros((128, 1748), np.float32)
    gblob[:, GW1:GW1 + 1152] = np.asarray(w1, np.float32).reshape(
        6, 128, 192).transpose(1, 0, 2).reshape(128, 1152)
    gblob[0, GB1:GB1 + 192] = np.asarray(b1, np.float32)
    gblob[0:96, GW20:GW20 + 2] = np.asarray(w2, np.float32)[0:96]
    gblob[0:96, GW21:GW21 + 2] = np.asarray(w2, np.float32)[96:192]
    gblob[0, GONES:GONES + 16] = 1.0
    gblob[0, GALR:GALR + 128] = sig(alpha_low)
    gblob[0, GAHR:GAHR + 128] = sig(alpha_high)
    gblob[0, GCNEG:GCNEG + 128] = -sig(alpha_high) / sig(alpha_low)
    return {"wtblob": wtblob.astype(ml_dtypes.bfloat16),
            "gblob": gblob.astype(ml_dtypes.bfloat16)}


def build_for_sim():
    """Program instance for cost-model simulation (dummy weights)."""
    import ml_dtypes
    consts = {
        "wtblob": np.zeros((128, 400), ml_dtypes.bfloat16),
        "gblob": np.ones((128, 1748), ml_dtypes.bfloat16),
    }
    return _build_program(consts, 0.0, 0.0)


def kernel(x, low_params, high_params, alpha_low, alpha_high,
           w1, b1, w2, b2, cls_token_idx):
    import ml_dtypes
    assert int(cls_token_idx) == 0
    x = np.asarray(x, dtype=np.float32)
    assert x.shape == (B, N, D)

    lm = _freq_mask_np(low_params, True)
    A = _conv_operator(lm)
    share_Y = np.allclose(np.asarray(low_params, np.float32),
                          np.asarray(high_params, np.float32))
    b2v = np.asarray(b2, np.float64).reshape(2)

    xbf = np.ascontiguousarray(x.astype(ml_dtypes.bfloat16))
    xs = xbf.reshape(NCORES, BS, N, D)
    in_maps = [{"xs": np.ascontiguousarray(xs[c])} for c in range(NCORES)]

    def run_once(OP):
        consts = _make_consts(OP, w1, b1, w2, alpha_low, alpha_high)
        nc = _build_program(consts, float(b2v[0]), float(b2v[1]))
        res = run_bass_kernel_spmd(nc, in_maps, core_ids=list(range(NCORES)))
        lo = np.concatenate([np.asarray(r["lo"]) for r in res.results],
                            axis=0).astype(np.float32)
        hi = np.concatenate([np.asarray(r["hi"]) for r in res.results],
                            axis=0).astype(np.float32)
        if getattr(res, "exec_time_ns", None) is not None:
            print(f"HW exec time: {res.exec_time_ns} ns")
        return lo, hi

    if share_Y:
        return run_once(A)
    # generic case (not hit by the reference inputs): hi needs its own
    # operator; run the validated single-operator program twice
    lo, _ = run_once(A)
    Cm = _conv_operator(_freq_mask_np(high_params, True))
    _, hi = run_once(Cm)
    return lo, hi


# revision 105
# speedup vs baseline: 2.8248x; 1.0235x over previous
"""DCT Frequency Splitter — Trainium2 Bass kernel (v3, bf16 end-to-end).

Math: FFT2 -> mask -> IFFT2 -> real is a linear operator on the 196 patch
tokens (per channel): z = A @ patches with A = Re(Finv diag(m) F) (196x196,
real, built on host from the 4 mask params).  With shared mask params the
high path is high = patches - z, so one matmul feeds both outputs:
lo = s_l * z, hi = s_h * (patches - z).

v3 layout decisions (all driven by the TimelineSim cost model):
- bf16 everywhere off-chip: x is converted to bf16 on the host, outputs are
  stored bf16 and upcast on the host.  Halves DMA traffic (the kernel is
  DMA-bound at ~360 GB/s/core); rel-err budget 2e-2 >> bf16's ~4e-3.
- gate-first: per-image token means are computed straight from the x tiles
  with tiny K-contraction matmuls into a PSUM arena (gT in [d, img] layout,
  no PSUM row drain + transpose shuffle), so the gate scales are ready when
  the main matmuls drain and the lo output leaves PSUM already scaled - one
  Activation pass instead of two.
- engine split per image: Act = scaled lo drains, DVE = hi subs + hi scale
  (a-part), Pool/GPSIMD = hi scale (b-part), PE = matmuls.
- group-batched DMAs (4 images per DMA) to keep the SP sequencer's ~1.1us
  per-DMA issue cost off the critical path.

Sharding: pure data parallel, batch 128 -> 16 per core across 8 cores.
"""

import os
import numpy as np

import concourse.bass as bass
import concourse.bacc as bacc_mod
import concourse.mybir as mybir
import concourse.tile as tile
from concourse.bass_utils import run_bass_kernel_spmd
from concourse.tile_rust import add_dep_helper

H, W = 14, 14
B, N, D = 128, 197, 768
P = 196  # patch tokens
NCORES = 8
BS = B // NCORES  # batches per core

GRP = int(os.environ.get("KRN_GROUP", "8"))        # images per group
BUFO = int(os.environ.get("KRN_BUFO", "4"))        # output half-tile buffers
F32 = mybir.dt.float32
BF16 = mybir.dt.bfloat16

# x tiles hold tokens 1..196 only (CLS skipped at load): xga = patches
# 0..127 on partitions 0..127, xgb = patches 128..195 on partitions 0..67,
# so matmul operands/outputs and the elementwise hi ops all share base
# partition 0 (the PE requires operand base partition in {0, 32, 64}).
# wtblob column layout (bf16, [128, 400])
WTA0 = 0      # A^T rows 0:128 (K = patches 0..127), cols 0:196
WTB0 = 196    # A^T rows 128:196 (K = patches 128..195), cols 196:392
ONES0 = 392   # [128, 1] column of 1/196
# gblob column layout (bf16, [128, 1748])
GW1, GB1 = 0, 1152
GW20, GW21 = 1344, 1346
GONES = 1348
GALR, GAHR = 1364, 1492
GCNEG = 1620  # -sig(alpha_high)/sig(alpha_low) row for the hi-ratio


def _freq_mask_np(params, low):
    ch, cw, radius, sharp = [np.float64(v) for v in np.asarray(params)]
    y = np.arange(H, dtype=np.float64)
    x = np.arange(W, dtype=np.float64)
    d2 = (y[:, None] - ch) ** 2 + (x[None, :] - cw) ** 2
    dist = np.sqrt(d2 + 1e-12)
    s = np.clip(sharp, 0.5, 10.0)
    r = np.clip(radius, 1.0, min(H, W) / 2.0)
    m = np.exp(-((dist / r) ** s))
    return m if low else 1.0 - m


def _conv_operator(mask):
    """Real 196x196 operator equivalent to ifft2(fft2(img)*mask).real."""
    F_H = np.exp(-2j * np.pi * np.outer(np.arange(H), np.arange(H)) / H)
    F_W = np.exp(-2j * np.pi * np.outer(np.arange(W), np.arange(W)) / W)
    Fi_H = np.conj(F_H) / H
    Fi_W = np.conj(F_W) / W
    op = np.kron(Fi_H, Fi_W) @ np.diag(mask.ravel()) @ np.kron(F_H, F_W)
    return np.real(op)


def _build_program(consts, b2lo, b2hi):
    nc = bacc_mod.Bacc(None)

    xs_h = nc.dram_tensor("xs", [BS, N, D], BF16, kind="ExternalInput")
    lo_h = nc.dram_tensor("lo", [BS, N, D], BF16, kind="ExternalOutput")
    hi_h = nc.dram_tensor("hi", [BS, N, D], BF16, kind="ExternalOutput")

    ch = {k: nc.inline_tensor(v, name=f"c_{k}") for k, v in consts.items()}

    Copy = mybir.ActivationFunctionType.Copy
    Sig = mybir.ActivationFunctionType.Sigmoid

    # small first group (early first stores = DMA ramps sooner) and small
    # last group (short drain+store tail); 4-image groups in the middle
    if os.environ.get("KRN_GPAT"):
        sizes = [int(v) for v in os.environ["KRN_GPAT"].split(",")]
        assert sum(sizes) == BS
    elif BS == 16 and GRP == 8:
        sizes = [5, 6, 5]   # best measured group pattern
    else:
        sizes = [min(GRP, BS - s) for s in range(0, BS, GRP)]
    groups, s = [], 0
    for sz in sizes:
        groups.append(list(range(s, s + sz)))
        s += sz
    n_groups = len(groups)

    with tile.TileContext(nc) as tc:
        with (
            tc.tile_pool(name="consts", bufs=1) as cp,
            tc.tile_pool(name="xp", bufs=n_groups) as xp,
            tc.tile_pool(name="outp", bufs=BUFO) as outp,
            tc.tile_pool(name="scr", bufs=2) as scr,
            tc.tile_pool(name="gp", bufs=2) as gp,
            tc.tile_pool(name="pm", bufs=2, space="PSUM") as pm,
            tc.tile_pool(name="par", bufs=2, space="PSUM") as par,
        ):
            def cload(key, dtype):
                arr = consts[key]
                t = cp.tile(list(arr.shape), dtype, tag=key)
                nc.sync.dma_start(out=t[:], in_=ch[key][...])
                return t

            wb = cload("wtblob", BF16)      # [128, 400]
            onescol = wb[:, ONES0:ONES0 + 1]

            def load_group(g):
                bs = groups[g]
                b0, gn = bs[0], len(bs)
                xga = xp.tile([128, GRP, D], BF16, tag="xga")
                xgb = xp.tile([68, GRP, D], BF16, tag="xgb")
                nc.sync.dma_start(
                    out=xga[:, 0:gn, :],
                    in_=xs_h[b0:b0 + gn, 1:129, :].rearrange("b t d -> t b d"))
                nc.sync.dma_start(
                    out=xgb[:, 0:gn, :],
                    in_=xs_h[b0:b0 + gn, 129:197, :].rearrange("b t d -> t b d"))
                return xga, xgb

            # group 0's loads go before the remaining consts so its gate
            # chain starts as early as possible; all loads precede all
            # stores so SP's in-order sequencer never parks a semaphore-
            # blocked store ahead of a ready load
            xt = {0: load_group(0)}

            gb = cload("gblob", BF16)       # [128, 1748]
            w1c = gb[:, GW1:GW1 + 1152].rearrange("p (a h b) -> p a h b",
                                                  a=6, h=2)
            w2c0 = gb[0:96, GW20:GW20 + 2]
            w2c1 = gb[0:96, GW21 + 0:GW21 + 2]
            ones1 = gb[0:1, GONES:GONES + 16]
            alr = gb[0:1, GALR:GALR + 128]
            ahr = gb[0:1, GAHR:GAHR + 128]
            cneg = gb[0:1, GCNEG:GCNEG + 128]

            # dummy activation so the act-func table load (~1.3us) runs at
            # t~=1us instead of stalling the first group's gate sigmoid;
            # reads the earliest const so it never blocks Act's queue
            warm = gp.tile([1, 16], F32, tag="warm")
            nc.scalar.activation(warm[:], wb[0:1, 0:16], Sig)

            # CLS passthrough for all batches (DRAM -> DRAM), issued from
            # the near-free GPSIMD DMA queue so SP only handles bulk I/O
            nc.gpsimd.dma_start(out=lo_h[:, 0:1, :], in_=xs_h[:, 0:1, :])
            nc.gpsimd.dma_start(out=hi_h[:, 0:1, :], in_=xs_h[:, 0:1, :])

            for g in range(1, n_groups):
                xt[g] = load_group(g)

            def gate_chain(g, xga, xgb):
                bs = groups[g]
                gn = len(bs)
                # gate for this group, ahead of the main matmuls.
                # gT[d, j] = mean over patch tokens of x (tiny K-contraction
                # matmuls straight into the psum arena, [d, img] layout).
                arena = par.tile([128, 384], F32, tag="arena")
                for j in range(gn):
                    for c in range(6):
                        col = c * 16 + j
                        nc.tensor.matmul(
                            arena[:, col:col + 1],
                            xga[:, j, c * 128:(c + 1) * 128],
                            onescol[:],
                            start=True, stop=False)
                        nc.tensor.matmul(
                            arena[:, col:col + 1],
                            xgb[0:68, j, c * 128:(c + 1) * 128],
                            onescol[0:68],
                            start=False, stop=True)
                gTt = gp.tile([128, 6, 16], BF16, tag="gTt")
                nc.vector.tensor_copy(
                    gTt[:].rearrange("p a b -> p (a b)"), arena[:, 0:96])

                # hidden layer directly in transposed [feature, img] layout
                # (w1 chunks stationary): no transposes, tiny moving dims
                gate_pe = None
                for h in range(2):
                    hps = arena[0:96, 96 + 16 * h:96 + 16 * h + 16]
                    for c in range(6):
                        nc.tensor.matmul(hps[:, 0:gn], w1c[:, c, h, :],
                                         gTt[:, c, 0:gn],
                                         start=(c == 0), stop=False)
                    gate_pe = nc.tensor.matmul(
                        hps[:, 0:gn], gb[0:1, GB1 + 96 * h:GB1 + 96 * h + 96],
                        ones1[0:1, 0:gn], start=False, stop=True)
                hTt = gp.tile([96, 2, 16], BF16, tag="hTt")
                for h in range(2):
                    hps = arena[0:96, 96 + 16 * h:96 + 16 * h + 16]
                    nc.vector.tensor_relu(hTt[:, h, 0:gn], hps[:, 0:gn])

                crows = []
                for col, b2f in ((0, b2lo), (1, b2hi)):
                    g_ps = arena[0:1, 128 + 16 * col:144 + 16 * col]
                    nc.tensor.matmul(g_ps[:, 0:gn], w2c0[:, col:col + 1],
                                     hTt[:, 0, 0:gn], start=True, stop=False)
                    nc.tensor.matmul(g_ps[:, 0:gn], w2c1[:, col:col + 1],
                                     hTt[:, 1, 0:gn], start=False, stop=True)
                    cr = gp.tile([1, 16], BF16, tag=f"crow{col}")
                    nc.scalar.activation(cr[:, 0:gn], g_ps[:, 0:gn], Sig,
                                         bias=b2f)
                    crows.append(cr)
                # per-image hi/lo gate ratio (the hi path is reconstructed
                # from the already-scaled lo tile: hi = crh*x - r*lo with
                # r = crh/crl; the -alpha ratio constant lives in cneg)
                rcp = gp.tile([1, 16], F32, tag="rcp")
                nc.vector.reciprocal(rcp[:, 0:gn], crows[0][0:1, 0:gn])
                rrow = gp.tile([1, 16], BF16, tag="rrow")
                nc.vector.tensor_mul(rrow[:, 0:gn], rcp[:, 0:gn],
                                     crows[1][0:1, 0:gn])
                # replicate the gate rows across partitions; the alpha
                # sigmoid is folded into the alr/ahr/cneg weight rows
                for k, (wrow, mov) in enumerate(
                        ((alr, crows[0][0:1, 0:gn]),
                         (ahr, crows[1][0:1, 0:gn]),
                         (cneg, rrow[0:1, 0:gn]))):
                    nc.tensor.matmul(arena[:, 160 + 16 * k:160 + 16 * k + gn],
                                     wrow[0:1, :], mov, start=True, stop=True)
                crlh = gp.tile([128, 48], F32, tag="crlh")
                nc.vector.tensor_copy(crlh[:], arena[:, 160:208])
                return crlh, gate_pe

            # gates run two groups ahead of their bodies so their small
            # DVE/Act steps never queue behind a full body's engine work
            gates = {g: gate_chain(g, *xt[g]) for g in range(min(2, n_groups))}

            for g, bs in enumerate(groups):
                gn = len(bs)
                b0 = bs[0]
                xga, xgb = xt[g]
                crlh, gate_pe = gates.pop(g)
                crl = crlh[:, 0:16]
                crh = crlh[:, 16:32]
                rneg = crlh[:, 32:48]

                # ---- main matmuls + scaled drains per image
                for j in range(gn):
                    # per-2-image output tiles: a store only waits on its
                    # own half's drains/adds (tile-granular dependency
                    # tracking would otherwise park it behind the whole
                    # group), and dedicated hi tiles release the x tiles to
                    # the pool at the final add rather than at the store
                    jj = j % 2
                    if jj == 0:
                        lo_ga = outp.tile([128, 2, D], BF16, tag="lo_ga")
                        lo_gb = outp.tile([68, 2, D], BF16, tag="lo_gb")
                        ho_ga = outp.tile([128, 2, D], BF16, tag="ho_ga")
                        ho_gb = outp.tile([68, 2, D], BF16, tag="ho_gb")
                    # PSUM packing: 3 banks/image — za_t[128,1024] holds
                    # za(0:768) + zb's tail chunk (768:1024), zb1 the rest.
                    # The a-side then drains in ONE 768-col Act pass; the
                    # two tiles free independently (a-side earlier), which
                    # beats a fully-merged 3-bank tile.
                    za_t = pm.tile([128, 1024], F32, tag="za_t")
                    zb1 = pm.tile([68, 512], F32, tag="zb1")
                    za_ch = [(0, 512, za_t[:, 0:512]),
                             (512, 768, za_t[:, 512:768])]
                    zb_ch = [(0, 512, zb1[0:68, :]),
                             (512, 768, za_t[0:68, 768:1024])]
                    for (n0, n1, zc) in za_ch:
                        mm = nc.tensor.matmul(zc, wb[:, 0:128],
                                              xga[:, j, n0:n1],
                                              start=True, stop=False)
                        add_dep_helper(mm.ins, gate_pe.ins,
                                       reason="gate chain schedules first")
                        nc.tensor.matmul(zc, wb[0:68, WTB0:WTB0 + 128],
                                         xgb[0:68, j, n0:n1],
                                         start=False, stop=True)
                    for (n0, n1, zc) in zb_ch:
                        mm = nc.tensor.matmul(zc, wb[:, 128:196],
                                              xga[:, j, n0:n1],
                                              start=True, stop=False)
                        add_dep_helper(mm.ins, gate_pe.ins,
                                       reason="gate chain schedules first")
                        nc.tensor.matmul(zc, wb[0:68, WTB0 + 128:WTB0 + 196],
                                         xgb[0:68, j, n0:n1],
                                         start=False, stop=True)

                    # GPSIMD cannot touch PSUM, so the legal engine split is:
                    #   Act:  scaled lo drains (+ gate sigmoids)
                    #   DVE:  hi reconstruction + every other b2 drain
                    #   Pool: b-side hi scale (SBUF-only bf16)
                    nc.scalar.activation(lo_ga[:, jj, :], za_t[:, 0:768],
                                         Copy, scale=crl[:, j:j + 1])
                    nc.scalar.activation(lo_gb[0:68, jj, 0:512], zb1[0:68, :],
                                         Copy, scale=crl[0:68, j:j + 1])
                    nc.scalar.activation(lo_gb[0:68, jj, 512:768],
                                         za_t[0:68, 768:1024], Copy,
                                         scale=crl[0:68, j:j + 1])
                    # hi = crh*x - (crh/crl)*lo: reconstructed from the
                    # drained lo tiles, entirely off PSUM (z has a single
                    # reader).  All-bf16-SBUF muls/adds hit DVE 2x/4x modes;
                    # the b-side x-scale rides the otherwise idle GPSIMD.
                    nc.gpsimd.tensor_scalar_mul(xgb[0:68, j, :],
                                                xgb[0:68, j, :],
                                                crh[0:68, j:j + 1])
                    nc.vector.tensor_scalar_mul(xga[:, j, :],
                                                xga[:, j, :],
                                                crh[:, j:j + 1])
                    ta = scr.tile([128, D], BF16, tag="ta")
                    tb = scr.tile([68, D], BF16, tag="tb")
                    nc.vector.tensor_scalar_mul(ta[:], lo_ga[:, jj, :],
                                                rneg[:, j:j + 1])
                    nc.vector.tensor_add(ho_ga[:, jj, :], xga[:, j, :],
                                         ta[:])
                    nc.vector.tensor_scalar_mul(tb[0:68, :],
                                                lo_gb[0:68, jj, :],
                                                rneg[0:68, j:j + 1])
                    nc.vector.tensor_add(ho_gb[0:68, jj, :], xgb[0:68, j, :],
                                         tb[0:68, :])

                    # ship each completed half immediately (stores from SP:
                    # all loads were pre-issued, so the in-order queue never
                    # parks a blocked store ahead of a ready load)
                    if jj == 1 or j == gn - 1:
                        k0 = j - jj
                        c0, c1 = b0 + k0, b0 + j + 1
                        kn = j + 1 - k0
                        nc.sync.dma_start(
                            out=lo_h[c0:c1, 1:129, :].rearrange(
                                "b t d -> t b d"),
                            in_=lo_ga[:, 0:kn, :])
                        nc.sync.dma_start(
                            out=lo_h[c0:c1, 129:197, :].rearrange(
                                "b t d -> t b d"),
                            in_=lo_gb[:, 0:kn, :])
                        nc.sync.dma_start(
                            out=hi_h[c0:c1, 1:129, :].rearrange(
                                "b t d -> t b d"),
                            in_=ho_ga[:, 0:kn, :])
                        nc.sync.dma_start(
                            out=hi_h[c0:c1, 129:197, :].rearrange(
                                "b t d -> t b d"),
                            in_=ho_gb[:, 0:kn, :])

                # next-next group's gate chain goes ahead of this group's
                # last stores (on PE it runs while Act/DVE/Pool finish here)
                if g + 2 < n_groups:
                    gates[g + 2] = gate_chain(g + 2, *xt[g + 2])

    if not nc.is_finalized():
        nc.finalize()
    return nc


def _make_consts(OP, w1, b1, w2, alpha_low, alpha_high):
    import ml_dtypes
    sig = lambda v: 1.0 / (1.0 + np.exp(-np.float64(v)))
    WT = np.ascontiguousarray(np.asarray(OP, np.float64).T)
    wtblob = np.zeros((128, 400), np.float32)
    wtblob[0:128, 0:196] = WT[0:128]
    wtblob[0:68, 196:392] = WT[128:196]
    wtblob[:, ONES0] = 1.0 / P
    gblob = np.zeros((128, 1748), np.float32)
    gblob[:, GW1:GW1 + 1152] = np.asarray(w1, np.float32).reshape(
        6, 128, 192).transpose(1, 0, 2).reshape(128, 1152)
    gblob[0, GB1:GB1 + 192] = np.asarray(b1, np.float32)
    gblob[0:96, GW20:GW20 + 2] = np.asarray(w2, np.float32)[0:96]
    gblob[0:96, GW21:GW21 + 2] = np.asarray(w2, np.float32)[96:192]
    gblob[0, GONES:GONES + 16] = 1.0
    gblob[0, GALR:GALR + 128] = sig(alpha_low)
    gblob[0, GAHR:GAHR + 128] = sig(alpha_high)
    gblob[0, GCNEG:GCNEG + 128] = -sig(alpha_high) / sig(alpha_low)
    return {"wtblob": wtblob.astype(ml_dtypes.bfloat16),
            "gblob": gblob.astype(ml_dtypes.bfloat16)}


def build_for_sim():
    """Program instance for cost-model simulation (dummy weights)."""
    import ml_dtypes
    consts = {
        "wtblob": np.zeros((128, 400), ml_dtypes.bfloat16),
        "gblob": np.ones((128, 1748), ml_dtypes.bfloat16),
    }
    return _build_program(consts, 0.0, 0.0)


def kernel(x, low_params, high_params, alpha_low, alpha_high,
           w1, b1, w2, b2, cls_token_idx):
    import ml_dtypes
    assert int(cls_token_idx) == 0
    x = np.asarray(x, dtype=np.float32)
    assert x.shape == (B, N, D)

    lm = _freq_mask_np(low_params, True)
    A = _conv_operator(lm)
    share_Y = np.allclose(np.asarray(low_params, np.float32),
                          np.asarray(high_params, np.float32))
    b2v = np.asarray(b2, np.float64).reshape(2)

    xbf = np.ascontiguousarray(x.astype(ml_dtypes.bfloat16))
    xs = xbf.reshape(NCORES, BS, N, D)
    in_maps = [{"xs": np.ascontiguousarray(xs[c])} for c in range(NCORES)]

    def run_once(OP):
        consts = _make_consts(OP, w1, b1, w2, alpha_low, alpha_high)
        nc = _build_program(consts, float(b2v[0]), float(b2v[1]))
        res = run_bass_kernel_spmd(nc, in_maps, core_ids=list(range(NCORES)))
        lo = np.concatenate([np.asarray(r["lo"]) for r in res.results],
                            axis=0).astype(np.float32)
        hi = np.concatenate([np.asarray(r["hi"]) for r in res.results],
                            axis=0).astype(np.float32)
        if getattr(res, "exec_time_ns", None) is not None:
            print(f"HW exec time: {res.exec_time_ns} ns")
        return lo, hi

    if share_Y:
        return run_once(A)
    # generic case (not hit by the reference inputs): hi needs its own
    # operator; run the validated single-operator program twice
    lo, _ = run_once(A)
    Cm = _conv_operator(_freq_mask_np(high_params, True))
    _, hi = run_once(Cm)
    return lo, hi


# revision 111
# speedup vs baseline: 2.8638x; 1.0138x over previous
"""DCT Frequency Splitter — Trainium2 Bass kernel (v3, bf16 end-to-end).

Math: FFT2 -> mask -> IFFT2 -> real is a linear operator on the 196 patch
tokens (per channel): z = A @ patches with A = Re(Finv diag(m) F) (196x196,
real, built on host from the 4 mask params).  With shared mask params the
high path is high = patches - z, so one matmul feeds both outputs:
lo = s_l * z, hi = s_h * (patches - z).

v3 layout decisions (all driven by the TimelineSim cost model):
- bf16 everywhere off-chip: x is converted to bf16 on the host, outputs are
  stored bf16 and upcast on the host.  Halves DMA traffic (the kernel is
  DMA-bound at ~360 GB/s/core); rel-err budget 2e-2 >> bf16's ~4e-3.
- gate-first: per-image token means are computed straight from the x tiles
  with tiny K-contraction matmuls into a PSUM arena (gT in [d, img] layout,
  no PSUM row drain + transpose shuffle), so the gate scales are ready when
  the main matmuls drain and the lo output leaves PSUM already scaled - one
  Activation pass instead of two.
- engine split per image: Act = scaled lo drains, DVE = hi subs + hi scale
  (a-part), Pool/GPSIMD = hi scale (b-part), PE = matmuls.
- group-batched DMAs (4 images per DMA) to keep the SP sequencer's ~1.1us
  per-DMA issue cost off the critical path.

Sharding: pure data parallel, batch 128 -> 16 per core across 8 cores.
"""

import os
import numpy as np

import concourse.bass as bass
import concourse.bacc as bacc_mod
import concourse.mybir as mybir
import concourse.tile as tile
from concourse.bass_utils import run_bass_kernel_spmd
from concourse.tile_rust import add_dep_helper

H, W = 14, 14
B, N, D = 128, 197, 768
P = 196  # patch tokens
NCORES = 8
BS = B // NCORES  # batches per core

GRP = int(os.environ.get("KRN_GROUP", "4"))        # images per group
BUFO = int(os.environ.get("KRN_BUFO", "5"))        # output half-tile buffers
F32 = mybir.dt.float32
BF16 = mybir.dt.bfloat16

# x tiles hold tokens 1..196 only (CLS skipped at load): xga = patches
# 0..127 on partitions 0..127, xgb = patches 128..195 on partitions 0..67,
# so matmul operands/outputs and the elementwise hi ops all share base
# partition 0 (the PE requires operand base partition in {0, 32, 64}).
# wtblob column layout (bf16, [128, 400])
WTA0 = 0      # A^T rows 0:128 (K = patches 0..127), cols 0:196
WTB0 = 196    # A^T rows 128:196 (K = patches 128..195), cols 196:392
ONES0 = 392   # [128, 1] column of 1/196
# gblob column layout (bf16, [128, 1748])
GW1, GB1 = 0, 1152
GW20, GW21 = 1344, 1346
GONES = 1348
GALR, GAHR = 1364, 1492
GCNEG = 1620  # -sig(alpha_high)/sig(alpha_low) row for the hi-ratio


def _freq_mask_np(params, low):
    ch, cw, radius, sharp = [np.float64(v) for v in np.asarray(params)]
    y = np.arange(H, dtype=np.float64)
    x = np.arange(W, dtype=np.float64)
    d2 = (y[:, None] - ch) ** 2 + (x[None, :] - cw) ** 2
    dist = np.sqrt(d2 + 1e-12)
    s = np.clip(sharp, 0.5, 10.0)
    r = np.clip(radius, 1.0, min(H, W) / 2.0)
    m = np.exp(-((dist / r) ** s))
    return m if low else 1.0 - m


def _conv_operator(mask):
    """Real 196x196 operator equivalent to ifft2(fft2(img)*mask).real."""
    F_H = np.exp(-2j * np.pi * np.outer(np.arange(H), np.arange(H)) / H)
    F_W = np.exp(-2j * np.pi * np.outer(np.arange(W), np.arange(W)) / W)
    Fi_H = np.conj(F_H) / H
    Fi_W = np.conj(F_W) / W
    op = np.kron(Fi_H, Fi_W) @ np.diag(mask.ravel()) @ np.kron(F_H, F_W)
    return np.real(op)


def _build_program(consts, b2lo, b2hi):
    nc = bacc_mod.Bacc(None)

    xs_h = nc.dram_tensor("xs", [BS, N, D], BF16, kind="ExternalInput")
    lo_h = nc.dram_tensor("lo", [BS, N, D], BF16, kind="ExternalOutput")
    hi_h = nc.dram_tensor("hi", [BS, N, D], BF16, kind="ExternalOutput")

    ch = {k: nc.inline_tensor(v, name=f"c_{k}") for k, v in consts.items()}

    Copy = mybir.ActivationFunctionType.Copy
    Sig = mybir.ActivationFunctionType.Sigmoid

    # small first group (early first stores = DMA ramps sooner) and small
    # last group (short drain+store tail); 4-image groups in the middle
    if os.environ.get("KRN_GPAT"):
        sizes = [int(v) for v in os.environ["KRN_GPAT"].split(",")]
        assert sum(sizes) == BS
    else:
        sizes = [min(GRP, BS - s) for s in range(0, BS, GRP)]
    groups, s = [], 0
    for sz in sizes:
        groups.append(list(range(s, s + sz)))
        s += sz
    n_groups = len(groups)

    with tile.TileContext(nc) as tc:
        with (
            tc.tile_pool(name="consts", bufs=1) as cp,
            tc.tile_pool(name="xp", bufs=n_groups) as xp,
            tc.tile_pool(name="outp", bufs=BUFO) as outp,
            tc.tile_pool(name="scr", bufs=2) as scr,
            tc.tile_pool(name="gp", bufs=2) as gp,
            tc.tile_pool(name="pm", bufs=2, space="PSUM") as pm,
            tc.tile_pool(name="par", bufs=2, space="PSUM") as par,
        ):
            def cload(key, dtype):
                arr = consts[key]
                t = cp.tile(list(arr.shape), dtype, tag=key)
                nc.sync.dma_start(out=t[:], in_=ch[key][...])
                return t

            wb = cload("wtblob", BF16)      # [128, 400]
            onescol = wb[:, ONES0:ONES0 + 1]

            def load_group(g):
                bs = groups[g]
                b0, gn = bs[0], len(bs)
                xga = xp.tile([128, GRP, D], BF16, tag="xga")
                xgb = xp.tile([68, GRP, D], BF16, tag="xgb")
                nc.sync.dma_start(
                    out=xga[:, 0:gn, :],
                    in_=xs_h[b0:b0 + gn, 1:129, :].rearrange("b t d -> t b d"))
                nc.sync.dma_start(
                    out=xgb[:, 0:gn, :],
                    in_=xs_h[b0:b0 + gn, 129:197, :].rearrange("b t d -> t b d"))
                return xga, xgb

            # group 0's loads go before the remaining consts so its gate
            # chain starts as early as possible; all loads precede all
            # stores so SP's in-order sequencer never parks a semaphore-
            # blocked store ahead of a ready load
            xt = {0: load_group(0)}

            gb = cload("gblob", BF16)       # [128, 1748]
            w1c = gb[:, GW1:GW1 + 1152].rearrange("p (a h b) -> p a h b",
                                                  a=6, h=2)
            w2c0 = gb[0:96, GW20:GW20 + 2]
            w2c1 = gb[0:96, GW21 + 0:GW21 + 2]
            ones1 = gb[0:1, GONES:GONES + 16]
            alr = gb[0:1, GALR:GALR + 128]
            ahr = gb[0:1, GAHR:GAHR + 128]
            cneg = gb[0:1, GCNEG:GCNEG + 128]

            # dummy activation so the act-func table load (~1.3us) runs at
            # t~=1us instead of stalling the first group's gate sigmoid;
            # reads the earliest const so it never blocks Act's queue
            warm = gp.tile([1, 16], F32, tag="warm")
            nc.scalar.activation(warm[:], wb[0:1, 0:16], Sig)

            # CLS passthrough for all batches (DRAM -> DRAM), issued from
            # the near-free GPSIMD DMA queue so SP only handles bulk I/O
            nc.gpsimd.dma_start(out=lo_h[:, 0:1, :], in_=xs_h[:, 0:1, :])
            nc.gpsimd.dma_start(out=hi_h[:, 0:1, :], in_=xs_h[:, 0:1, :])

            for g in range(1, n_groups):
                xt[g] = load_group(g)

            def gate_chain(g, xga, xgb):
                bs = groups[g]
                gn = len(bs)
                # gate for this group, ahead of the main matmuls.
                # gT[d, j] = mean over patch tokens of x (tiny K-contraction
                # matmuls straight into the psum arena, [d, img] layout).
                arena = par.tile([128, 384], F32, tag="arena")
                for j in range(gn):
                    for c in range(6):
                        col = c * 16 + j
                        nc.tensor.matmul(
                            arena[:, col:col + 1],
                            xga[:, j, c * 128:(c + 1) * 128],
                            onescol[:],
                            start=True, stop=False)
                        nc.tensor.matmul(
                            arena[:, col:col + 1],
                            xgb[0:68, j, c * 128:(c + 1) * 128],
                            onescol[0:68],
                            start=False, stop=True)
                gTt = gp.tile([128, 6, 16], BF16, tag="gTt")
                nc.vector.tensor_copy(
                    gTt[:].rearrange("p a b -> p (a b)"), arena[:, 0:96])

                # hidden layer directly in transposed [feature, img] layout
                # (w1 chunks stationary): no transposes, tiny moving dims
                gate_pe = None
                for h in range(2):
                    hps = arena[0:96, 96 + 16 * h:96 + 16 * h + 16]
                    for c in range(6):
                        nc.tensor.matmul(hps[:, 0:gn], w1c[:, c, h, :],
                                         gTt[:, c, 0:gn],
                                         start=(c == 0), stop=False)
                    gate_pe = nc.tensor.matmul(
                        hps[:, 0:gn], gb[0:1, GB1 + 96 * h:GB1 + 96 * h + 96],
                        ones1[0:1, 0:gn], start=False, stop=True)
                hTt = gp.tile([96, 2, 16], BF16, tag="hTt")
                for h in range(2):
                    hps = arena[0:96, 96 + 16 * h:96 + 16 * h + 16]
                    nc.vector.tensor_relu(hTt[:, h, 0:gn], hps[:, 0:gn])

                crows = []
                for col, b2f in ((0, b2lo), (1, b2hi)):
                    g_ps = arena[0:1, 128 + 16 * col:144 + 16 * col]
                    nc.tensor.matmul(g_ps[:, 0:gn], w2c0[:, col:col + 1],
                                     hTt[:, 0, 0:gn], start=True, stop=False)
                    nc.tensor.matmul(g_ps[:, 0:gn], w2c1[:, col:col + 1],
                                     hTt[:, 1, 0:gn], start=False, stop=True)
                    cr = gp.tile([1, 16], BF16, tag=f"crow{col}")
                    nc.scalar.activation(cr[:, 0:gn], g_ps[:, 0:gn], Sig,
                                         bias=b2f)
                    crows.append(cr)
                # per-image hi/lo gate ratio (the hi path is reconstructed
                # from the already-scaled lo tile: hi = crh*x - r*lo with
                # r = crh/crl; the -alpha ratio constant lives in cneg)
                rcp = gp.tile([1, 16], F32, tag="rcp")
                nc.vector.reciprocal(rcp[:, 0:gn], crows[0][0:1, 0:gn])
                rrow = gp.tile([1, 16], BF16, tag="rrow")
                nc.vector.tensor_mul(rrow[:, 0:gn], rcp[:, 0:gn],
                                     crows[1][0:1, 0:gn])
                # replicate the gate rows across partitions; the alpha
                # sigmoid is folded into the alr/ahr/cneg weight rows
                for k, (wrow, mov) in enumerate(
                        ((alr, crows[0][0:1, 0:gn]),
                         (ahr, crows[1][0:1, 0:gn]),
                         (cneg, rrow[0:1, 0:gn]))):
                    nc.tensor.matmul(arena[:, 160 + 16 * k:160 + 16 * k + gn],
                                     wrow[0:1, :], mov, start=True, stop=True)
                crlh = gp.tile([128, 48], F32, tag="crlh")
                nc.vector.tensor_copy(crlh[:], arena[:, 160:208])
                return crlh, gate_pe

            # gates run two groups ahead of their bodies so their small
            # DVE/Act steps never queue behind a full body's engine work
            gates = {g: gate_chain(g, *xt[g]) for g in range(min(2, n_groups))}

            for g, bs in enumerate(groups):
                gn = len(bs)
                b0 = bs[0]
                xga, xgb = xt[g]
                crlh, gate_pe = gates.pop(g)
                crl = crlh[:, 0:16]
                crh = crlh[:, 16:32]
                rneg = crlh[:, 32:48]

                # ---- main matmuls + scaled drains per image
                for j in range(gn):
                    # per-2-image output tiles: a store only waits on its
                    # own half's drains/adds (tile-granular dependency
                    # tracking would otherwise park it behind the whole
                    # group), and dedicated hi tiles release the x tiles to
                    # the pool at the final add rather than at the store
                    jj = j % 2
                    if jj == 0:
                        lo_ga = outp.tile([128, 2, D], BF16, tag="lo_ga")
                        lo_gb = outp.tile([68, 2, D], BF16, tag="lo_gb")
                        ho_ga = outp.tile([128, 2, D], BF16, tag="ho_ga")
                        ho_gb = outp.tile([68, 2, D], BF16, tag="ho_gb")
                    # PSUM packing: 3 banks/image — za_t[128,1024] holds
                    # za(0:768) + zb's tail chunk (768:1024), zb1 the rest.
                    # The a-side then drains in ONE 768-col Act pass; the
                    # two tiles free independently (a-side earlier), which
                    # beats a fully-merged 3-bank tile.
                    za_t = pm.tile([128, 1024], F32, tag="za_t")
                    zb1 = pm.tile([68, 512], F32, tag="zb1")
                    za_ch = [(0, 512, za_t[:, 0:512]),
                             (512, 768, za_t[:, 512:768])]
                    zb_ch = [(0, 512, zb1[0:68, :]),
                             (512, 768, za_t[0:68, 768:1024])]
                    for (n0, n1, zc) in za_ch:
                        mm = nc.tensor.matmul(zc, wb[:, 0:128],
                                              xga[:, j, n0:n1],
                                              start=True, stop=False)
                        add_dep_helper(mm.ins, gate_pe.ins,
                                       reason="gate chain schedules first")
                        nc.tensor.matmul(zc, wb[0:68, WTB0:WTB0 + 128],
                                         xgb[0:68, j, n0:n1],
                                         start=False, stop=True)
                    for (n0, n1, zc) in zb_ch:
                        mm = nc.tensor.matmul(zc, wb[:, 128:196],
                                              xga[:, j, n0:n1],
                                              start=True, stop=False)
                        add_dep_helper(mm.ins, gate_pe.ins,
                                       reason="gate chain schedules first")
                        nc.tensor.matmul(zc, wb[0:68, WTB0 + 128:WTB0 + 196],
                                         xgb[0:68, j, n0:n1],
                                         start=False, stop=True)

                    # GPSIMD cannot touch PSUM, so the legal engine split is:
                    #   Act:  scaled lo drains (+ gate sigmoids)
                    #   DVE:  hi reconstruction + every other b2 drain
                    #   Pool: b-side hi scale (SBUF-only bf16)
                    nc.scalar.activation(lo_ga[:, jj, :], za_t[:, 0:768],
                                         Copy, scale=crl[:, j:j + 1])
                    nc.scalar.activation(lo_gb[0:68, jj, 0:512], zb1[0:68, :],
                                         Copy, scale=crl[0:68, j:j + 1])
                    nc.scalar.activation(lo_gb[0:68, jj, 512:768],
                                         za_t[0:68, 768:1024], Copy,
                                         scale=crl[0:68, j:j + 1])
                    # hi = crh*x - (crh/crl)*lo: reconstructed from the
                    # drained lo tiles, entirely off PSUM (z has a single
                    # reader).  All-bf16-SBUF muls/adds hit DVE 2x/4x modes;
                    # the b-side x-scale rides the otherwise idle GPSIMD.
                    nc.gpsimd.tensor_scalar_mul(xgb[0:68, j, :],
                                                xgb[0:68, j, :],
                                                crh[0:68, j:j + 1])
                    nc.vector.tensor_scalar_mul(xga[:, j, :],
                                                xga[:, j, :],
                                                crh[:, j:j + 1])
                    ta = scr.tile([128, D], BF16, tag="ta")
                    tb = scr.tile([68, D], BF16, tag="tb")
                    nc.vector.tensor_scalar_mul(ta[:], lo_ga[:, jj, :],
                                                rneg[:, j:j + 1])
                    nc.vector.tensor_add(ho_ga[:, jj, :], xga[:, j, :],
                                         ta[:])
                    nc.vector.tensor_scalar_mul(tb[0:68, :],
                                                lo_gb[0:68, jj, :],
                                                rneg[0:68, j:j + 1])
                    nc.vector.tensor_add(ho_gb[0:68, jj, :], xgb[0:68, j, :],
                                         tb[0:68, :])

                    # ship each completed half immediately (stores from SP:
                    # all loads were pre-issued, so the in-order queue never
                    # parks a blocked store ahead of a ready load)
                    if jj == 1 or j == gn - 1:
                        k0 = j - jj
                        c0, c1 = b0 + k0, b0 + j + 1
                        kn = j + 1 - k0
                        nc.sync.dma_start(
                            out=lo_h[c0:c1, 1:129, :].rearrange(
                                "b t d -> t b d"),
                            in_=lo_ga[:, 0:kn, :])
                        nc.sync.dma_start(
                            out=lo_h[c0:c1, 129:197, :].rearrange(
                                "b t d -> t b d"),
                            in_=lo_gb[:, 0:kn, :])
                        nc.sync.dma_start(
                            out=hi_h[c0:c1, 1:129, :].rearrange(
                                "b t d -> t b d"),
                            in_=ho_ga[:, 0:kn, :])
                        nc.sync.dma_start(
                            out=hi_h[c0:c1, 129:197, :].rearrange(
                                "b t d -> t b d"),
                            in_=ho_gb[:, 0:kn, :])

                # next-next group's gate chain goes ahead of this group's
                # last stores (on PE it runs while Act/DVE/Pool finish here)
                if g + 2 < n_groups:
                    gates[g + 2] = gate_chain(g + 2, *xt[g + 2])

    if not nc.is_finalized():
        nc.finalize()
    return nc


def _make_consts(OP, w1, b1, w2, alpha_low, alpha_high):
    import ml_dtypes
    sig = lambda v: 1.0 / (1.0 + np.exp(-np.float64(v)))
    WT = np.ascontiguousarray(np.asarray(OP, np.float64).T)
    wtblob = np.zeros((128, 400), np.float32)
    wtblob[0:128, 0:196] = WT[0:128]
    wtblob[0:68, 196:392] = WT[128:196]
    wtblob[:, ONES0] = 1.0 / P
    gblob = np.zeros((128, 1748), np.float32)
    gblob[:, GW1:GW1 + 1152] = np.asarray(w1, np.float32).reshape(
        6, 128, 192).transpose(1, 0, 2).reshape(128, 1152)
    gblob[0, GB1:GB1 + 192] = np.asarray(b1, np.float32)
    gblob[0:96, GW20:GW20 + 2] = np.asarray(w2, np.float32)[0:96]
    gblob[0:96, GW21:GW21 + 2] = np.asarray(w2, np.float32)[96:192]
    gblob[0, GONES:GONES + 16] = 1.0
    gblob[0, GALR:GALR + 128] = sig(alpha_low)
    gblob[0, GAHR:GAHR + 128] = sig(alpha_high)
    gblob[0, GCNEG:GCNEG + 128] = -sig(alpha_high) / sig(alpha_low)
    return {"wtblob": wtblob.astype(ml_dtypes.bfloat16),
            "gblob": gblob.astype(ml_dtypes.bfloat16)}


def build_for_sim():
    """Program instance for cost-model simulation (dummy weights)."""
    import ml_dtypes
    consts = {
        "wtblob": np.zeros((128, 400), ml_dtypes.bfloat16),
        "gblob": np.ones((128, 1748), ml_dtypes.bfloat16),
    }
    return _build_program(consts, 0.0, 0.0)


def kernel(x, low_params, high_params, alpha_low, alpha_high,
           w1, b1, w2, b2, cls_token_idx):
    import ml_dtypes
    assert int(cls_token_idx) == 0
    x = np.asarray(x, dtype=np.float32)
    assert x.shape == (B, N, D)

    lm = _freq_mask_np(low_params, True)
    A = _conv_operator(lm)
    share_Y = np.allclose(np.asarray(low_params, np.float32),
                          np.asarray(high_params, np.float32))
    b2v = np.asarray(b2, np.float64).reshape(2)

    xbf = np.ascontiguousarray(x.astype(ml_dtypes.bfloat16))
    xs = xbf.reshape(NCORES, BS, N, D)
    in_maps = [{"xs": np.ascontiguousarray(xs[c])} for c in range(NCORES)]

    def run_once(OP):
        consts = _make_consts(OP, w1, b1, w2, alpha_low, alpha_high)
        nc = _build_program(consts, float(b2v[0]), float(b2v[1]))
        res = run_bass_kernel_spmd(nc, in_maps, core_ids=list(range(NCORES)))
        lo = np.concatenate([np.asarray(r["lo"]) for r in res.results],
                            axis=0).astype(np.float32)
        hi = np.concatenate([np.asarray(r["hi"]) for r in res.results],
                            axis=0).astype(np.float32)
        if getattr(res, "exec_time_ns", None) is not None:
            print(f"HW exec time: {res.exec_time_ns} ns")
        return lo, hi

    if share_Y:
        return run_once(A)
    # generic case (not hit by the reference inputs): hi needs its own
    # operator; run the validated single-operator program twice
    lo, _ = run_once(A)
    Cm = _conv_operator(_freq_mask_np(high_params, True))
    _, hi = run_once(Cm)
    return lo, hi
